# revision 1
# baseline (speedup 1.0000x reference)
"""BiLevelGAT (2-branch x 3-layer GATv2, N=50000, E=500000, D=96) on 8 TRN2 cores.

Sharding: nodes + incoming edges partitioned by dst; per-layer AllGather of a
bf16 per-node table [hl_loc 96|1|w_loc|pad30|hl_glob 96|1|w_glob|pad30] (512B
rows) gathered per edge by src.

Math: lrelu(x) = 0.6x+0.4|x| splits the GATv2 logit into linear terms (per-src
w=exp(0.6*att.hl) folded into the softmax weight; per-dst term cancels in
softmax; per-edge ea term precomputed host-side as ln psi) plus 0.4*att.|m|
computed on device. Softmax max-subtraction skipped (logits O(1), fp32 safe).
"""
import sys
sys.path.insert(0, '/opt/trn_rl_repo')
import numpy as np
import ml_dtypes

BF16 = ml_dtypes.bfloat16

N, E, D, EDIM, L, DENSE, OUT = 50000, 500000, 96, 8, 3, 256, 2
NCORES = 8
NLOC = N // NCORES            # 6250
WIN, HALF = 96, 48
NWIN = (NLOC + WIN - 1) // WIN  # 53
NPAD = NWIN * WIN             # 6360
NCH = (NPAD + 127) // 128     # 50 chunks of 128 (PASS A / table)
SPLIT = 32768
TROW = 256

_CACHE = {}


def _host_prep(x, edge_index, edge_attr, weights):
    src = edge_index[0].astype(np.int64)
    dst = edge_index[1].astype(np.int64)
    mean_ea = edge_attr.mean(0).astype(np.float32)
    loop = np.arange(N, dtype=np.int64)
    src_a = np.concatenate([src, loop])
    dst_a = np.concatenate([dst, loop])
    ea_a = np.concatenate([edge_attr.astype(np.float32),
                           np.broadcast_to(mean_ea, (N, EDIM))], 0)

    owner = dst_a // NLOC
    dloc = dst_a - owner * NLOC
    win = dloc // WIN
    half = (dloc % WIN) // HALF
    stream = (src_a >= SPLIT).astype(np.int64)

    per_core = []
    secs = np.zeros((NCORES, NWIN, 2, 2), np.int64)
    for c in range(NCORES):
        m = owner == c
        s_c, d_c, e_c = src_a[m], dloc[m], ea_a[m]
        w_c, h_c, st_c = win[m], half[m], stream[m]
        sec = ((w_c * 2 + h_c) * 2 + st_c)
        order = np.argsort(sec * NLOC + d_c, kind='stable')
        s_c, d_c, e_c, sec = s_c[order], d_c[order], e_c[order], sec[order]
        st_c = st_c[order]
        per_core.append((s_c, d_c, e_c, sec, st_c))
        secs[c] = np.bincount(sec, minlength=NWIN * 4).reshape(NWIN, 2, 2)

    K = np.maximum((secs.max(0) + 127) // 128, 1)       # [NWIN, 2, 2]
    Kf = K.reshape(-1)
    sec_slot = np.zeros(NWIN * 4 + 1, np.int64)
    np.cumsum(Kf * 128, out=sec_slot[1:])
    NSLOT = int(sec_slot[-1])

    gidx = np.zeros((NCORES, NSLOT), np.int16)
    dcol = np.zeros((NCORES, NSLOT), np.float32)
    psi = np.zeros((NCORES, NSLOT, 2 * L), np.float32)  # ea.(We@att), pads get -big later
    valid = np.zeros((NCORES, NSLOT), bool)
    R = np.zeros((NCORES, 128, NSLOT), np.float32)

    for c in range(NCORES):
        s_c, d_c, e_c, sec, st_c = per_core[c]
        counts = np.bincount(sec, minlength=NWIN * 4)
        starts = np.concatenate([[0], np.cumsum(counts)])[:-1]
        pos = np.arange(len(s_c)) - starts[sec]
        slot = sec_slot[sec] + pos
        gidx[c, slot] = (s_c - st_c * SPLIT).astype(np.int16)
        dcol[c, slot] = (d_c % HALF).astype(np.float32)
        valid[c, slot] = True
        for l in range(L):
            for b, p in enumerate(['local', 'global']):
                v = weights[f'{p}_We'][l] @ weights[f'{p}_att'][l]
                psi[c, slot, 2 * l + b] = e_c @ v
        R[c, d_c % WIN, slot] = 1.0
        for d8 in range(8):
            R[c, 96 + d8, slot] = e_c[:, d8]

    # psi -> ln weights, edge-major per section: [L, 128, NSEC*16], col si*16+2j+b
    NSEC = NWIN * 4
    psiln = np.full((NCORES, L, 128, NSEC * 16), -100.0, np.float32)
    for c in range(NCORES):
        for si in range(NSEC):
            k = int(Kf[si])
            sl0 = int(sec_slot[si])
            for j in range(k):
                seg = slice(sl0 + j * 128, sl0 + (j + 1) * 128)
                v = valid[c, seg]
                for l in range(L):
                    for b in range(2):
                        col = np.full(128, -100.0, np.float32)
                        col[v] = 0.6 * psi[c, seg, 2 * l + b][v]
                        psiln[c, l, :, si * 16 + 2 * j + b] = col

    gw = np.zeros((NCORES, 128, NSLOT // 16), np.int16)
    for c in range(NCORES):
        w = gidx[c].reshape(-1, 16).T
        for g in range(8):
            gw[c, g * 16:(g + 1) * 16] = w
    dcol_t = dcol.reshape(NCORES, -1, 128).transpose(0, 2, 1).copy()  # [128, NB]
    iotaf = np.broadcast_to(np.arange(HALF, dtype=np.float32), (128, HALF)).copy()

    return dict(K=K, Kf=Kf, sec_slot=sec_slot, NSLOT=NSLOT, NSEC=NSEC,
                gw=gw, dcol=dcol_t, R=R.astype(BF16), psiln=psiln.astype(BF16),
                iotaf=iotaf, mean_ea=mean_ea)


def _wpack(w):
    o = {}
    for l in range(L):
        for b, p in enumerate(['local', 'global']):
            o[f'Wl_{l}_{b}'] = np.ascontiguousarray(w[f'{p}_Wl'][l], np.float32)
            o[f'Wr_{l}_{b}'] = np.ascontiguousarray(w[f'{p}_Wr'][l], np.float32)
            o[f'att_{l}_{b}'] = np.ascontiguousarray(w[f'{p}_att'][l].reshape(96, 1), np.float32)
            o[f'bb_{l}_{b}'] = np.ascontiguousarray(w[f'{p}_b'][l].reshape(96, 1), np.float32)
            o[f'We_{l}_{b}'] = np.ascontiguousarray(w[f'{p}_We'][l], np.float32).astype(BF16)
    o['fusion_Wt'] = np.ascontiguousarray(w['fusion_W'][:96], np.float32)
    o['fusion_Wb'] = np.ascontiguousarray(w['fusion_W'][96:], np.float32)
    o['fusion_b'] = np.ascontiguousarray(w['fusion_b'].reshape(96, 1), np.float32)
    o['pred_W1a'] = np.ascontiguousarray(w['pred_W1'][:, :128], np.float32)
    o['pred_W1b'] = np.ascontiguousarray(w['pred_W1'][:, 128:], np.float32)
    o['pred_b1a'] = np.ascontiguousarray(w['pred_b1'][:128].reshape(128, 1), np.float32)
    o['pred_b1b'] = np.ascontiguousarray(w['pred_b1'][128:].reshape(128, 1), np.float32)
    o['pred_W2a'] = np.ascontiguousarray(w['pred_W2'][:128], np.float32)
    o['pred_W2b'] = np.ascontiguousarray(w['pred_W2'][128:], np.float32)
    o['pred_b2'] = np.broadcast_to(w['pred_b2'].reshape(1, 2), (128, 2)).astype(np.float32).copy()
    return o


WSHAPES = {}
for _l in range(L):
    for _b in range(2):
        WSHAPES[f'Wl_{_l}_{_b}'] = [96, 96]
        WSHAPES[f'Wr_{_l}_{_b}'] = [96, 96]
        WSHAPES[f'att_{_l}_{_b}'] = [96, 1]
        WSHAPES[f'bb_{_l}_{_b}'] = [96, 1]
        WSHAPES[f'We_{_l}_{_b}'] = [8, 96]
WSHAPES.update({'fusion_Wt': [96, 96], 'fusion_Wb': [96, 96], 'fusion_b': [96, 1],
                'pred_W1a': [96, 128], 'pred_W1b': [96, 128],
                'pred_b1a': [128, 1], 'pred_b1b': [128, 1],
                'pred_W2a': [128, 2], 'pred_W2b': [128, 2], 'pred_b2': [128, 2]})


def build_kernel(pp):
    import os as _os
    SKIP_EDGE = _os.environ.get('SKIP_EDGE', '0') == '1'
    SKIP_GATHER = _os.environ.get('SKIP_GATHER', '0') == '1'
    from concourse import mybir, bacc
    import concourse.tile as tile
    Kf, sec_slot, NSLOT, NSEC = pp['Kf'], pp['sec_slot'], pp['NSLOT'], pp['NSEC']
    f32, bf16, i16 = mybir.dt.float32, mybir.dt.bfloat16, mybir.dt.int16
    AF = mybir.ActivationFunctionType
    OP = mybir.AluOpType

    nc = bacc.Bacc("TRN2", target_bir_lowering=False, debug=False, num_devices=NCORES)
    dx = nc.dram_tensor("x", [NLOC, D], f32, kind="ExternalInput")
    dR = nc.dram_tensor("R", [128, NSLOT], bf16, kind="ExternalInput")
    dgw = nc.dram_tensor("gw", [128, NSLOT // 16], i16, kind="ExternalInput")
    ddc = nc.dram_tensor("dcol", [128, NSLOT // 128], f32, kind="ExternalInput")
    dpsi = nc.dram_tensor("psiln", [L, 128, NSEC * 16], bf16, kind="ExternalInput")
    diota = nc.dram_tensor("iotaf", [128, HALF], f32, kind="ExternalInput")
    dw = {k: nc.dram_tensor(k, shp, bf16 if k.startswith('We') else f32,
                            kind="ExternalInput") for k, shp in WSHAPES.items()}
    dout = nc.dram_tensor("out", [NLOC, OUT], f32, kind="ExternalOutput")

    tab_slice = nc.dram_tensor("tab_slice", [NLOC, TROW], bf16)
    tab_sh = nc.dram_tensor("tab_sh", [N, TROW], bf16, addr_space="Shared")
    tab = nc.dram_tensor("tab", [N, TROW], bf16)

    with tile.TileContext(nc) as tc:
      with (tc.tile_pool(name="const", bufs=1) as cp,
            tc.tile_pool(name="hp", bufs=1) as hp,
            tc.tile_pool(name="wp", bufs=1) as wp,
            tc.tile_pool(name="sp", bufs=3) as sp,
            tc.tile_pool(name="gpool", bufs=2) as gpl,
            tc.tile_pool(name="ps", bufs=2, space="PSUM") as psp,
            tc.tile_pool(name="psA", bufs=2, space="PSUM") as psA,
            tc.tile_pool(name="psagg", bufs=1, space="PSUM") as psG):

        ident = cp.tile([128, 128], bf16)
        nc.sync.dma_start(out=ident[:], in_=nc.inline_tensor(np.eye(128, dtype=BF16), name="idb").ap())
        identf = cp.tile([128, 128], f32)
        nc.sync.dma_start(out=identf[:], in_=nc.inline_tensor(np.eye(128, dtype=np.float32), name="idf").ap())
        iota_t = cp.tile([128, HALF], f32)
        nc.sync.dma_start(out=iota_t[:], in_=diota[:])
        gw_t = cp.tile([128, NSLOT // 16], i16)
        nc.sync.dma_start(out=gw_t[:], in_=dgw[:])
        dc_t = cp.tile([128, NSLOT // 128], f32)
        nc.sync.dma_start(out=dc_t[:], in_=ddc[:])
        wt = {}
        for k, t in dw.items():
            wt[k] = cp.tile(list(t.shape), bf16 if k.startswith('We') else f32, tag=k, name=k)
            nc.sync.dma_start(out=wt[k][:], in_=t.ap())
        one1 = cp.tile([1, 96], f32)
        nc.vector.memset(one1[:], 1.0)
        att04 = {}
        for l in range(L):
            for b in range(2):
                att04[(l, b)] = cp.tile([96, 1], bf16, tag=f"att04_{l}_{b}", name=f"att04_{l}_{b}")
                nc.vector.tensor_scalar(out=att04[(l, b)][:], in0=wt[f'att_{l}_{b}'][:],
                                        scalar1=0.4, scalar2=None, op0=OP.mult)

        # h_T feature-major [96, NPAD] (cols beyond NLOC are pad)
        h_T = [hp.tile([96, NCH * 128], f32, tag=f"h{b}", name=f"h{b}") for b in range(2)]
        for ch in range(NCH):
            n0 = ch * 128
            nreal = max(0, min(NLOC - n0, 128))
            xin = sp.tile([128, 128], f32, tag="xin")
            nc.vector.memset(xin[:], 0.0)
            if nreal > 0:
                nc.sync.dma_start(out=xin[:nreal, :96], in_=dx[n0:n0 + nreal, :])
            pt = psA.tile([128, 128], f32, tag="pbig")
            nc.tensor.transpose(out=pt[:], in_=xin[:], identity=identf[:])
            for b in range(2):
                nc.vector.tensor_copy(out=h_T[b][:, n0:n0 + 128], in_=pt[:96, :])

        hw_T = [wp.tile([96, NCH * 128], f32, tag=f"hw{b}", name=f"hw{b}") for b in range(2)]

        for l in range(L):
            # ---------- PASS A ----------
            for b in range(2):
                for cs in range(0, NCH * 128, 512):
                    ce = min(cs + 512, NCH * 128)
                    w_ = ce - cs
                    pl = psA.tile([96, 512], f32, tag="pbig")
                    nc.tensor.matmul(out=pl[:, :w_], lhsT=wt[f'Wl_{l}_{b}'][:],
                                     rhs=h_T[b][:, cs:ce], start=True, stop=True)
                    nc.vector.tensor_copy(out=hw_T[b][:, cs:ce], in_=pl[:, :w_])
            # table slice + allgather
            for ch in range(NCH):
                n0 = ch * 128
                nreal = max(0, min(NLOC - n0, 128))
                if nreal == 0:
                    continue
                stg = sp.tile([128, TROW], bf16, tag="stg")
                nc.vector.memset(stg[:], 0.0)
                for b in range(2):
                    pt = psA.tile([128, 128], f32, tag="pbig")
                    nc.tensor.transpose(out=pt[:, :96], in_=hw_T[b][:, n0:n0 + 128],
                                        identity=identf[:96, :96])
                    nc.vector.tensor_copy(out=stg[:, b * 128:b * 128 + 96], in_=pt[:, :96])
                    # w = exp(0.6*att.hl) for this chunk; ones at ext row 32
                    pphi = psA.tile([1, 128], f32, tag="pbig")
                    nc.tensor.matmul(out=pphi[:], lhsT=wt[f'att_{l}_{b}'][:],
                                     rhs=hw_T[b][:, n0:n0 + 128], start=True, stop=True)
                    ext = sp.tile([64, 128], f32, tag="ext")
                    nc.scalar.activation(out=ext[0:1, :], in_=pphi[:], func=AF.Exp, scale=0.6)
                    nc.vector.memset(ext[32:33, :], 1.0)
                    pt2 = psA.tile([128, 64], f32, tag="pbig")
                    nc.tensor.transpose(out=pt2[:], in_=ext[:], identity=identf[:64, :64])
                    nc.vector.tensor_copy(out=stg[:, b * 128 + 96:b * 128 + 97], in_=pt2[:, 32:33])
                    nc.vector.tensor_copy(out=stg[:, b * 128 + 97:b * 128 + 98], in_=pt2[:, 0:1])
                nc.vector.tensor_copy(out=stg[:, 98:99], in_=stg[:, 225:226])
                nc.sync.dma_start(out=tab_slice[n0:n0 + nreal, :], in_=stg[:nreal, :])
            nc.gpsimd.collective_compute(
                "AllGather", mybir.AluOpType.bypass,
                replica_groups=[list(range(NCORES))],
                ins=[tab_slice[:]], outs=[tab_sh[:]],
            )
            nc.sync.dma_start(out=tab[:], in_=tab_sh[:])

            # ---------- edge phase ----------
            for w in range(0 if not SKIP_EDGE else NWIN, NWIN):
                aggp = {}
                first = {b: True for b in range(2)}
                nagg = {b: 0 for b in range(2)}
                tot = {b: sum(int(Kf[(w * 2 + h) * 2 + s]) for h in range(2) for s in range(2))
                       for b in range(2)}
                for b in range(2):
                    aggp[b] = psG.tile([97, WIN], f32, tag=f"agg{b}", name=f"agg{b}")
                # base lhsT per branch for this window (hr = h @ Wr computed here)
                basel = {}
                for b in range(2):
                    phr = psA.tile([96, WIN], f32, tag="pbig")
                    nc.tensor.matmul(out=phr[:], lhsT=wt[f'Wr_{l}_{b}'][:],
                                     rhs=h_T[b][:, w * WIN:(w + 1) * WIN],
                                     start=True, stop=True)
                    hrs = sp.tile([96, WIN], f32, tag="hrs")
                    nc.vector.tensor_copy(out=hrs[:], in_=phr[:])
                    pt = psA.tile([WIN, 96], f32, tag="pbig")
                    nc.tensor.transpose(out=pt[:], in_=hrs[:], identity=identf[:96, :96])
                    bl = sp.tile([128, 96], bf16, tag=f"basel{b}", name=f"basel{b}")
                    nc.vector.memset(bl[:], 0.0)
                    nc.vector.tensor_copy(out=bl[:WIN, :], in_=pt[:])
                    nc.vector.tensor_copy(out=bl[WIN:WIN + 8, :], in_=wt[f'We_{l}_{b}'][:])
                    basel[b] = bl
                for h in range(2):
                    for s in range(2):
                        si = (w * 2 + h) * 2 + s
                        Ks = int(Kf[si])
                        sl0 = int(sec_slot[si])
                        nsl = Ks * 128
                        g = gpl.tile([128, 6, TROW], bf16, tag="gath")
                        if SKIP_GATHER:
                            nc.vector.memset(g[:, :Ks, :], 0.0)
                        else:
                            nc.gpsimd.dma_gather(
                                out_ap=g[:, :Ks, :],
                                in_ap=tab[SPLIT:, :] if s else tab[:SPLIT, :],
                                idxs_ap=gw_t[:, sl0 // 16:(sl0 + nsl) // 16],
                                num_idxs=nsl, num_idxs_reg=nsl, elem_size=TROW)
                        Rt = sp.tile([128, 6 * 128], bf16, tag="Rt")
                        nc.sync.dma_start(out=Rt[:, :nsl], in_=dR[:, sl0:sl0 + nsl])
                        pst = sp.tile([128, 16], bf16, tag="pst")
                        nc.sync.dma_start(out=pst[:, :2 * Ks],
                                          in_=dpsi[l, :, si * 16:si * 16 + 2 * Ks])
                        lgp = psp.tile([128, 16], f32, tag="lgp", bufs=1)
                        for j0 in range(0, Ks, 4):
                            jw = min(4, Ks - j0)
                            for b in range(2):
                                mps = psp.tile([96, 512], f32, tag="mps")
                                nc.tensor.matmul(out=mps[:, :jw * 128], lhsT=basel[b][:],
                                                 rhs=Rt[:, j0 * 128:(j0 + jw) * 128],
                                                 start=True, stop=False)
                                for dj in range(jw):
                                    j = j0 + dj
                                    nc.tensor.matmul(out=mps[:, dj * 128:(dj + 1) * 128],
                                                     lhsT=g[:, j, b * 128:b * 128 + 96],
                                                     rhs=ident[:], start=False,
                                                     stop=(dj == jw - 1),
                                                     skip_group_check=True)
                                am = sp.tile([96, 512], bf16, tag="am")
                                nc.scalar.activation(out=am[:, :jw * 128],
                                                     in_=mps[:, :jw * 128], func=AF.Abs)
                                for dj in range(jw):
                                    j = j0 + dj
                                    nc.tensor.matmul(out=lgp[:, 2 * j + b:2 * j + b + 1],
                                                     lhsT=am[:, dj * 128:(dj + 1) * 128],
                                                     rhs=att04[(l, b)][:],
                                                     start=(j == 0 and b == 0), stop=False,
                                                     skip_group_check=True)
                        nc.tensor.matmul(out=lgp[:, :2 * Ks], lhsT=ident[:],
                                         rhs=pst[:, :2 * Ks], start=False, stop=True,
                                         skip_group_check=True)
                        exw = sp.tile([128, 16], f32, tag="exw")
                        nc.scalar.activation(out=exw[:, :2 * Ks], in_=lgp[:, :2 * Ks],
                                             func=AF.Exp)
                        nc.vector.tensor_tensor(
                            out=exw[:, :2 * Ks].rearrange("p (j b) -> p j b", b=2),
                            in0=exw[:, :2 * Ks].rearrange("p (j b) -> p j b", b=2),
                            in1=g[:, :Ks, 97:99], op=OP.mult)
                        for j in range(Ks):
                            blk = sl0 // 128 + j
                            for b in range(2):
                                es = sp.tile([128, HALF], bf16, tag="es")
                                nc.vector.tensor_scalar(
                                    out=es[:], in0=iota_t[:], scalar1=dc_t[:, blk:blk + 1],
                                    scalar2=exw[:, 2 * j + b:2 * j + b + 1],
                                    op0=OP.is_equal, op1=OP.mult)
                                nagg[b] += 1
                                nc.tensor.matmul(out=aggp[b][:, h * HALF:(h + 1) * HALF],
                                                 lhsT=g[:, j, b * 128:b * 128 + 97],
                                                 rhs=es[:],
                                                 start=first[b], stop=(nagg[b] == tot[b]),
                                                 skip_group_check=True)
                                first[b] = False
                # finalize window -> h_T
                for b in range(2):
                    num = sp.tile([96, WIN], f32, tag="num")
                    den = sp.tile([1, WIN], f32, tag="den")
                    nc.vector.tensor_copy(out=num[:], in_=aggp[b][:96, :])
                    nc.vector.tensor_scalar(out=den[:], in0=aggp[b][96:97, :],
                                            scalar1=1e-30, scalar2=None, op0=OP.add)
                    rec = sp.tile([1, WIN], f32, tag="rec")
                    nc.vector.reciprocal(out=rec[:], in_=den[:])
                    pb = psp.tile([96, WIN], f32, tag="mps")
                    nc.tensor.matmul(out=pb[:], lhsT=one1[:], rhs=rec[:], start=True, stop=True)
                    tdiv = sp.tile([96, WIN], f32, tag="tdiv")
                    nc.vector.tensor_tensor(out=tdiv[:], in0=num[:], in1=pb[:], op=OP.mult)
                    lin = sp.tile([96, WIN], f32, tag="lin")
                    nc.scalar.activation(out=lin[:], in_=tdiv[:], func=AF.Identity,
                                         bias=wt[f'bb_{l}_{b}'][:])
                    ab = sp.tile([96, WIN], f32, tag="ab")
                    nc.scalar.activation(out=ab[:], in_=tdiv[:], func=AF.Abs,
                                         bias=wt[f'bb_{l}_{b}'][:])
                    nc.vector.tensor_scalar(out=lin[:], in0=lin[:], scalar1=0.505,
                                            scalar2=None, op0=OP.mult)
                    nc.vector.tensor_scalar(out=ab[:], in0=ab[:], scalar1=0.495,
                                            scalar2=None, op0=OP.mult)
                    nc.vector.tensor_tensor(out=h_T[b][:, w * WIN:(w + 1) * WIN],
                                            in0=lin[:], in1=ab[:], op=OP.add)

        # ---------- head ----------
        hid_T = [wp.tile([128, NCH * 128], f32, tag=f"hw{p}", name=f"hid{p}") for p in range(2)]
        for cs in range(0, NCH * 128, 512):
            ce = min(cs + 512, NCH * 128)
            w_ = ce - cs
            pf = psA.tile([96, 512], f32, tag="pbig")
            nc.tensor.matmul(out=pf[:, :w_], lhsT=wt['fusion_Wt'][:],
                             rhs=h_T[0][:, cs:ce], start=True, stop=False)
            nc.tensor.matmul(out=pf[:, :w_], lhsT=wt['fusion_Wb'][:],
                             rhs=h_T[1][:, cs:ce], start=False, stop=True)
            fus = sp.tile([96, 512], f32, tag="fus")
            lin = sp.tile([96, 512], f32, tag="flin")
            nc.scalar.activation(out=lin[:, :w_], in_=pf[:, :w_], func=AF.Identity,
                                 bias=wt['fusion_b'][:])
            ab = sp.tile([96, 512], f32, tag="fab")
            nc.scalar.activation(out=ab[:, :w_], in_=pf[:, :w_], func=AF.Abs,
                                 bias=wt['fusion_b'][:])
            nc.vector.tensor_scalar(out=lin[:, :w_], in0=lin[:, :w_], scalar1=0.505,
                                    scalar2=None, op0=OP.mult)
            nc.vector.tensor_scalar(out=ab[:, :w_], in0=ab[:, :w_], scalar1=0.495,
                                    scalar2=None, op0=OP.mult)
            nc.vector.tensor_tensor(out=fus[:, :w_], in0=lin[:, :w_], in1=ab[:, :w_],
                                    op=OP.add)
            for p, (wk, bk) in enumerate([('pred_W1a', 'pred_b1a'), ('pred_W1b', 'pred_b1b')]):
                ph = psA.tile([128, 512], f32, tag="pbig")
                nc.tensor.matmul(out=ph[:, :w_], lhsT=wt[wk][:], rhs=fus[:, :w_],
                                 start=True, stop=True)
                l2 = sp.tile([128, 512], f32, tag=f"l2{p}")
                a2 = sp.tile([128, 512], f32, tag=f"a2{p}")
                nc.scalar.activation(out=l2[:, :w_], in_=ph[:, :w_], func=AF.Identity,
                                     bias=wt[bk][:])
                nc.scalar.activation(out=a2[:, :w_], in_=ph[:, :w_], func=AF.Abs,
                                     bias=wt[bk][:])
                nc.vector.tensor_scalar(out=l2[:, :w_], in0=l2[:, :w_], scalar1=0.505,
                                        scalar2=None, op0=OP.mult)
                nc.vector.tensor_scalar(out=a2[:, :w_], in0=a2[:, :w_], scalar1=0.495,
                                        scalar2=None, op0=OP.mult)
                nc.vector.tensor_tensor(out=hid_T[p][:, cs:ce], in0=l2[:, :w_],
                                        in1=a2[:, :w_], op=OP.add)
        for ch in range(NCH):
            n0 = ch * 128
            nreal = max(0, min(NLOC - n0, 128))
            if nreal == 0:
                continue
            po = psp.tile([128, 2], f32, tag="mps")
            nc.tensor.matmul(out=po[:], lhsT=hid_T[0][:, n0:n0 + 128],
                             rhs=wt['pred_W2a'][:], start=True, stop=False)
            nc.tensor.matmul(out=po[:], lhsT=hid_T[1][:, n0:n0 + 128],
                             rhs=wt['pred_W2b'][:], start=False, stop=True)
            ot = sp.tile([128, 2], f32, tag="ot")
            nc.vector.tensor_tensor(out=ot[:], in0=po[:], in1=wt['pred_b2'][:], op=OP.add)
            nc.sync.dma_start(out=dout[n0:n0 + nreal, :], in_=ot[:nreal, :])

    nc.compile()
    return nc


def kernel(**inputs):
    from concourse import bass_utils
    x = np.asarray(inputs['x'], np.float32)
    ei = np.asarray(inputs['edge_index'])
    ea = np.asarray(inputs['edge_attr'], np.float32)
    pp = _host_prep(x, ei, ea, inputs)
    if 'nc' not in _CACHE or _CACHE.get('NSLOT') != pp['NSLOT']:
        _CACHE['nc'] = build_kernel(pp)
        _CACHE['NSLOT'] = pp['NSLOT']
    nc = _CACHE['nc']
    wpk = _wpack(inputs)
    in_maps = []
    for c in range(NCORES):
        m = {'x': np.ascontiguousarray(x[c * NLOC:(c + 1) * NLOC]),
             'R': np.ascontiguousarray(pp['R'][c]),
             'gw': np.ascontiguousarray(pp['gw'][c]),
             'dcol': np.ascontiguousarray(pp['dcol'][c]),
             'psiln': np.ascontiguousarray(pp['psiln'][c]),
             'iotaf': pp['iotaf']}
        m.update(wpk)
        in_maps.append(m)
    res = bass_utils.run_bass_kernel_spmd(nc, in_maps, core_ids=list(range(NCORES)))
    out = np.concatenate([res.results[c]['out'] for c in range(NCORES)], axis=0)
    return out.astype(np.float32)



# revision 6
# speedup vs baseline: 12.0659x; 12.0659x over previous
"""BiLevelGAT (2-branch x 3-layer GATv2, N=50000, E=500000, D=96) on 8 TRN2 cores.

Sharding: nodes + incoming edges partitioned by dst; per-layer AllGather of a
bf16 per-node table [hl_loc 96|1|w_loc|w_glob|pad29|hl_glob 96|1|w_glob|pad30]
(512B rows) gathered per edge by src.

Math: lrelu(x) = 0.6x+0.4|x| splits the GATv2 logit into linear terms (per-src
w=exp(0.6*att.hl) folded into the softmax weight; per-dst term cancels in
softmax; per-edge ea term computed on device from a compact [9, NSLOT] edge
table: rows 0-7 ea, row 8 dst-col) plus 0.4*att.|m| computed on device.
Softmax max-subtraction skipped (logits O(1), fp32 safe).

The per-edge one-hot "R" matrix (dst selector + ea rows used as matmul rhs to
form m = hr[dst] + ea@We) is built on device per section: a rank-1 broadcast
matmul of the dst-col row followed by an is_equal against a partition iota.
Inputs per core are ~5MB (x bf16, edge table bf16, gather idx, dst cols,
one packed weight blob) so the host->device transfer over the axon tunnel
stays small; the jitted shard_map wrapper is cached across calls.
"""
import sys
sys.path.insert(0, '/opt/trn_rl_repo')
import numpy as np
import ml_dtypes

BF16 = ml_dtypes.bfloat16

N, E, D, EDIM, L, DENSE, OUT = 50000, 500000, 96, 8, 3, 256, 2
NCORES = 8
NLOC = N // NCORES            # 6250
WIN, HALF = 96, 48
NWIN = (NLOC + WIN - 1) // WIN  # 66
NPAD = NWIN * WIN
NCH = (NPAD + 127) // 128     # chunks of 128 (PASS A / table)
SPLIT = 32768
TROW = 256
KMAX = 6

_CACHE = {}

# ---- packed weight blob layout (cols in a [128, WB] f32 tensor) ----
WOFF = {}
_col = 0
def _alloc(name, width):
    global _col
    WOFF[name] = _col
    _col += width
for _l in range(L):
    for _b in range(2):
        _alloc(f'Wl_{_l}_{_b}', 96)
        _alloc(f'Wr_{_l}_{_b}', 96)
        _alloc(f'att_{_l}_{_b}', 1)
        _alloc(f'bb_{_l}_{_b}', 1)
        _alloc(f'We_{_l}_{_b}', 96)   # rows 0-7
    _alloc(f'V_{_l}', 2)              # rows 0-7: 0.6*(We@att) per branch
_alloc('fusion_Wt', 96)
_alloc('fusion_Wb', 96)
_alloc('fusion_b', 1)
_alloc('pred_W1a', 128)
_alloc('pred_W1b', 128)
_alloc('pred_b1a', 1)
_alloc('pred_b1b', 1)
_alloc('pred_W2a', 2)
_alloc('pred_W2b', 2)
_alloc('pred_b2', 2)
_alloc('iotaf', 48)                   # all 128 rows = arange(48)
_alloc('iota96', 1)                   # rows 0-95 = arange(96)
WB = _col


def _host_prep(x, edge_index, edge_attr):
    src = edge_index[0].astype(np.int64)
    dst = edge_index[1].astype(np.int64)
    mean_ea = edge_attr.mean(0).astype(np.float32)
    loop = np.arange(N, dtype=np.int64)
    src_a = np.concatenate([src, loop])
    dst_a = np.concatenate([dst, loop])
    ea_a = np.concatenate([edge_attr.astype(np.float32),
                           np.broadcast_to(mean_ea, (N, EDIM))], 0)

    owner = dst_a // NLOC
    dloc = dst_a - owner * NLOC
    win = dloc // WIN
    half = (dloc % WIN) // HALF
    stream = (src_a >= SPLIT).astype(np.int64)

    per_core = []
    secs = np.zeros((NCORES, NWIN, 2, 2), np.int64)
    for c in range(NCORES):
        m = owner == c
        s_c, d_c, e_c = src_a[m], dloc[m], ea_a[m]
        w_c, h_c, st_c = win[m], half[m], stream[m]
        sec = ((w_c * 2 + h_c) * 2 + st_c)
        order = np.argsort(sec * NLOC + d_c, kind='stable')
        s_c, d_c, e_c, sec = s_c[order], d_c[order], e_c[order], sec[order]
        st_c = st_c[order]
        per_core.append((s_c, d_c, e_c, sec, st_c))
        secs[c] = np.bincount(sec, minlength=NWIN * 4).reshape(NWIN, 2, 2)

    K = np.maximum((secs.max(0) + 127) // 128, 1)       # [NWIN, 2, 2]
    assert K.max() <= KMAX
    Kf = K.reshape(-1)
    sec_slot = np.zeros(NWIN * 4 + 1, np.int64)
    np.cumsum(Kf * 128, out=sec_slot[1:])
    NSLOT = int(sec_slot[-1])

    gidx = np.zeros((NCORES, NSLOT), np.int16)
    # invalid slots: dcol=99 -> es one-hot misses all cols -> no contribution
    dcol = np.full((NCORES, NSLOT), 99.0, np.float32)
    eav = np.zeros((NCORES, 9, NSLOT), np.float32)

    for c in range(NCORES):
        s_c, d_c, e_c, sec, st_c = per_core[c]
        counts = np.bincount(sec, minlength=NWIN * 4)
        starts = np.concatenate([[0], np.cumsum(counts)])[:-1]
        pos = np.arange(len(s_c)) - starts[sec]
        slot = sec_slot[sec] + pos
        gidx[c, slot] = (s_c - st_c * SPLIT).astype(np.int16)
        dcol[c, slot] = (d_c % HALF).astype(np.float32)
        eav[c, 0:8, slot] = e_c  # advanced idx puts slot axis first: (nedge, 8)
        eav[c, 8, slot] = (d_c % WIN).astype(np.float32)

    gw = np.ascontiguousarray(
        gidx.reshape(NCORES, -1, 16).transpose(0, 2, 1))     # [NCORES, 16, NSLOT//16]
    dcol_t = dcol.reshape(NCORES, -1, 128).transpose(0, 2, 1).copy()  # [128, NB]

    return dict(K=K, Kf=Kf, sec_slot=sec_slot, NSLOT=NSLOT, NSEC=NWIN * 4,
                gw=gw, dcol=dcol_t, eav=eav.astype(BF16))


def _wblob(w):
    blob = np.zeros((128, WB), np.float32)
    def put(name, arr, rows=None):
        a = np.asarray(arr, np.float32)
        if a.ndim == 1:
            a = a.reshape(-1, 1)
        r = a.shape[0] if rows is None else rows
        blob[:r, WOFF[name]:WOFF[name] + a.shape[1]] = a
    for l in range(L):
        V = np.zeros((8, 2), np.float32)
        for b, p in enumerate(['local', 'global']):
            put(f'Wl_{l}_{b}', w[f'{p}_Wl'][l])
            put(f'Wr_{l}_{b}', w[f'{p}_Wr'][l])
            put(f'att_{l}_{b}', w[f'{p}_att'][l])
            put(f'bb_{l}_{b}', w[f'{p}_b'][l])
            put(f'We_{l}_{b}', w[f'{p}_We'][l])
            V[:, b] = 0.6 * (np.asarray(w[f'{p}_We'][l], np.float32)
                             @ np.asarray(w[f'{p}_att'][l], np.float32))
        put(f'V_{l}', V)
    put('fusion_Wt', w['fusion_W'][:96])
    put('fusion_Wb', w['fusion_W'][96:])
    put('fusion_b', w['fusion_b'])
    put('pred_W1a', w['pred_W1'][:, :128])
    put('pred_W1b', w['pred_W1'][:, 128:])
    put('pred_b1a', w['pred_b1'][:128])
    put('pred_b1b', w['pred_b1'][128:])
    put('pred_W2a', w['pred_W2'][:128])
    put('pred_W2b', w['pred_W2'][128:])
    put('pred_b2', np.broadcast_to(np.asarray(w['pred_b2']).reshape(1, 2), (128, 2)))
    put('iotaf', np.broadcast_to(np.arange(48, dtype=np.float32), (128, 48)))
    put('iota96', np.arange(96, dtype=np.float32))
    return blob


def build_kernel(pp):
    import os as _os
    SKIP_EDGE = _os.environ.get('SKIP_EDGE', '0') == '1'
    SKIP_GATHER = _os.environ.get('SKIP_GATHER', '0') == '1'
    from concourse import mybir, bacc
    import concourse.tile as tile
    Kf, sec_slot, NSLOT = pp['Kf'], pp['sec_slot'], pp['NSLOT']
    f32, bf16, i16 = mybir.dt.float32, mybir.dt.bfloat16, mybir.dt.int16
    AF = mybir.ActivationFunctionType
    OP = mybir.AluOpType

    nc = bacc.Bacc("TRN2", target_bir_lowering=False, debug=False, num_devices=NCORES)
    dx = nc.dram_tensor("x", [NLOC, D], bf16, kind="ExternalInput")
    deav = nc.dram_tensor("eav", [9, NSLOT], bf16, kind="ExternalInput")
    dgw = nc.dram_tensor("gw", [16, NSLOT // 16], i16, kind="ExternalInput")
    ddc = nc.dram_tensor("dcol", [128, NSLOT // 128], f32, kind="ExternalInput")
    dwb = nc.dram_tensor("wb", [128, WB], f32, kind="ExternalInput")
    dout = nc.dram_tensor("out", [NLOC, OUT], f32, kind="ExternalOutput")

    tab_slice = nc.dram_tensor("tab_slice", [NLOC, TROW], bf16)
    tab_sh = nc.dram_tensor("tab_sh", [N, TROW], bf16, addr_space="Shared")
    tab = nc.dram_tensor("tab", [N, TROW], bf16)

    def wo(name, rows=96, width=None):
        w_ = width if width is not None else 96
        return (WOFF[name], WOFF[name] + w_, rows)

    with tile.TileContext(nc) as tc:
      with (tc.tile_pool(name="const", bufs=1) as cp,
            tc.tile_pool(name="hp", bufs=1) as hp,
            tc.tile_pool(name="wp", bufs=1) as wp,
            tc.tile_pool(name="sp", bufs=3) as sp,
            tc.tile_pool(name="gpool", bufs=2) as gpl,
            tc.tile_pool(name="ps", bufs=2, space="PSUM") as psp,
            tc.tile_pool(name="psA", bufs=2, space="PSUM") as psA,
            tc.tile_pool(name="psagg", bufs=1, space="PSUM") as psG):

        ident = cp.tile([128, 128], bf16)
        nc.sync.dma_start(out=ident[:], in_=nc.inline_tensor(np.eye(128, dtype=BF16), name="idb").ap())
        identf = cp.tile([128, 128], f32)
        nc.sync.dma_start(out=identf[:], in_=nc.inline_tensor(np.eye(128, dtype=np.float32), name="idf").ap())
        wb_t = cp.tile([128, WB], f32)
        nc.sync.dma_start(out=wb_t[:], in_=dwb[:])
        gw_t = cp.tile([128, NSLOT // 16], i16)
        for k in range(8):
            nc.sync.dma_start(out=gw_t[16 * k:16 * (k + 1), :], in_=dgw[:])
        dc_t = cp.tile([128, NSLOT // 128], f32)
        nc.sync.dma_start(out=dc_t[:], in_=ddc[:])

        def W(name, rows=96, width=96):
            return wb_t[:rows, WOFF[name]:WOFF[name] + width]

        iota_t = wb_t[:, WOFF['iotaf']:WOFF['iotaf'] + 48]
        iota96 = wb_t[:96, WOFF['iota96']:WOFF['iota96'] + 1]

        one1 = cp.tile([1, 96], f32)
        nc.vector.memset(one1[:], 1.0)
        ones1 = cp.tile([1, 128], bf16)
        nc.vector.memset(ones1[:], 1.0)
        att04 = {}
        for l in range(L):
            for b in range(2):
                att04[(l, b)] = cp.tile([96, 1], bf16, tag=f"att04_{l}_{b}", name=f"att04_{l}_{b}")
                nc.vector.tensor_scalar(out=att04[(l, b)][:], in0=W(f'att_{l}_{b}', 96, 1),
                                        scalar1=0.4, scalar2=None, op0=OP.mult)
        Vt = {}
        for l in range(L):
            Vt[l] = cp.tile([8, 2], bf16, tag=f"V_{l}", name=f"V_{l}")
            nc.vector.tensor_copy(out=Vt[l][:], in_=wb_t[:8, WOFF[f'V_{l}']:WOFF[f'V_{l}'] + 2])

        # h_T feature-major [96, NPAD] (cols beyond NLOC are pad)
        h_T = [hp.tile([96, NCH * 128], f32, tag=f"h{b}", name=f"h{b}") for b in range(2)]
        for ch in range(NCH):
            n0 = ch * 128
            nreal = max(0, min(NLOC - n0, 128))
            xin = sp.tile([128, 128], bf16, tag="xin")
            nc.vector.memset(xin[:], 0.0)
            if nreal > 0:
                nc.sync.dma_start(out=xin[:nreal, :96], in_=dx[n0:n0 + nreal, :])
            pt = psA.tile([128, 128], bf16, tag="pbig")
            nc.tensor.transpose(out=pt[:], in_=xin[:], identity=ident[:])
            for b in range(2):
                nc.vector.tensor_copy(out=h_T[b][:, n0:n0 + 128], in_=pt[:96, :])

        hw_T = [wp.tile([96, NCH * 128], f32, tag=f"hw{b}", name=f"hw{b}") for b in range(2)]

        for l in range(L):
            # ---------- PASS A ----------
            for b in range(2):
                for cs in range(0, NCH * 128, 512):
                    ce = min(cs + 512, NCH * 128)
                    w_ = ce - cs
                    pl = psA.tile([96, 512], f32, tag="pbig")
                    nc.tensor.matmul(out=pl[:, :w_], lhsT=W(f'Wl_{l}_{b}'),
                                     rhs=h_T[b][:, cs:ce], start=True, stop=True)
                    nc.vector.tensor_copy(out=hw_T[b][:, cs:ce], in_=pl[:, :w_])
            # table slice + allgather
            for ch in range(NCH):
                n0 = ch * 128
                nreal = max(0, min(NLOC - n0, 128))
                if nreal == 0:
                    continue
                stg = sp.tile([128, TROW], bf16, tag="stg")
                nc.vector.memset(stg[:], 0.0)
                for b in range(2):
                    pt = psA.tile([128, 128], f32, tag="pbig")
                    nc.tensor.transpose(out=pt[:, :96], in_=hw_T[b][:, n0:n0 + 128],
                                        identity=identf[:96, :96])
                    nc.vector.tensor_copy(out=stg[:, b * 128:b * 128 + 96], in_=pt[:, :96])
                    # w = exp(0.6*att.hl) for this chunk; ones at ext row 32
                    pphi = psA.tile([1, 128], f32, tag="pbig")
                    nc.tensor.matmul(out=pphi[:], lhsT=W(f'att_{l}_{b}', 96, 1),
                                     rhs=hw_T[b][:, n0:n0 + 128], start=True, stop=True)
                    ext = sp.tile([64, 128], f32, tag="ext")
                    nc.scalar.activation(out=ext[0:1, :], in_=pphi[:], func=AF.Exp, scale=0.6)
                    nc.vector.memset(ext[32:33, :], 1.0)
                    pt2 = psA.tile([128, 64], f32, tag="pbig")
                    nc.tensor.transpose(out=pt2[:], in_=ext[:], identity=identf[:64, :64])
                    nc.vector.tensor_copy(out=stg[:, b * 128 + 96:b * 128 + 97], in_=pt2[:, 32:33])
                    nc.vector.tensor_copy(out=stg[:, b * 128 + 97:b * 128 + 98], in_=pt2[:, 0:1])
                nc.vector.tensor_copy(out=stg[:, 98:99], in_=stg[:, 225:226])
                nc.sync.dma_start(out=tab_slice[n0:n0 + nreal, :], in_=stg[:nreal, :])
            nc.gpsimd.collective_compute(
                "AllGather", mybir.AluOpType.bypass,
                replica_groups=[list(range(NCORES))],
                ins=[tab_slice[:]], outs=[tab_sh[:]],
            )
            nc.sync.dma_start(out=tab[:], in_=tab_sh[:])

            # ---------- edge phase ----------
            for w in range(0 if not SKIP_EDGE else NWIN, NWIN):
                aggp = {}
                first = {b: True for b in range(2)}
                nagg = {b: 0 for b in range(2)}
                tot = {b: sum(int(Kf[(w * 2 + h) * 2 + s]) for h in range(2) for s in range(2))
                       for b in range(2)}
                for b in range(2):
                    aggp[b] = psG.tile([97, WIN], f32, tag=f"agg{b}", name=f"agg{b}")
                # base lhsT per branch for this window (hr = h @ Wr computed here)
                basel = {}
                for b in range(2):
                    phr = psA.tile([96, WIN], f32, tag="pbig")
                    nc.tensor.matmul(out=phr[:], lhsT=W(f'Wr_{l}_{b}'),
                                     rhs=h_T[b][:, w * WIN:(w + 1) * WIN],
                                     start=True, stop=True)
                    hrs = sp.tile([96, WIN], f32, tag="hrs")
                    nc.vector.tensor_copy(out=hrs[:], in_=phr[:])
                    pt = psA.tile([WIN, 96], f32, tag="pbig")
                    nc.tensor.transpose(out=pt[:], in_=hrs[:], identity=identf[:96, :96])
                    bl = sp.tile([128, 96], bf16, tag=f"basel{b}", name=f"basel{b}")
                    nc.vector.memset(bl[:], 0.0)
                    nc.vector.tensor_copy(out=bl[:WIN, :], in_=pt[:])
                    nc.vector.tensor_copy(out=bl[WIN:WIN + 8, :], in_=W(f'We_{l}_{b}', 8, 96))
                    basel[b] = bl
                for h in range(2):
                    for s in range(2):
                        si = (w * 2 + h) * 2 + s
                        Ks = int(Kf[si])
                        sl0 = int(sec_slot[si])
                        nsl = Ks * 128
                        g = gpl.tile([128, KMAX, TROW], bf16, tag="gath")
                        if SKIP_GATHER:
                            nc.vector.memset(g[:, :Ks, :], 0.0)
                        else:
                            nc.gpsimd.dma_gather(
                                out_ap=g[:, :Ks, :],
                                in_ap=tab[SPLIT:, :] if s else tab[:SPLIT, :],
                                idxs_ap=gw_t[:, sl0 // 16:(sl0 + nsl) // 16],
                                num_idxs=nsl, num_idxs_reg=nsl, elem_size=TROW)
                        # compact edge table slice: rows 0-7 ea, row 8 dstcol
                        eavs = sp.tile([8, KMAX * 128], bf16, tag="eavs")
                        nc.sync.dma_start(out=eavs[:, :nsl], in_=deav[0:8, sl0:sl0 + nsl])
                        dcw = sp.tile([1, KMAX * 128], bf16, tag="dcw")
                        nc.sync.dma_start(out=dcw[:, :nsl], in_=deav[8:9, sl0:sl0 + nsl])
                        # build Rt on device: rows 0-95 one-hot(dstcol), 96-103 ea
                        Rt = sp.tile([128, KMAX * 128], bf16, tag="Rt")
                        for c0 in range(0, nsl, 512):
                            cw = min(512, nsl - c0)
                            pbc = psA.tile([128, 512], f32, tag="pbig")
                            nc.tensor.matmul(out=pbc[:, :cw], lhsT=ones1[:],
                                             rhs=dcw[0:1, c0:c0 + cw], start=True, stop=True)
                            nc.vector.tensor_scalar(out=Rt[0:96, c0:c0 + cw],
                                                    in0=pbc[0:96, :cw], scalar1=iota96,
                                                    scalar2=None, op0=OP.is_equal)
                            nc.vector.tensor_copy(out=Rt[96:104, c0:c0 + cw],
                                                  in_=eavs[0:8, c0:c0 + cw])
                        lgp = psp.tile([128, 16], f32, tag="lgp", bufs=1)
                        for j0 in range(0, Ks, 4):
                            jw = min(4, Ks - j0)
                            for b in range(2):
                                mps = psp.tile([96, 512], f32, tag="mps")
                                nc.tensor.matmul(out=mps[:, :jw * 128], lhsT=basel[b][:],
                                                 rhs=Rt[:, j0 * 128:(j0 + jw) * 128],
                                                 start=True, stop=False)
                                for dj in range(jw):
                                    j = j0 + dj
                                    nc.tensor.matmul(out=mps[:, dj * 128:(dj + 1) * 128],
                                                     lhsT=g[:, j, b * 128:b * 128 + 96],
                                                     rhs=ident[:], start=False,
                                                     stop=(dj == jw - 1),
                                                     skip_group_check=True)
                                am = sp.tile([96, 512], bf16, tag="am")
                                nc.scalar.activation(out=am[:, :jw * 128],
                                                     in_=mps[:, :jw * 128], func=AF.Abs)
                                for dj in range(jw):
                                    j = j0 + dj
                                    nc.tensor.matmul(out=lgp[:, 2 * j + b:2 * j + b + 1],
                                                     lhsT=am[:, dj * 128:(dj + 1) * 128],
                                                     rhs=att04[(l, b)][:],
                                                     start=(j == 0 and b == 0), stop=False,
                                                     skip_group_check=True)
                        # += 0.6*ea.(We@att) per branch (cols 2j|2j+1), on device
                        for j in range(Ks):
                            nc.tensor.matmul(out=lgp[:, 2 * j:2 * j + 2],
                                             lhsT=eavs[0:8, j * 128:(j + 1) * 128],
                                             rhs=Vt[l][:], start=False, stop=(j == Ks - 1),
                                             skip_group_check=True)
                        exw = sp.tile([128, 16], f32, tag="exw")
                        nc.scalar.activation(out=exw[:, :2 * Ks], in_=lgp[:, :2 * Ks],
                                             func=AF.Exp)
                        nc.vector.tensor_tensor(
                            out=exw[:, :2 * Ks].rearrange("p (j b) -> p j b", b=2),
                            in0=exw[:, :2 * Ks].rearrange("p (j b) -> p j b", b=2),
                            in1=g[:, :Ks, 97:99], op=OP.mult)
                        for j in range(Ks):
                            blk = sl0 // 128 + j
                            for b in range(2):
                                es = sp.tile([128, HALF], bf16, tag="es")
                                nc.vector.tensor_scalar(
                                    out=es[:], in0=iota_t, scalar1=dc_t[:, blk:blk + 1],
                                    scalar2=exw[:, 2 * j + b:2 * j + b + 1],
                                    op0=OP.is_equal, op1=OP.mult)
                                nagg[b] += 1
                                nc.tensor.matmul(out=aggp[b][:, h * HALF:(h + 1) * HALF],
                                                 lhsT=g[:, j, b * 128:b * 128 + 97],
                                                 rhs=es[:],
                                                 start=first[b], stop=(nagg[b] == tot[b]),
                                                 skip_group_check=True)
                                first[b] = False
                # finalize window -> h_T
                for b in range(2):
                    num = sp.tile([96, WIN], f32, tag="num")
                    den = sp.tile([1, WIN], f32, tag="den")
                    nc.vector.tensor_copy(out=num[:], in_=aggp[b][:96, :])
                    nc.vector.tensor_scalar(out=den[:], in0=aggp[b][96:97, :],
                                            scalar1=1e-30, scalar2=None, op0=OP.add)
                    rec = sp.tile([1, WIN], f32, tag="rec")
                    nc.vector.reciprocal(out=rec[:], in_=den[:])
                    pb = psp.tile([96, WIN], f32, tag="mps")
                    nc.tensor.matmul(out=pb[:], lhsT=one1[:], rhs=rec[:], start=True, stop=True)
                    tdiv = sp.tile([96, WIN], f32, tag="tdiv")
                    nc.vector.tensor_tensor(out=tdiv[:], in0=num[:], in1=pb[:], op=OP.mult)
                    lin = sp.tile([96, WIN], f32, tag="lin")
                    nc.scalar.activation(out=lin[:], in_=tdiv[:], func=AF.Identity,
                                         bias=W(f'bb_{l}_{b}', 96, 1))
                    ab = sp.tile([96, WIN], f32, tag="ab")
                    nc.scalar.activation(out=ab[:], in_=tdiv[:], func=AF.Abs,
                                         bias=W(f'bb_{l}_{b}', 96, 1))
                    nc.vector.tensor_scalar(out=lin[:], in0=lin[:], scalar1=0.505,
                                            scalar2=None, op0=OP.mult)
                    nc.vector.tensor_scalar(out=ab[:], in0=ab[:], scalar1=0.495,
                                            scalar2=None, op0=OP.mult)
                    nc.vector.tensor_tensor(out=h_T[b][:, w * WIN:(w + 1) * WIN],
                                            in0=lin[:], in1=ab[:], op=OP.add)

        # ---------- head ----------
        hid_T = [wp.tile([128, NCH * 128], f32, tag=f"hw{p}", name=f"hid{p}") for p in range(2)]
        for cs in range(0, NCH * 128, 512):
            ce = min(cs + 512, NCH * 128)
            w_ = ce - cs
            pf = psA.tile([96, 512], f32, tag="pbig")
            nc.tensor.matmul(out=pf[:, :w_], lhsT=W('fusion_Wt'),
                             rhs=h_T[0][:, cs:ce], start=True, stop=False)
            nc.tensor.matmul(out=pf[:, :w_], lhsT=W('fusion_Wb'),
                             rhs=h_T[1][:, cs:ce], start=False, stop=True)
            fus = sp.tile([96, 512], f32, tag="fus")
            lin = sp.tile([96, 512], f32, tag="flin")
            nc.scalar.activation(out=lin[:, :w_], in_=pf[:, :w_], func=AF.Identity,
                                 bias=W('fusion_b', 96, 1))
            ab = sp.tile([96, 512], f32, tag="fab")
            nc.scalar.activation(out=ab[:, :w_], in_=pf[:, :w_], func=AF.Abs,
                                 bias=W('fusion_b', 96, 1))
            nc.vector.tensor_scalar(out=lin[:, :w_], in0=lin[:, :w_], scalar1=0.505,
                                    scalar2=None, op0=OP.mult)
            nc.vector.tensor_scalar(out=ab[:, :w_], in0=ab[:, :w_], scalar1=0.495,
                                    scalar2=None, op0=OP.mult)
            nc.vector.tensor_tensor(out=fus[:, :w_], in0=lin[:, :w_], in1=ab[:, :w_],
                                    op=OP.add)
            for p, (wk, bk) in enumerate([('pred_W1a', 'pred_b1a'), ('pred_W1b', 'pred_b1b')]):
                ph = psA.tile([128, 512], f32, tag="pbig")
                nc.tensor.matmul(out=ph[:, :w_], lhsT=W(wk, 96, 128), rhs=fus[:, :w_],
                                 start=True, stop=True)
                l2 = sp.tile([128, 512], f32, tag=f"l2{p}")
                a2 = sp.tile([128, 512], f32, tag=f"a2{p}")
                nc.scalar.activation(out=l2[:, :w_], in_=ph[:, :w_], func=AF.Identity,
                                     bias=W(bk, 128, 1))
                nc.scalar.activation(out=a2[:, :w_], in_=ph[:, :w_], func=AF.Abs,
                                     bias=W(bk, 128, 1))
                nc.vector.tensor_scalar(out=l2[:, :w_], in0=l2[:, :w_], scalar1=0.505,
                                        scalar2=None, op0=OP.mult)
                nc.vector.tensor_scalar(out=a2[:, :w_], in0=a2[:, :w_], scalar1=0.495,
                                        scalar2=None, op0=OP.mult)
                nc.vector.tensor_tensor(out=hid_T[p][:, cs:ce], in0=l2[:, :w_],
                                        in1=a2[:, :w_], op=OP.add)
        for ch in range(NCH):
            n0 = ch * 128
            nreal = max(0, min(NLOC - n0, 128))
            if nreal == 0:
                continue
            po = psp.tile([128, 2], f32, tag="mps")
            nc.tensor.matmul(out=po[:], lhsT=hid_T[0][:, n0:n0 + 128],
                             rhs=W('pred_W2a', 128, 2), start=True, stop=False)
            nc.tensor.matmul(out=po[:], lhsT=hid_T[1][:, n0:n0 + 128],
                             rhs=W('pred_W2b', 128, 2), start=False, stop=True)
            ot = sp.tile([128, 2], f32, tag="ot")
            nc.vector.tensor_tensor(out=ot[:], in0=po[:], in1=W('pred_b2', 128, 2), op=OP.add)
            nc.sync.dma_start(out=dout[n0:n0 + nreal, :], in_=ot[:nreal, :])

    nc.compile()
    return nc


def _make_runner(nc):
    """Build (once) a cached jitted shard_map wrapper around the compiled
    Bass module — same lowering as bass2jax.run_bass_via_pjrt, but the jit
    object is reused across calls so warm calls skip retrace/recompile."""
    import jax
    import jax.core as jcore
    from jax.experimental.shard_map import shard_map
    from jax.sharding import Mesh, PartitionSpec
    from concourse import bass2jax, mybir
    bass2jax.install_neuronx_cc_hook()

    partition_name = nc.partition_id_tensor.name if nc.partition_id_tensor else None
    in_names, out_names, out_avals, zero_shapes = [], [], [], []
    for alloc in nc.m.functions[0].allocations:
        if not isinstance(alloc, mybir.MemoryLocationSet):
            continue
        name = alloc.memorylocations[0].name
        if alloc.kind == "ExternalInput":
            if name != partition_name:
                in_names.append(name)
        elif alloc.kind == "ExternalOutput":
            shape = tuple(alloc.tensor_shape)
            dtype = mybir.dt.np(alloc.dtype)
            out_names.append(name)
            out_avals.append(jcore.ShapedArray(shape, dtype))
            zero_shapes.append((shape, dtype))
    n_params = len(in_names)
    n_outs = len(out_avals)
    all_in = list(in_names) + list(out_names)
    if partition_name is not None:
        all_in.append(partition_name)
    donate = tuple(range(n_params, n_params + n_outs))

    dbg_name = None
    if nc.dbg_addr is not None:
        assert not nc.dbg_callbacks
        dbg_name = nc.dbg_addr.name

    def _body(*args):
        operands = list(args)
        if partition_name is not None:
            operands.append(bass2jax.partition_id_tensor())
        outs = bass2jax._bass_exec_p.bind(
            *operands, out_avals=tuple(out_avals), in_names=tuple(all_in),
            out_names=tuple(out_names), lowering_input_output_aliases=(),
            sim_require_finite=True, sim_require_nnan=True, nc=nc)
        return tuple(outs)

    devices = jax.devices()[:NCORES]
    mesh = Mesh(np.asarray(devices), ("core",))
    in_specs = (PartitionSpec("core"),) * (n_params + n_outs)
    out_specs = (PartitionSpec("core"),) * n_outs
    fn = jax.jit(shard_map(_body, mesh=mesh, in_specs=in_specs,
                           out_specs=out_specs, check_rep=False),
                 donate_argnums=donate, keep_unused=True)
    return dict(fn=fn, in_names=in_names, out_names=out_names,
                out_avals=out_avals, zero_shapes=zero_shapes, dbg_name=dbg_name)


def _execute(runner, in_maps):
    n = len(in_maps)
    if runner['dbg_name'] is not None:
        z = np.zeros((1, 2), np.uint32)
        in_maps = [{**m, runner['dbg_name']: z} for m in in_maps]
    concat_in = [np.concatenate([np.asarray(in_maps[c][nm]) for c in range(n)], axis=0)
                 for nm in runner['in_names']]
    zeros = [np.zeros((n * s[0], *s[1:]), dt) for (s, dt) in runner['zero_shapes']]
    outs = runner['fn'](*concat_in, *zeros)
    return [{nm: np.asarray(outs[i]).reshape(n, *runner['out_avals'][i].shape)[c]
             for i, nm in enumerate(runner['out_names'])} for c in range(n)]


def _in_maps(x, pp, blob):
    xb = np.ascontiguousarray(x.astype(BF16))
    maps = []
    for c in range(NCORES):
        maps.append({'x': xb[c * NLOC:(c + 1) * NLOC],
                     'eav': pp['eav'][c],
                     'gw': pp['gw'][c],
                     'dcol': np.ascontiguousarray(pp['dcol'][c]),
                     'wb': blob})
    return maps


def kernel(**inputs):
    x = np.asarray(inputs['x'], np.float32)
    ei = np.asarray(inputs['edge_index'])
    ea = np.asarray(inputs['edge_attr'], np.float32)
    pp = _host_prep(x, ei, ea)
    if _CACHE.get('NSLOT') != pp['NSLOT']:
        _CACHE['nc'] = build_kernel(pp)
        _CACHE['runner'] = _make_runner(_CACHE['nc'])
        _CACHE['NSLOT'] = pp['NSLOT']
    blob = _wblob(inputs)
    res = _execute(_CACHE['runner'], _in_maps(x, pp, blob))
    out = np.concatenate([res[c]['out'] for c in range(NCORES)], axis=0)
    return out.astype(np.float32)


# revision 18
# speedup vs baseline: 13.8484x; 1.1477x over previous
"""BiLevelGAT (2-branch x 3-layer GATv2, N=50000, E=500000, D=96) on 8 TRN2 cores.

Sharding: nodes + incoming edges partitioned by dst; per-layer AllGather of a
bf16 per-node table [hl_loc 96|1|w_loc|w_glob|pad29|hl_glob 96|1|w_glob|pad30]
(512B rows) gathered per edge by src.

Math: lrelu(x) = 0.6x+0.4|x| splits the GATv2 logit into linear terms (per-src
w=exp(0.6*att.hl) folded into the softmax weight; per-dst term cancels in
softmax; per-edge ea term computed on device from a compact [9, NSLOT] edge
table: rows 0-7 ea, row 8 dst-col) plus 0.4*att.|m| computed on device.
Softmax max-subtraction skipped (logits O(1), fp32 safe).

The per-edge one-hot "R" matrix (dst selector + ea rows used as matmul rhs to
form m = hr[dst] + ea@We) is built on device per section: a rank-1 broadcast
matmul of the dst-col row followed by an is_equal against a partition iota.
Inputs per core are ~5MB (x bf16, edge table bf16, gather idx, dst cols,
one packed weight blob) so the host->device transfer over the axon tunnel
stays small; the jitted shard_map wrapper is cached across calls.
"""
import sys
sys.path.insert(0, '/opt/trn_rl_repo')
import numpy as np
import ml_dtypes

BF16 = ml_dtypes.bfloat16

N, E, D, EDIM, L, DENSE, OUT = 50000, 500000, 96, 8, 3, 256, 2
NCORES = 8
NLOC = N // NCORES            # 6250
WIN, HALF = 96, 48
NWIN = (NLOC + WIN - 1) // WIN  # 66
NPAD = NWIN * WIN
NCH = (NPAD + 127) // 128     # chunks of 128 (PASS A / table)
SPLIT = 32768
TROW = 256
KMAX = 6

_CACHE = {}

# ---- packed weight blobs ----
# blob96 [96, WB96]: 96-row tensors; We_{l,b} stacked at rows 8*(2l+b);
# V_l at row base 32*l (matmul rhs needs base partition 0/32/64).
# blob128 [128, WB128]: 128-row tensors + iotaf.
WOFF = {}   # name -> (row0, col0)
_col96 = 0
_col128 = 0
def _a96(name, width, row=0):
    global _col96
    WOFF[name] = (row, _col96)
    _col96 += width
def _a128(name, width):
    global _col128
    WOFF[name] = (0, _col128)
    _col128 += width
for _l in range(L):
    for _b in range(2):
        _a96(f'Wl_{_l}_{_b}', 96)
        _a96(f'Wr_{_l}_{_b}', 96)
        _a96(f'att_{_l}_{_b}', 1)
        _a96(f'bb_{_l}_{_b}', 1)
# partition base of any access must be a multiple of 32: stack We six-up
# in two 96-col groups at row bases 0/32/64
for _i in range(6):
    _l, _b = divmod(_i, 2)
    WOFF[f'We_{_l}_{_b}'] = (32 * (_i % 3), _col96 + 96 * (_i // 3))
_col96 += 192
for _l in range(L):
    WOFF[f'V_{_l}'] = (32 * _l, _col96)                       # shared 2 cols
_col96 += 2
_a96('fusion_Wt', 96)
_a96('fusion_Wb', 96)
_a96('fusion_b', 1)
_a96('pred_W1a', 128)
_a96('pred_W1b', 128)
_a96('iota96', 1)                     # rows 0-95 = arange(96)
WB96 = _col96
_a128('pred_b1a', 1)
_a128('pred_b1b', 1)
_a128('pred_W2a', 2)
_a128('pred_W2b', 2)
_a128('pred_b2', 2)
_a128('iotaf', 48)                    # all 128 rows = arange(48)
WB128 = _col128
B128 = {'pred_b1a', 'pred_b1b', 'pred_W2a', 'pred_W2b', 'pred_b2', 'iotaf'}


def _host_prep(x, edge_index, edge_attr):
    src = edge_index[0].astype(np.int64)
    dst = edge_index[1].astype(np.int64)
    mean_ea = edge_attr.mean(0).astype(np.float32)
    loop = np.arange(N, dtype=np.int64)
    src_a = np.concatenate([src, loop])
    dst_a = np.concatenate([dst, loop])
    ea_a = np.concatenate([edge_attr.astype(np.float32),
                           np.broadcast_to(mean_ea, (N, EDIM))], 0)

    owner = dst_a // NLOC
    dloc = dst_a - owner * NLOC
    win = dloc // WIN
    half = (dloc % WIN) // HALF
    stream = (src_a >= SPLIT).astype(np.int64)

    per_core = []
    secs = np.zeros((NCORES, NWIN, 2, 2), np.int64)
    for c in range(NCORES):
        m = owner == c
        s_c, d_c, e_c = src_a[m], dloc[m], ea_a[m]
        w_c, h_c, st_c = win[m], half[m], stream[m]
        sec = ((w_c * 2 + h_c) * 2 + st_c)
        order = np.argsort(sec * NLOC + d_c, kind='stable')
        s_c, d_c, e_c, sec = s_c[order], d_c[order], e_c[order], sec[order]
        st_c = st_c[order]
        per_core.append((s_c, d_c, e_c, sec, st_c))
        secs[c] = np.bincount(sec, minlength=NWIN * 4).reshape(NWIN, 2, 2)

    K = np.maximum((secs.max(0) + 127) // 128, 1)       # [NWIN, 2, 2]
    assert K.max() <= KMAX
    Kf = K.reshape(-1)
    sec_slot = np.zeros(NWIN * 4 + 1, np.int64)
    np.cumsum(Kf * 128, out=sec_slot[1:])
    NSLOT = int(sec_slot[-1])

    gidx = np.zeros((NCORES, NSLOT), np.int16)
    eav = np.zeros((NCORES, 9, NSLOT), np.float32)
    # invalid slots: dstcol=147 misses the 96-wide one-hot AND (after -48h)
    # the 48-wide es window -> no contribution
    eav[:, 8, :] = 147.0

    for c in range(NCORES):
        s_c, d_c, e_c, sec, st_c = per_core[c]
        counts = np.bincount(sec, minlength=NWIN * 4)
        starts = np.concatenate([[0], np.cumsum(counts)])[:-1]
        pos = np.arange(len(s_c)) - starts[sec]
        slot = sec_slot[sec] + pos
        gidx[c, slot] = (s_c - st_c * SPLIT).astype(np.int16)
        eav[c, 0:8, slot] = e_c  # advanced idx puts slot axis first: (nedge, 8)
        eav[c, 8, slot] = (d_c % WIN).astype(np.float32)

    gw = np.ascontiguousarray(
        gidx.reshape(NCORES, -1, 16).transpose(0, 2, 1))     # [NCORES, 16, NSLOT//16]

    return dict(K=K, Kf=Kf, sec_slot=sec_slot, NSLOT=NSLOT, NSEC=NWIN * 4,
                gw=gw, eav=eav.astype(BF16))


def _wblob(w):
    b96 = np.zeros((96, WB96), np.float32)
    b128 = np.zeros((128, WB128), np.float32)
    def put(name, arr):
        a = np.asarray(arr, np.float32)
        if a.ndim == 1:
            a = a.reshape(-1, 1)
        r0, c0 = WOFF[name]
        dst = b96 if name not in B128 else b128
        dst[r0:r0 + a.shape[0], c0:c0 + a.shape[1]] = a
    for l in range(L):
        V = np.zeros((8, 2), np.float32)
        for b, p in enumerate(['local', 'global']):
            put(f'Wl_{l}_{b}', w[f'{p}_Wl'][l])
            put(f'Wr_{l}_{b}', w[f'{p}_Wr'][l])
            put(f'att_{l}_{b}', w[f'{p}_att'][l])
            put(f'bb_{l}_{b}', w[f'{p}_b'][l])
            put(f'We_{l}_{b}', w[f'{p}_We'][l])
            V[:, b] = 0.6 * (np.asarray(w[f'{p}_We'][l], np.float32)
                             @ np.asarray(w[f'{p}_att'][l], np.float32))
        put(f'V_{l}', V)
    put('fusion_Wt', w['fusion_W'][:96])
    put('fusion_Wb', w['fusion_W'][96:])
    put('fusion_b', w['fusion_b'])
    put('pred_W1a', w['pred_W1'][:, :128])
    put('pred_W1b', w['pred_W1'][:, 128:])
    put('pred_b1a', w['pred_b1'][:128])
    put('pred_b1b', w['pred_b1'][128:])
    put('pred_W2a', w['pred_W2'][:128])
    put('pred_W2b', w['pred_W2'][128:])
    put('pred_b2', np.broadcast_to(np.asarray(w['pred_b2']).reshape(1, 2), (128, 2)))
    put('iotaf', np.broadcast_to(np.arange(48, dtype=np.float32), (128, 48)))
    put('iota96', np.arange(96, dtype=np.float32))
    return b96, b128


def build_kernel(pp):
    import os as _os
    SKIP_EDGE = _os.environ.get('SKIP_EDGE', '0') == '1'
    SKIP_GATHER = _os.environ.get('SKIP_GATHER', '0') == '1'
    from concourse import mybir, bacc
    import concourse.tile as tile
    Kf, sec_slot, NSLOT = pp['Kf'], pp['sec_slot'], pp['NSLOT']
    f32, bf16, i16 = mybir.dt.float32, mybir.dt.bfloat16, mybir.dt.int16
    AF = mybir.ActivationFunctionType
    OP = mybir.AluOpType

    nc = bacc.Bacc("TRN2", target_bir_lowering=False, debug=False, num_devices=NCORES)
    dx = nc.dram_tensor("x", [NLOC, D], bf16, kind="ExternalInput")
    deav = nc.dram_tensor("eav", [9, NSLOT], bf16, kind="ExternalInput")
    dgw = nc.dram_tensor("gw", [16, NSLOT // 16], i16, kind="ExternalInput")
    dwb96 = nc.dram_tensor("wb96", [96, WB96], f32, kind="ExternalInput")
    dwb128 = nc.dram_tensor("wb128", [128, WB128], f32, kind="ExternalInput")
    dout = nc.dram_tensor("out", [NLOC, OUT], f32, kind="ExternalOutput")

    tab_slice = nc.dram_tensor("tab_slice", [NLOC, TROW], bf16)
    tab_sh = nc.dram_tensor("tab_sh", [N, TROW], bf16, addr_space="Shared")
    tab = nc.dram_tensor("tab", [N, TROW], bf16)

    def wo(name, rows=96, width=None):
        w_ = width if width is not None else 96
        return (WOFF[name], WOFF[name] + w_, rows)

    with tile.TileContext(nc) as tc:
      with (tc.tile_pool(name="const", bufs=1) as cp,
            tc.tile_pool(name="hp", bufs=1) as hp,
            tc.tile_pool(name="wp", bufs=1) as wp,
            tc.tile_pool(name="sp", bufs=3) as sp,
            tc.tile_pool(name="gpool", bufs=2) as gpl,
            tc.tile_pool(name="ps", bufs=2, space="PSUM") as psp,
            tc.tile_pool(name="psA", bufs=2, space="PSUM") as psA,
            tc.tile_pool(name="psagg", bufs=1, space="PSUM") as psG):

        ident = cp.tile([128, 128], bf16)
        nc.sync.dma_start(out=ident[:], in_=nc.inline_tensor(np.eye(128, dtype=BF16), name="idb").ap())
        identf = cp.tile([128, 128], f32)
        nc.sync.dma_start(out=identf[:], in_=nc.inline_tensor(np.eye(128, dtype=np.float32), name="idf").ap())
        wb96_t = cp.tile([96, WB96], f32)
        nc.sync.dma_start(out=wb96_t[:], in_=dwb96[:])
        wb128_t = cp.tile([128, WB128], f32)
        nc.sync.dma_start(out=wb128_t[:], in_=dwb128[:])
        gw_t = cp.tile([128, NSLOT // 16], i16)
        for k in range(8):
            nc.sync.dma_start(out=gw_t[16 * k:16 * (k + 1), :], in_=dgw[:])

        def W(name, rows=96, width=96):
            r0, c0 = WOFF[name]
            t = wb128_t if name in B128 else wb96_t
            return t[r0:r0 + rows, c0:c0 + width]

        iota_t = wb128_t[:, WOFF['iotaf'][1]:WOFF['iotaf'][1] + 48]
        iota96 = wb96_t[:96, WOFF['iota96'][1]:WOFF['iota96'][1] + 1]

        one1 = cp.tile([1, 96], f32)
        nc.vector.memset(one1[:], 1.0)
        ones1 = cp.tile([1, 128], bf16)
        nc.vector.memset(ones1[:], 1.0)
        att04 = {}
        for l in range(L):
            for b in range(2):
                att04[(l, b)] = cp.tile([96, 1], bf16, tag=f"att04_{l}_{b}", name=f"att04_{l}_{b}")
                nc.vector.tensor_scalar(out=att04[(l, b)][:], in0=W(f'att_{l}_{b}', 96, 1),
                                        scalar1=0.4, scalar2=None, op0=OP.mult)
        Vt = {}
        for l in range(L):
            Vt[l] = cp.tile([8, 2], bf16, tag=f"V_{l}", name=f"V_{l}")
            nc.vector.tensor_copy(out=Vt[l][:], in_=W(f'V_{l}', 8, 2))

        # h_T feature-major [96, NPAD] (cols beyond NLOC are pad)
        h_T = [hp.tile([96, NCH * 128], f32, tag=f"h{b}", name=f"h{b}") for b in range(2)]
        for ch in range(NCH):
            n0 = ch * 128
            nreal = max(0, min(NLOC - n0, 128))
            xin = sp.tile([128, 128], bf16, tag="xin")
            nc.vector.memset(xin[:], 0.0)
            if nreal > 0:
                nc.sync.dma_start(out=xin[:nreal, :96], in_=dx[n0:n0 + nreal, :])
            pt = psA.tile([128, 128], bf16, tag="pbig")
            nc.tensor.transpose(out=pt[:], in_=xin[:], identity=ident[:])
            for b in range(2):
                nc.vector.tensor_copy(out=h_T[b][:, n0:n0 + 128], in_=pt[:96, :])

        hw_T = [wp.tile([96, NCH * 128], f32, tag=f"hw{b}", name=f"hw{b}") for b in range(2)]

        for l in range(L):
            # ---------- PASS A ----------
            for b in range(2):
                for cs in range(0, NCH * 128, 512):
                    ce = min(cs + 512, NCH * 128)
                    w_ = ce - cs
                    pl = psA.tile([96, 512], f32, tag="pbig")
                    nc.tensor.matmul(out=pl[:, :w_], lhsT=W(f'Wl_{l}_{b}'),
                                     rhs=h_T[b][:, cs:ce], start=True, stop=True)
                    nc.vector.tensor_copy(out=hw_T[b][:, cs:ce], in_=pl[:, :w_])
            # table slice + allgather
            for ch in range(NCH):
                n0 = ch * 128
                nreal = max(0, min(NLOC - n0, 128))
                if nreal == 0:
                    continue
                stg = sp.tile([128, TROW], bf16, tag="stg")
                nc.vector.memset(stg[:], 0.0)
                for b in range(2):
                    pt = psA.tile([128, 128], f32, tag="pbig")
                    nc.tensor.transpose(out=pt[:, :96], in_=hw_T[b][:, n0:n0 + 128],
                                        identity=identf[:96, :96])
                    nc.vector.tensor_copy(out=stg[:, b * 128:b * 128 + 96], in_=pt[:, :96])
                    # w = exp(0.6*att.hl) for this chunk; ones at ext row 32
                    pphi = psA.tile([1, 128], f32, tag="pbig")
                    nc.tensor.matmul(out=pphi[:], lhsT=W(f'att_{l}_{b}', 96, 1),
                                     rhs=hw_T[b][:, n0:n0 + 128], start=True, stop=True)
                    ext = sp.tile([64, 128], f32, tag="ext")
                    nc.scalar.activation(out=ext[0:1, :], in_=pphi[:], func=AF.Exp, scale=0.6)
                    nc.vector.memset(ext[32:33, :], 1.0)
                    pt2 = psA.tile([128, 64], f32, tag="pbig")
                    nc.tensor.transpose(out=pt2[:], in_=ext[:], identity=identf[:64, :64])
                    nc.vector.tensor_copy(out=stg[:, b * 128 + 96:b * 128 + 97], in_=pt2[:, 32:33])
                    nc.vector.tensor_copy(out=stg[:, b * 128 + 97:b * 128 + 98], in_=pt2[:, 0:1])
                nc.vector.tensor_copy(out=stg[:, 98:99], in_=stg[:, 225:226])
                nc.sync.dma_start(out=tab_slice[n0:n0 + nreal, :], in_=stg[:nreal, :])
            nc.gpsimd.collective_compute(
                "AllGather", mybir.AluOpType.bypass,
                replica_groups=[list(range(NCORES))],
                ins=[tab_slice[:]], outs=[tab_sh[:]],
            )
            nc.sync.dma_start(out=tab[:], in_=tab_sh[:])

            # ---------- edge phase ----------
            for w in range(0 if not SKIP_EDGE else NWIN, NWIN):
                aggp = {}
                first = {b: True for b in range(2)}
                nagg = {b: 0 for b in range(2)}
                tot = {b: sum(int(Kf[(w * 2 + h) * 2 + s]) for h in range(2) for s in range(2))
                       for b in range(2)}
                for b in range(2):
                    aggp[b] = psG.tile([97, WIN], f32, tag=f"agg{b}", name=f"agg{b}")
                # base lhsT per branch for this window (hr = h @ Wr computed here)
                basel = {}
                for b in range(2):
                    phr = psA.tile([96, WIN], f32, tag="pbig")
                    nc.tensor.matmul(out=phr[:], lhsT=W(f'Wr_{l}_{b}'),
                                     rhs=h_T[b][:, w * WIN:(w + 1) * WIN],
                                     start=True, stop=True)
                    hrs = sp.tile([96, WIN], f32, tag="hrs")
                    nc.vector.tensor_copy(out=hrs[:], in_=phr[:])
                    pt = psA.tile([WIN, 96], f32, tag="pbig")
                    nc.tensor.transpose(out=pt[:], in_=hrs[:], identity=identf[:96, :96])
                    bl = sp.tile([128, 96], bf16, tag=f"basel{b}", name=f"basel{b}")
                    nc.vector.memset(bl[:], 0.0)
                    nc.vector.tensor_copy(out=bl[:WIN, :], in_=pt[:])
                    nc.vector.tensor_copy(out=bl[WIN:WIN + 8, :], in_=W(f'We_{l}_{b}', 8, 96))
                    basel[b] = bl
                for h in range(2):
                    for s in range(2):
                        si = (w * 2 + h) * 2 + s
                        Ks = int(Kf[si])
                        sl0 = int(sec_slot[si])
                        nsl = Ks * 128
                        g = gpl.tile([128, KMAX, TROW], bf16, tag="gath")
                        if SKIP_GATHER:
                            nc.vector.memset(g[:, :Ks, :], 0.0)
                        else:
                            nc.gpsimd.dma_gather(
                                out_ap=g[:, :Ks, :],
                                in_ap=tab[SPLIT:, :] if s else tab[:SPLIT, :],
                                idxs_ap=gw_t[:, sl0 // 16:(sl0 + nsl) // 16],
                                num_idxs=nsl, num_idxs_reg=nsl, elem_size=TROW)
                        # compact edge table slice: rows 0-7 ea, row 8 dstcol
                        eavs = sp.tile([8, KMAX * 128], bf16, tag="eavs")
                        nc.sync.dma_start(out=eavs[:, :nsl], in_=deav[0:8, sl0:sl0 + nsl])
                        dcw = sp.tile([1, KMAX * 128], bf16, tag="dcw")
                        nc.sync.dma_start(out=dcw[:, :nsl], in_=deav[8:9, sl0:sl0 + nsl])
                        # build Rt on device: rows 0-95 one-hot(dstcol), 96-103 ea
                        Rt = sp.tile([128, KMAX * 128], bf16, tag="Rt")
                        for c0 in range(0, nsl, 512):
                            cw = min(512, nsl - c0)
                            pbc = psA.tile([128, 512], f32, tag="pbig")
                            nc.tensor.matmul(out=pbc[:, :cw], lhsT=ones1[:],
                                             rhs=dcw[0:1, c0:c0 + cw], start=True, stop=True)
                            nc.vector.tensor_scalar(out=Rt[0:96, c0:c0 + cw],
                                                    in0=pbc[0:96, :cw], scalar1=iota96,
                                                    scalar2=None, op0=OP.is_equal)
                            nc.vector.tensor_copy(out=Rt[96:104, c0:c0 + cw],
                                                  in_=eavs[0:8, c0:c0 + cw])
                        # per-slot dst col within half-window: transpose dcw
                        # blocks to partitions, -48h (invalid 147 -> 147/99)
                        dcsec = sp.tile([128, KMAX], f32, tag="dcs")
                        for j in range(Ks):
                            pt1 = psA.tile([128, 1], bf16, tag="pbig")
                            nc.tensor.transpose(out=pt1[:],
                                                in_=dcw[0:1, j * 128:(j + 1) * 128],
                                                identity=ident[:1, :1])
                            nc.vector.tensor_scalar(out=dcsec[:, j:j + 1], in0=pt1[:],
                                                    scalar1=float(-48 * h), scalar2=None,
                                                    op0=OP.add)
                        lgp = psp.tile([128, 16], f32, tag="lgp", bufs=1)
                        for j0 in range(0, Ks, 4):
                            jw = min(4, Ks - j0)
                            for b in range(2):
                                mps = psp.tile([96, 512], f32, tag="mps")
                                nc.tensor.matmul(out=mps[:, :jw * 128], lhsT=basel[b][:],
                                                 rhs=Rt[:, j0 * 128:(j0 + jw) * 128],
                                                 start=True, stop=False)
                                for dj in range(jw):
                                    j = j0 + dj
                                    nc.tensor.matmul(out=mps[:, dj * 128:(dj + 1) * 128],
                                                     lhsT=g[:, j, b * 128:b * 128 + 96],
                                                     rhs=ident[:], start=False,
                                                     stop=(dj == jw - 1),
                                                     skip_group_check=True)
                                am = sp.tile([96, 512], bf16, tag="am")
                                nc.scalar.activation(out=am[:, :jw * 128],
                                                     in_=mps[:, :jw * 128], func=AF.Abs)
                                for dj in range(jw):
                                    j = j0 + dj
                                    nc.tensor.matmul(out=lgp[:, 2 * j + b:2 * j + b + 1],
                                                     lhsT=am[:, dj * 128:(dj + 1) * 128],
                                                     rhs=att04[(l, b)][:],
                                                     start=(j == 0 and b == 0), stop=False,
                                                     skip_group_check=True)
                        # += 0.6*ea.(We@att) per branch (cols 2j|2j+1), on device
                        for j in range(Ks):
                            nc.tensor.matmul(out=lgp[:, 2 * j:2 * j + 2],
                                             lhsT=eavs[0:8, j * 128:(j + 1) * 128],
                                             rhs=Vt[l][:], start=False, stop=(j == Ks - 1),
                                             skip_group_check=True)
                        exw = sp.tile([128, 16], f32, tag="exw")
                        nc.scalar.activation(out=exw[:, :2 * Ks], in_=lgp[:, :2 * Ks],
                                             func=AF.Exp)
                        nc.vector.tensor_tensor(
                            out=exw[:, :2 * Ks].rearrange("p (j b) -> p j b", b=2),
                            in0=exw[:, :2 * Ks].rearrange("p (j b) -> p j b", b=2),
                            in1=g[:, :Ks, 97:99], op=OP.mult)
                        for j in range(Ks):
                            for b in range(2):
                                es = sp.tile([128, HALF], bf16, tag="es")
                                nc.vector.tensor_scalar(
                                    out=es[:], in0=iota_t, scalar1=dcsec[:, j:j + 1],
                                    scalar2=exw[:, 2 * j + b:2 * j + b + 1],
                                    op0=OP.is_equal, op1=OP.mult)
                                nagg[b] += 1
                                nc.tensor.matmul(out=aggp[b][:, h * HALF:(h + 1) * HALF],
                                                 lhsT=g[:, j, b * 128:b * 128 + 97],
                                                 rhs=es[:],
                                                 start=first[b], stop=(nagg[b] == tot[b]),
                                                 skip_group_check=True)
                                first[b] = False
                # finalize window -> h_T
                for b in range(2):
                    num = sp.tile([96, WIN], f32, tag="num")
                    den = sp.tile([1, WIN], f32, tag="den")
                    nc.vector.tensor_copy(out=num[:], in_=aggp[b][:96, :])
                    nc.vector.tensor_scalar(out=den[:], in0=aggp[b][96:97, :],
                                            scalar1=1e-30, scalar2=None, op0=OP.add)
                    rec = sp.tile([1, WIN], f32, tag="rec")
                    nc.vector.reciprocal(out=rec[:], in_=den[:])
                    pb = psp.tile([96, WIN], f32, tag="mps")
                    nc.tensor.matmul(out=pb[:], lhsT=one1[:], rhs=rec[:], start=True, stop=True)
                    tdiv = sp.tile([96, WIN], f32, tag="tdiv")
                    nc.vector.tensor_tensor(out=tdiv[:], in0=num[:], in1=pb[:], op=OP.mult)
                    lin = sp.tile([96, WIN], f32, tag="lin")
                    nc.scalar.activation(out=lin[:], in_=tdiv[:], func=AF.Identity,
                                         bias=W(f'bb_{l}_{b}', 96, 1))
                    ab = sp.tile([96, WIN], f32, tag="ab")
                    nc.scalar.activation(out=ab[:], in_=tdiv[:], func=AF.Abs,
                                         bias=W(f'bb_{l}_{b}', 96, 1))
                    nc.vector.tensor_scalar(out=lin[:], in0=lin[:], scalar1=0.505,
                                            scalar2=None, op0=OP.mult)
                    nc.vector.tensor_scalar(out=ab[:], in0=ab[:], scalar1=0.495,
                                            scalar2=None, op0=OP.mult)
                    nc.vector.tensor_tensor(out=h_T[b][:, w * WIN:(w + 1) * WIN],
                                            in0=lin[:], in1=ab[:], op=OP.add)

        # ---------- head ----------
        hid_T = [wp.tile([128, NCH * 128], f32, tag=f"hw{p}", name=f"hid{p}") for p in range(2)]
        for cs in range(0, NCH * 128, 512):
            ce = min(cs + 512, NCH * 128)
            w_ = ce - cs
            pf = psA.tile([96, 512], f32, tag="pbig")
            nc.tensor.matmul(out=pf[:, :w_], lhsT=W('fusion_Wt'),
                             rhs=h_T[0][:, cs:ce], start=True, stop=False)
            nc.tensor.matmul(out=pf[:, :w_], lhsT=W('fusion_Wb'),
                             rhs=h_T[1][:, cs:ce], start=False, stop=True)
            fus = sp.tile([96, 512], f32, tag="fus")
            lin = sp.tile([96, 512], f32, tag="flin")
            nc.scalar.activation(out=lin[:, :w_], in_=pf[:, :w_], func=AF.Identity,
                                 bias=W('fusion_b', 96, 1))
            ab = sp.tile([96, 512], f32, tag="fab")
            nc.scalar.activation(out=ab[:, :w_], in_=pf[:, :w_], func=AF.Abs,
                                 bias=W('fusion_b', 96, 1))
            nc.vector.tensor_scalar(out=lin[:, :w_], in0=lin[:, :w_], scalar1=0.505,
                                    scalar2=None, op0=OP.mult)
            nc.vector.tensor_scalar(out=ab[:, :w_], in0=ab[:, :w_], scalar1=0.495,
                                    scalar2=None, op0=OP.mult)
            nc.vector.tensor_tensor(out=fus[:, :w_], in0=lin[:, :w_], in1=ab[:, :w_],
                                    op=OP.add)
            for p, (wk, bk) in enumerate([('pred_W1a', 'pred_b1a'), ('pred_W1b', 'pred_b1b')]):
                ph = psA.tile([128, 512], f32, tag="pbig")
                nc.tensor.matmul(out=ph[:, :w_], lhsT=W(wk, 96, 128), rhs=fus[:, :w_],
                                 start=True, stop=True)
                l2 = sp.tile([128, 512], f32, tag=f"l2{p}")
                a2 = sp.tile([128, 512], f32, tag=f"a2{p}")
                nc.scalar.activation(out=l2[:, :w_], in_=ph[:, :w_], func=AF.Identity,
                                     bias=W(bk, 128, 1))
                nc.scalar.activation(out=a2[:, :w_], in_=ph[:, :w_], func=AF.Abs,
                                     bias=W(bk, 128, 1))
                nc.vector.tensor_scalar(out=l2[:, :w_], in0=l2[:, :w_], scalar1=0.505,
                                        scalar2=None, op0=OP.mult)
                nc.vector.tensor_scalar(out=a2[:, :w_], in0=a2[:, :w_], scalar1=0.495,
                                        scalar2=None, op0=OP.mult)
                nc.vector.tensor_tensor(out=hid_T[p][:, cs:ce], in0=l2[:, :w_],
                                        in1=a2[:, :w_], op=OP.add)
        for ch in range(NCH):
            n0 = ch * 128
            nreal = max(0, min(NLOC - n0, 128))
            if nreal == 0:
                continue
            po = psp.tile([128, 2], f32, tag="mps")
            nc.tensor.matmul(out=po[:], lhsT=hid_T[0][:, n0:n0 + 128],
                             rhs=W('pred_W2a', 128, 2), start=True, stop=False)
            nc.tensor.matmul(out=po[:], lhsT=hid_T[1][:, n0:n0 + 128],
                             rhs=W('pred_W2b', 128, 2), start=False, stop=True)
            ot = sp.tile([128, 2], f32, tag="ot")
            nc.vector.tensor_tensor(out=ot[:], in0=po[:], in1=W('pred_b2', 128, 2), op=OP.add)
            nc.sync.dma_start(out=dout[n0:n0 + nreal, :], in_=ot[:nreal, :])

    nc.compile()
    return nc


def _make_runner(nc):
    """Build (once) a cached jitted shard_map wrapper around the compiled
    Bass module — same lowering as bass2jax.run_bass_via_pjrt, but the jit
    object is reused across calls so warm calls skip retrace/recompile."""
    import jax
    import jax.core as jcore
    from jax.experimental.shard_map import shard_map
    from jax.sharding import Mesh, PartitionSpec
    from concourse import bass2jax, mybir
    bass2jax.install_neuronx_cc_hook()

    partition_name = nc.partition_id_tensor.name if nc.partition_id_tensor else None
    in_names, out_names, out_avals, zero_shapes = [], [], [], []
    for alloc in nc.m.functions[0].allocations:
        if not isinstance(alloc, mybir.MemoryLocationSet):
            continue
        name = alloc.memorylocations[0].name
        if alloc.kind == "ExternalInput":
            if name != partition_name:
                in_names.append(name)
        elif alloc.kind == "ExternalOutput":
            shape = tuple(alloc.tensor_shape)
            dtype = mybir.dt.np(alloc.dtype)
            out_names.append(name)
            out_avals.append(jcore.ShapedArray(shape, dtype))
            zero_shapes.append((shape, dtype))
    n_params = len(in_names)
    n_outs = len(out_avals)
    all_in = list(in_names) + list(out_names)
    if partition_name is not None:
        all_in.append(partition_name)
    donate = tuple(range(n_params, n_params + n_outs))

    dbg_name = None
    if nc.dbg_addr is not None:
        assert not nc.dbg_callbacks
        dbg_name = nc.dbg_addr.name

    def _body(*args):
        operands = list(args)
        if partition_name is not None:
            operands.append(bass2jax.partition_id_tensor())
        outs = bass2jax._bass_exec_p.bind(
            *operands, out_avals=tuple(out_avals), in_names=tuple(all_in),
            out_names=tuple(out_names), lowering_input_output_aliases=(),
            sim_require_finite=True, sim_require_nnan=True, nc=nc)
        return tuple(outs)

    devices = jax.devices()[:NCORES]
    mesh = Mesh(np.asarray(devices), ("core",))
    in_specs = (PartitionSpec("core"),) * (n_params + n_outs)
    out_specs = (PartitionSpec("core"),) * n_outs
    fn = jax.jit(shard_map(_body, mesh=mesh, in_specs=in_specs,
                           out_specs=out_specs, check_rep=False),
                 donate_argnums=donate, keep_unused=True)
    return dict(fn=fn, in_names=in_names, out_names=out_names,
                out_avals=out_avals, zero_shapes=zero_shapes, dbg_name=dbg_name)


def _execute(runner, in_maps):
    n = len(in_maps)
    if runner['dbg_name'] is not None:
        z = np.zeros((1, 2), np.uint32)
        in_maps = [{**m, runner['dbg_name']: z} for m in in_maps]
    concat_in = [np.concatenate([np.asarray(in_maps[c][nm]) for c in range(n)], axis=0)
                 for nm in runner['in_names']]
    zeros = [np.zeros((n * s[0], *s[1:]), dt) for (s, dt) in runner['zero_shapes']]
    outs = runner['fn'](*concat_in, *zeros)
    return [{nm: np.asarray(outs[i]).reshape(n, *runner['out_avals'][i].shape)[c]
             for i, nm in enumerate(runner['out_names'])} for c in range(n)]


def _in_maps(x, pp, blob):
    b96, b128 = blob
    xb = np.ascontiguousarray(x.astype(BF16))
    maps = []
    for c in range(NCORES):
        maps.append({'x': xb[c * NLOC:(c + 1) * NLOC],
                     'eav': pp['eav'][c],
                     'gw': pp['gw'][c],
                     'wb96': b96,
                     'wb128': b128})
    return maps


def kernel(**inputs):
    x = np.asarray(inputs['x'], np.float32)
    ei = np.asarray(inputs['edge_index'])
    ea = np.asarray(inputs['edge_attr'], np.float32)
    pp = _host_prep(x, ei, ea)
    if _CACHE.get('NSLOT') != pp['NSLOT']:
        _CACHE['nc'] = build_kernel(pp)
        _CACHE['runner'] = _make_runner(_CACHE['nc'])
        _CACHE['NSLOT'] = pp['NSLOT']
    blob = _wblob(inputs)
    res = _execute(_CACHE['runner'], _in_maps(x, pp, blob))
    out = np.concatenate([res[c]['out'] for c in range(NCORES)], axis=0)
    return out.astype(np.float32)


# revision 35
# speedup vs baseline: 18.9368x; 1.3674x over previous
"""BiLevelGAT (2-branch x 3-layer GATv2, N=50000, E=500000, D=96) on 8 TRN2 cores.

Sharding: nodes + incoming edges partitioned by dst; per-layer AllGather of a
bf16 per-node table [hl_loc 96|1|w_loc|w_glob|pad29|hl_glob 96|1|w_glob|pad30]
(512B rows) gathered per edge by src.

Math: lrelu(x) = 0.6x+0.4|x| splits the GATv2 logit into linear terms (per-src
w=exp(0.6*att.hl) folded into the softmax weight; per-dst term cancels in
softmax; per-edge ea term computed on device from a compact [9, NSLOT] edge
table: rows 0-7 ea, row 8 dst-col) plus 0.4*att.|m| computed on device.
Softmax max-subtraction skipped (logits O(1), fp32 safe).

The per-edge one-hot "R" matrix (dst selector + ea rows used as matmul rhs to
form m = hr[dst] + ea@We) is built on device per section: a rank-1 broadcast
matmul of the dst-col row followed by an is_equal against a partition iota.
Inputs per core are ~5MB (x bf16, edge table bf16, gather idx, dst cols,
one packed weight blob) so the host->device transfer over the axon tunnel
stays small; the jitted shard_map wrapper is cached across calls.
"""
import sys
sys.path.insert(0, '/opt/trn_rl_repo')
import numpy as np
import ml_dtypes

BF16 = ml_dtypes.bfloat16

N, E, D, EDIM, L, DENSE, OUT = 50000, 500000, 96, 8, 3, 256, 2
NCORES = 8
NLOC = N // NCORES            # 6250
WIN, HALF = 96, 48
NWIN = (NLOC + WIN - 1) // WIN  # 66
NPAD = NWIN * WIN
NCH = (NPAD + 127) // 128     # chunks of 128 (PASS A / table)
SPLIT = 32768
TROW = 256
KMAX = 6

_CACHE = {}

# ---- packed weight blobs ----
# blob96 [96, WB96]: 96-row tensors; We_{l,b} stacked at rows 8*(2l+b);
# V_l at row base 32*l (matmul rhs needs base partition 0/32/64).
# blob128 [128, WB128]: 128-row tensors + iotaf.
WOFF = {}   # name -> (row0, col0)
_col96 = 0
_col128 = 0
def _a96(name, width, row=0):
    global _col96
    WOFF[name] = (row, _col96)
    _col96 += width
def _a128(name, width):
    global _col128
    WOFF[name] = (0, _col128)
    _col128 += width
for _l in range(L):
    for _b in range(2):
        _a96(f'Wl_{_l}_{_b}', 96)
        _a96(f'Wr_{_l}_{_b}', 96)
        _a96(f'att_{_l}_{_b}', 1)
        _a96(f'bb_{_l}_{_b}', 1)
# partition base of any access must be a multiple of 32: stack We six-up
# in two 96-col groups at row bases 0/32/64
for _i in range(6):
    _l, _b = divmod(_i, 2)
    WOFF[f'We_{_l}_{_b}'] = (32 * (_i % 3), _col96 + 96 * (_i // 3))
_col96 += 192
for _l in range(L):
    WOFF[f'V_{_l}'] = (32 * _l, _col96)                       # shared 2 cols
_col96 += 2
_a96('fusion_Wt', 96)
_a96('fusion_Wb', 96)
_a96('fusion_b', 1)
_a96('pred_W1a', 128)
_a96('pred_W1b', 128)
_a96('iota96', 1)                     # rows 0-95 = arange(96)
WB96 = _col96
_a128('pred_b1a', 1)
_a128('pred_b1b', 1)
_a128('pred_W2a', 2)
_a128('pred_W2b', 2)
_a128('pred_b2', 2)
_a128('iotaf', 48)                    # all 128 rows = arange(48)
_a128('iota128', 1)                   # rows = arange(128)
WB128 = _col128
B128 = {'pred_b1a', 'pred_b1b', 'pred_W2a', 'pred_W2b', 'pred_b2', 'iotaf',
        'iota128'}
EASCALE = 32.0
XSCALE = 32.0


def _host_prep(x, edge_index, edge_attr):
    src = edge_index[0].astype(np.int64)
    dst = edge_index[1].astype(np.int64)
    mean_ea = edge_attr.mean(0).astype(np.float32)
    loop = np.arange(N, dtype=np.int64)
    src_a = np.concatenate([src, loop])
    dst_a = np.concatenate([dst, loop])
    ea_a = np.concatenate([edge_attr.astype(np.float32),
                           np.broadcast_to(mean_ea, (N, EDIM))], 0)

    owner = dst_a // NLOC
    dloc = dst_a - owner * NLOC
    win = dloc // WIN
    half = (dloc % WIN) // HALF
    stream = (src_a >= SPLIT).astype(np.int64)

    per_core = []
    secs = np.zeros((NCORES, NWIN, 2, 2), np.int64)
    for c in range(NCORES):
        m = owner == c
        s_c, d_c, e_c = src_a[m], dloc[m], ea_a[m]
        w_c, h_c, st_c = win[m], half[m], stream[m]
        sec = ((w_c * 2 + h_c) * 2 + st_c)
        order = np.argsort(sec * NLOC + d_c, kind='stable')
        s_c, d_c, e_c, sec = s_c[order], d_c[order], e_c[order], sec[order]
        st_c = st_c[order]
        per_core.append((s_c, d_c, e_c, sec, st_c))
        secs[c] = np.bincount(sec, minlength=NWIN * 4).reshape(NWIN, 2, 2)

    K = np.maximum((secs.max(0) + 127) // 128, 1)       # [NWIN, 2, 2]
    assert K.max() <= KMAX
    Kf = K.reshape(-1)
    sec_slot = np.zeros(NWIN * 4 + 1, np.int64)
    np.cumsum(Kf * 128, out=sec_slot[1:])
    NSLOT = int(sec_slot[-1])

    gidx = np.zeros((NCORES, NSLOT), np.int16)
    eav = np.zeros((NCORES, 9, NSLOT), np.float32)
    # invalid slots: dstcol=147 misses the 96-wide one-hot AND (after -48h)
    # the 48-wide es window -> no contribution
    eav[:, 8, :] = 147.0

    for c in range(NCORES):
        s_c, d_c, e_c, sec, st_c = per_core[c]
        counts = np.bincount(sec, minlength=NWIN * 4)
        starts = np.concatenate([[0], np.cumsum(counts)])[:-1]
        pos = np.arange(len(s_c)) - starts[sec]
        slot = sec_slot[sec] + pos
        gidx[c, slot] = (s_c - st_c * SPLIT).astype(np.int16)
        eav[c, 0:8, slot] = e_c  # advanced idx puts slot axis first: (nedge, 8)
        eav[c, 8, slot] = (d_c % WIN).astype(np.float32)

    gw = np.ascontiguousarray(
        gidx.reshape(NCORES, -1, 16).transpose(0, 2, 1))     # [NCORES, 16, NSLOT//16]

    ea_q = np.clip(np.rint(eav[:, 0:8, :] * EASCALE), -127, 127).astype(np.int8)
    return dict(K=K, Kf=Kf, sec_slot=sec_slot, NSLOT=NSLOT, NSEC=NWIN * 4,
                gw=gw, ea8=ea_q,
                dcw=eav[:, 8, :].astype(np.uint8).reshape(NCORES, 1, NSLOT))


def _wblob(w):
    b96 = np.zeros((96, WB96), np.float32)
    b128 = np.zeros((128, WB128), np.float32)
    def put(name, arr):
        a = np.asarray(arr, np.float32)
        if a.ndim == 1:
            a = a.reshape(-1, 1)
        r0, c0 = WOFF[name]
        dst = b96 if name not in B128 else b128
        dst[r0:r0 + a.shape[0], c0:c0 + a.shape[1]] = a
    for l in range(L):
        V = np.zeros((8, 2), np.float32)
        # x ships as int8 * XSCALE; fold 1/XSCALE into the layer-0 weights
        xs = XSCALE if l == 0 else 1.0
        for b, p in enumerate(['local', 'global']):
            put(f'Wl_{l}_{b}', np.asarray(w[f'{p}_Wl'][l], np.float32) / xs)
            put(f'Wr_{l}_{b}', np.asarray(w[f'{p}_Wr'][l], np.float32) / xs)
            put(f'att_{l}_{b}', w[f'{p}_att'][l])
            put(f'bb_{l}_{b}', w[f'{p}_b'][l])
            # ea ships as int8 * EASCALE; fold 1/EASCALE into We and V
            put(f'We_{l}_{b}', np.asarray(w[f'{p}_We'][l], np.float32) / EASCALE)
            V[:, b] = (0.6 / EASCALE) * (np.asarray(w[f'{p}_We'][l], np.float32)
                                         @ np.asarray(w[f'{p}_att'][l], np.float32))
        put(f'V_{l}', V)
    put('fusion_Wt', w['fusion_W'][:96])
    put('fusion_Wb', w['fusion_W'][96:])
    put('fusion_b', w['fusion_b'])
    put('pred_W1a', w['pred_W1'][:, :128])
    put('pred_W1b', w['pred_W1'][:, 128:])
    put('pred_b1a', w['pred_b1'][:128])
    put('pred_b1b', w['pred_b1'][128:])
    put('pred_W2a', w['pred_W2'][:128])
    put('pred_W2b', w['pred_W2'][128:])
    put('pred_b2', np.broadcast_to(np.asarray(w['pred_b2']).reshape(1, 2), (128, 2)))
    put('iotaf', np.broadcast_to(np.arange(48, dtype=np.float32), (128, 48)))
    put('iota96', np.arange(96, dtype=np.float32))
    put('iota128', np.arange(128, dtype=np.float32))
    return b96, b128


def build_kernel(pp):
    import os as _os
    SKIP_EDGE = _os.environ.get('SKIP_EDGE', '0') == '1'
    SKIP_GATHER = _os.environ.get('SKIP_GATHER', '0') == '1'
    from concourse import mybir, bacc
    import concourse.tile as tile
    Kf, sec_slot, NSLOT = pp['Kf'], pp['sec_slot'], pp['NSLOT']
    f32, bf16, i16 = mybir.dt.float32, mybir.dt.bfloat16, mybir.dt.int16
    AF = mybir.ActivationFunctionType
    OP = mybir.AluOpType

    i8, u8 = mybir.dt.int8, mybir.dt.uint8
    nc = bacc.Bacc("TRN2", target_bir_lowering=False, debug=False, num_devices=NCORES)
    dx = nc.dram_tensor("x", [NLOC, D], i8, kind="ExternalInput")
    dea8 = nc.dram_tensor("ea8", [8, NSLOT], i8, kind="ExternalInput")
    ddcw = nc.dram_tensor("dcw", [1, NSLOT], u8, kind="ExternalInput")
    dgw = nc.dram_tensor("gw", [16, NSLOT // 16], i16, kind="ExternalInput")
    dwb96 = nc.dram_tensor("wb96", [96, WB96], f32, kind="ExternalInput")
    dwb128 = nc.dram_tensor("wb128", [128, WB128], f32, kind="ExternalInput")
    dout = nc.dram_tensor("out", [NLOC, OUT], f32, kind="ExternalOutput")

    tab_slice = nc.dram_tensor("tab_slice", [NLOC, TROW], bf16)
    tab_sh = nc.dram_tensor("tab_sh", [N, TROW], bf16, addr_space="Shared")
    tab = nc.dram_tensor("tab", [N, TROW], bf16)

    def wo(name, rows=96, width=None):
        w_ = width if width is not None else 96
        return (WOFF[name], WOFF[name] + w_, rows)

    with tile.TileContext(nc) as tc:
      with (tc.tile_pool(name="const", bufs=1) as cp,
            tc.tile_pool(name="hp", bufs=1) as hp,
            tc.tile_pool(name="wp", bufs=1) as wp,
            tc.tile_pool(name="sp", bufs=3) as sp,
            tc.tile_pool(name="gpool", bufs=2) as gpl,
            tc.tile_pool(name="ps", bufs=2, space="PSUM") as psp,
            tc.tile_pool(name="psA", bufs=2, space="PSUM") as psA,
            tc.tile_pool(name="psagg", bufs=1, space="PSUM") as psG):

        ident = cp.tile([128, 128], bf16)
        nc.sync.dma_start(out=ident[:], in_=nc.inline_tensor(np.eye(128, dtype=BF16), name="idb").ap())
        identf = cp.tile([128, 128], f32)
        nc.sync.dma_start(out=identf[:], in_=nc.inline_tensor(np.eye(128, dtype=np.float32), name="idf").ap())
        wb96_t = cp.tile([96, WB96], f32)
        nc.sync.dma_start(out=wb96_t[:], in_=dwb96[:])
        wb128_t = cp.tile([128, WB128], f32)
        nc.sync.dma_start(out=wb128_t[:], in_=dwb128[:])
        gw_t = cp.tile([128, NSLOT // 16], i16)
        for k in range(8):
            nc.sync.dma_start(out=gw_t[16 * k:16 * (k + 1), :], in_=dgw[:])

        def W(name, rows=96, width=96):
            r0, c0 = WOFF[name]
            t = wb128_t if name in B128 else wb96_t
            return t[r0:r0 + rows, c0:c0 + width]

        iota_t = wb128_t[:, WOFF['iotaf'][1]:WOFF['iotaf'][1] + 48]
        iota96 = wb96_t[:96, WOFF['iota96'][1]:WOFF['iota96'][1] + 1]

        one1 = cp.tile([1, 96], f32)
        nc.vector.memset(one1[:], 1.0)
        ones1 = cp.tile([1, 128], bf16)
        nc.vector.memset(ones1[:], 1.0)
        att04 = {}
        for l in range(L):
            for b in range(2):
                att04[(l, b)] = cp.tile([96, 1], bf16, tag=f"att04_{l}_{b}", name=f"att04_{l}_{b}")
                nc.vector.tensor_scalar(out=att04[(l, b)][:], in0=W(f'att_{l}_{b}', 96, 1),
                                        scalar1=0.4, scalar2=None, op0=OP.mult)
        Vt = {}
        for l in range(L):
            Vt[l] = cp.tile([8, 2], bf16, tag=f"V_{l}", name=f"V_{l}")
            nc.vector.tensor_copy(out=Vt[l][:], in_=W(f'V_{l}', 8, 2))

        # h_T feature-major [96, NPAD] (cols beyond NLOC are pad)
        h_T = [hp.tile([96, NCH * 128], f32, tag=f"h{b}", name=f"h{b}") for b in range(2)]
        for ch in range(NCH):
            n0 = ch * 128
            nreal = max(0, min(NLOC - n0, 128))
            xin8 = sp.tile([128, 128], i8, tag="xin8")
            nc.vector.memset(xin8[:], 0)
            if nreal > 0:
                nc.sync.dma_start(out=xin8[:nreal, :96], in_=dx[n0:n0 + nreal, :])
            xin = sp.tile([128, 128], bf16, tag="xin")
            nc.vector.tensor_copy(out=xin[:], in_=xin8[:])
            pt = psA.tile([128, 128], bf16, tag="pbig")
            nc.tensor.transpose(out=pt[:], in_=xin[:], identity=ident[:])
            for b in range(2):
                nc.vector.tensor_copy(out=h_T[b][:, n0:n0 + 128], in_=pt[:96, :])

        hw_T = [wp.tile([96, NCH * 128], f32, tag=f"hw{b}", name=f"hw{b}") for b in range(2)]

        for l in range(L):
            # ---------- PASS A ----------
            for b in range(2):
                for cs in range(0, NCH * 128, 512):
                    ce = min(cs + 512, NCH * 128)
                    w_ = ce - cs
                    pl = psA.tile([96, 512], f32, tag="pbig")
                    nc.tensor.matmul(out=pl[:, :w_], lhsT=W(f'Wl_{l}_{b}'),
                                     rhs=h_T[b][:, cs:ce], start=True, stop=True)
                    nc.vector.tensor_copy(out=hw_T[b][:, cs:ce], in_=pl[:, :w_])
            # table slice + allgather
            for ch in range(NCH):
                n0 = ch * 128
                nreal = max(0, min(NLOC - n0, 128))
                if nreal == 0:
                    continue
                stg = sp.tile([128, TROW], bf16, tag="stg")
                nc.vector.memset(stg[:], 0.0)
                for b in range(2):
                    pt = psA.tile([128, 128], f32, tag="pbig")
                    nc.tensor.transpose(out=pt[:, :96], in_=hw_T[b][:, n0:n0 + 128],
                                        identity=identf[:96, :96])
                    nc.vector.tensor_copy(out=stg[:, b * 128:b * 128 + 96], in_=pt[:, :96])
                    # w = exp(0.6*att.hl) for this chunk; ones at ext row 32
                    pphi = psA.tile([1, 128], f32, tag="pbig")
                    nc.tensor.matmul(out=pphi[:], lhsT=W(f'att_{l}_{b}', 96, 1),
                                     rhs=hw_T[b][:, n0:n0 + 128], start=True, stop=True)
                    ext = sp.tile([64, 128], f32, tag="ext")
                    nc.scalar.activation(out=ext[0:1, :], in_=pphi[:], func=AF.Exp, scale=0.6)
                    nc.vector.memset(ext[32:33, :], 1.0)
                    pt2 = psA.tile([128, 64], f32, tag="pbig")
                    nc.tensor.transpose(out=pt2[:], in_=ext[:], identity=identf[:64, :64])
                    nc.vector.tensor_copy(out=stg[:, b * 128 + 96:b * 128 + 97], in_=pt2[:, 32:33])
                    nc.vector.tensor_copy(out=stg[:, b * 128 + 97:b * 128 + 98], in_=pt2[:, 0:1])
                nc.vector.tensor_copy(out=stg[:, 98:99], in_=stg[:, 225:226])
                nc.sync.dma_start(out=tab_slice[n0:n0 + nreal, :], in_=stg[:nreal, :])
            nc.gpsimd.collective_compute(
                "AllGather", mybir.AluOpType.bypass,
                replica_groups=[list(range(NCORES))],
                ins=[tab_slice[:]], outs=[tab_sh[:]],
            )
            nc.sync.dma_start(out=tab[:], in_=tab_sh[:])

            # ---------- edge phase ----------
            for w in range(0 if not SKIP_EDGE else NWIN, NWIN):
                aggp = {}
                first = {b: True for b in range(2)}
                nagg = {b: 0 for b in range(2)}
                tot = {b: sum(int(Kf[(w * 2 + h) * 2 + s]) for h in range(2) for s in range(2))
                       for b in range(2)}
                for b in range(2):
                    aggp[b] = psG.tile([97, WIN], f32, tag=f"agg{b}", name=f"agg{b}")
                # base lhsT per branch for this window (hr = h @ Wr computed here)
                basel = {}
                for b in range(2):
                    phr = psA.tile([96, WIN], f32, tag="pbig")
                    nc.tensor.matmul(out=phr[:], lhsT=W(f'Wr_{l}_{b}'),
                                     rhs=h_T[b][:, w * WIN:(w + 1) * WIN],
                                     start=True, stop=True)
                    hrs = sp.tile([96, WIN], f32, tag="hrs")
                    nc.vector.tensor_copy(out=hrs[:], in_=phr[:])
                    pt = psA.tile([WIN, 96], f32, tag="pbig")
                    nc.tensor.transpose(out=pt[:], in_=hrs[:], identity=identf[:96, :96])
                    bl = sp.tile([128, 96], bf16, tag=f"basel{b}", name=f"basel{b}")
                    nc.vector.memset(bl[:], 0.0)
                    nc.vector.tensor_copy(out=bl[:WIN, :], in_=pt[:])
                    nc.vector.tensor_copy(out=bl[WIN:WIN + 8, :], in_=W(f'We_{l}_{b}', 8, 96))
                    basel[b] = bl
                for h in range(2):
                    for s in range(2):
                        si = (w * 2 + h) * 2 + s
                        Ks = int(Kf[si])
                        sl0 = int(sec_slot[si])
                        nsl = Ks * 128
                        g = gpl.tile([128, KMAX, TROW], bf16, tag="gath")
                        if SKIP_GATHER:
                            nc.vector.memset(g[:, :Ks, :], 0.0)
                        else:
                            nc.gpsimd.dma_gather(
                                out_ap=g[:, :Ks, :],
                                in_ap=tab[SPLIT:, :] if s else tab[:SPLIT, :],
                                idxs_ap=gw_t[:, sl0 // 16:(sl0 + nsl) // 16],
                                num_idxs=nsl, num_idxs_reg=nsl, elem_size=TROW)
                        # compact edge table slice (i8/u8 on the wire -> bf16)
                        ea8s = sp.tile([8, KMAX * 128], i8, tag="ea8s")
                        nc.sync.dma_start(out=ea8s[:, :nsl], in_=dea8[:, sl0:sl0 + nsl])
                        eavs = sp.tile([8, KMAX * 128], bf16, tag="eavs")
                        nc.vector.tensor_copy(out=eavs[:, :nsl], in_=ea8s[:, :nsl])
                        dc8s = sp.tile([1, KMAX * 128], u8, tag="dc8s")
                        nc.sync.dma_start(out=dc8s[:, :nsl], in_=ddcw[:, sl0:sl0 + nsl])
                        dcw = sp.tile([1, KMAX * 128], bf16, tag="dcw")
                        nc.vector.tensor_copy(out=dcw[:, :nsl], in_=dc8s[:, :nsl])
                        # build Rt on device: rows 0-95 one-hot(dstcol), 96-103
                        # ea. is_equal covers ALL 128 rows (rows 96-127 can
                        # never match dstcol<=147... rows 96-103 overwritten by
                        # ea below, 104-127 exact zeros) so no partition of Rt
                        # is left uninitialized -- the mps matmul reads all 128
                        # partitions and 0 * NaN-garbage = NaN.
                        Rt = sp.tile([128, KMAX * 128], bf16, tag="Rt")
                        for c0 in range(0, nsl, 512):
                            cw = min(512, nsl - c0)
                            pbc = psA.tile([128, 512], f32, tag="pbig")
                            nc.tensor.matmul(out=pbc[:, :cw], lhsT=ones1[:],
                                             rhs=dcw[0:1, c0:c0 + cw], start=True, stop=True)
                            nc.vector.tensor_scalar(out=Rt[:, c0:c0 + cw],
                                                    in0=pbc[:, :cw],
                                                    scalar1=W('iota128', 128, 1),
                                                    scalar2=None, op0=OP.is_equal)
                            nc.vector.tensor_copy(out=Rt[96:104, c0:c0 + cw],
                                                  in_=eavs[0:8, c0:c0 + cw])
                        # per-slot dst col within half-window: transpose dcw
                        # blocks to partitions, -48h (invalid 147 -> 147/99)
                        dcsec = sp.tile([128, KMAX], f32, tag="dcs")
                        for j in range(Ks):
                            pt1 = psA.tile([128, 1], bf16, tag="pbig")
                            nc.tensor.transpose(out=pt1[:],
                                                in_=dcw[0:1, j * 128:(j + 1) * 128],
                                                identity=ident[:1, :1])
                            nc.vector.tensor_scalar(out=dcsec[:, j:j + 1], in0=pt1[:],
                                                    scalar1=float(-48 * h), scalar2=None,
                                                    op0=OP.add)
                        lgp = psp.tile([128, 16], f32, tag="lgp", bufs=1)
                        for j0 in range(0, Ks, 4):
                            jw = min(4, Ks - j0)
                            for b in range(2):
                                mps = psp.tile([96, 512], f32, tag="mps")
                                nc.tensor.matmul(out=mps[:, :jw * 128], lhsT=basel[b][:],
                                                 rhs=Rt[:, j0 * 128:(j0 + jw) * 128],
                                                 start=True, stop=False)
                                for dj in range(jw):
                                    j = j0 + dj
                                    nc.tensor.matmul(out=mps[:, dj * 128:(dj + 1) * 128],
                                                     lhsT=g[:, j, b * 128:b * 128 + 96],
                                                     rhs=ident[:], start=False,
                                                     stop=(dj == jw - 1),
                                                     skip_group_check=True)
                                am = sp.tile([96, 512], bf16, tag="am")
                                nc.scalar.activation(out=am[:, :jw * 128],
                                                     in_=mps[:, :jw * 128], func=AF.Abs)
                                for dj in range(jw):
                                    j = j0 + dj
                                    nc.tensor.matmul(out=lgp[:, 2 * j + b:2 * j + b + 1],
                                                     lhsT=am[:, dj * 128:(dj + 1) * 128],
                                                     rhs=att04[(l, b)][:],
                                                     start=(j == 0 and b == 0), stop=False,
                                                     skip_group_check=True)
                        # += 0.6*ea.(We@att) per branch (cols 2j|2j+1), on device
                        for j in range(Ks):
                            nc.tensor.matmul(out=lgp[:, 2 * j:2 * j + 2],
                                             lhsT=eavs[0:8, j * 128:(j + 1) * 128],
                                             rhs=Vt[l][:], start=False, stop=(j == Ks - 1),
                                             skip_group_check=True)
                        exw = sp.tile([128, 16], f32, tag="exw")
                        nc.scalar.activation(out=exw[:, :2 * Ks], in_=lgp[:, :2 * Ks],
                                             func=AF.Exp)
                        nc.vector.tensor_tensor(
                            out=exw[:, :2 * Ks].rearrange("p (j b) -> p j b", b=2),
                            in0=exw[:, :2 * Ks].rearrange("p (j b) -> p j b", b=2),
                            in1=g[:, :Ks, 97:99], op=OP.mult)
                        for j in range(Ks):
                            for b in range(2):
                                es = sp.tile([128, HALF], bf16, tag="es")
                                nc.vector.tensor_scalar(
                                    out=es[:], in0=iota_t, scalar1=dcsec[:, j:j + 1],
                                    scalar2=exw[:, 2 * j + b:2 * j + b + 1],
                                    op0=OP.is_equal, op1=OP.mult)
                                nagg[b] += 1
                                nc.tensor.matmul(out=aggp[b][:, h * HALF:(h + 1) * HALF],
                                                 lhsT=g[:, j, b * 128:b * 128 + 97],
                                                 rhs=es[:],
                                                 start=first[b], stop=(nagg[b] == tot[b]),
                                                 skip_group_check=True)
                                first[b] = False
                # finalize window -> h_T
                for b in range(2):
                    num = sp.tile([96, WIN], f32, tag="num")
                    den = sp.tile([1, WIN], f32, tag="den")
                    nc.vector.tensor_copy(out=num[:], in_=aggp[b][:96, :])
                    nc.vector.tensor_scalar(out=den[:], in0=aggp[b][96:97, :],
                                            scalar1=1e-30, scalar2=None, op0=OP.add)
                    rec = sp.tile([1, WIN], f32, tag="rec")
                    nc.vector.reciprocal(out=rec[:], in_=den[:])
                    pb = psp.tile([96, WIN], f32, tag="mps")
                    nc.tensor.matmul(out=pb[:], lhsT=one1[:], rhs=rec[:], start=True, stop=True)
                    tdiv = sp.tile([96, WIN], f32, tag="tdiv")
                    nc.vector.tensor_tensor(out=tdiv[:], in0=num[:], in1=pb[:], op=OP.mult)
                    lin = sp.tile([96, WIN], f32, tag="lin")
                    nc.scalar.activation(out=lin[:], in_=tdiv[:], func=AF.Identity,
                                         bias=W(f'bb_{l}_{b}', 96, 1))
                    ab = sp.tile([96, WIN], f32, tag="ab")
                    nc.scalar.activation(out=ab[:], in_=tdiv[:], func=AF.Abs,
                                         bias=W(f'bb_{l}_{b}', 96, 1))
                    nc.vector.tensor_scalar(out=lin[:], in0=lin[:], scalar1=0.505,
                                            scalar2=None, op0=OP.mult)
                    nc.vector.tensor_scalar(out=ab[:], in0=ab[:], scalar1=0.495,
                                            scalar2=None, op0=OP.mult)
                    nc.vector.tensor_tensor(out=h_T[b][:, w * WIN:(w + 1) * WIN],
                                            in0=lin[:], in1=ab[:], op=OP.add)

        # ---------- head ----------
        hid_T = [wp.tile([128, NCH * 128], f32, tag=f"hw{p}", name=f"hid{p}") for p in range(2)]
        for cs in range(0, NCH * 128, 512):
            ce = min(cs + 512, NCH * 128)
            w_ = ce - cs
            pf = psA.tile([96, 512], f32, tag="pbig")
            nc.tensor.matmul(out=pf[:, :w_], lhsT=W('fusion_Wt'),
                             rhs=h_T[0][:, cs:ce], start=True, stop=False)
            nc.tensor.matmul(out=pf[:, :w_], lhsT=W('fusion_Wb'),
                             rhs=h_T[1][:, cs:ce], start=False, stop=True)
            fus = sp.tile([96, 512], f32, tag="fus")
            lin = sp.tile([96, 512], f32, tag="flin")
            nc.scalar.activation(out=lin[:, :w_], in_=pf[:, :w_], func=AF.Identity,
                                 bias=W('fusion_b', 96, 1))
            ab = sp.tile([96, 512], f32, tag="fab")
            nc.scalar.activation(out=ab[:, :w_], in_=pf[:, :w_], func=AF.Abs,
                                 bias=W('fusion_b', 96, 1))
            nc.vector.tensor_scalar(out=lin[:, :w_], in0=lin[:, :w_], scalar1=0.505,
                                    scalar2=None, op0=OP.mult)
            nc.vector.tensor_scalar(out=ab[:, :w_], in0=ab[:, :w_], scalar1=0.495,
                                    scalar2=None, op0=OP.mult)
            nc.vector.tensor_tensor(out=fus[:, :w_], in0=lin[:, :w_], in1=ab[:, :w_],
                                    op=OP.add)
            for p, (wk, bk) in enumerate([('pred_W1a', 'pred_b1a'), ('pred_W1b', 'pred_b1b')]):
                ph = psA.tile([128, 512], f32, tag="pbig")
                nc.tensor.matmul(out=ph[:, :w_], lhsT=W(wk, 96, 128), rhs=fus[:, :w_],
                                 start=True, stop=True)
                l2 = sp.tile([128, 512], f32, tag=f"l2{p}")
                a2 = sp.tile([128, 512], f32, tag=f"a2{p}")
                nc.scalar.activation(out=l2[:, :w_], in_=ph[:, :w_], func=AF.Identity,
                                     bias=W(bk, 128, 1))
                nc.scalar.activation(out=a2[:, :w_], in_=ph[:, :w_], func=AF.Abs,
                                     bias=W(bk, 128, 1))
                nc.vector.tensor_scalar(out=l2[:, :w_], in0=l2[:, :w_], scalar1=0.505,
                                        scalar2=None, op0=OP.mult)
                nc.vector.tensor_scalar(out=a2[:, :w_], in0=a2[:, :w_], scalar1=0.495,
                                        scalar2=None, op0=OP.mult)
                nc.vector.tensor_tensor(out=hid_T[p][:, cs:ce], in0=l2[:, :w_],
                                        in1=a2[:, :w_], op=OP.add)
        for ch in range(NCH):
            n0 = ch * 128
            nreal = max(0, min(NLOC - n0, 128))
            if nreal == 0:
                continue
            po = psp.tile([128, 2], f32, tag="mps")
            nc.tensor.matmul(out=po[:], lhsT=hid_T[0][:, n0:n0 + 128],
                             rhs=W('pred_W2a', 128, 2), start=True, stop=False)
            nc.tensor.matmul(out=po[:], lhsT=hid_T[1][:, n0:n0 + 128],
                             rhs=W('pred_W2b', 128, 2), start=False, stop=True)
            ot = sp.tile([128, 2], f32, tag="ot")
            nc.vector.tensor_tensor(out=ot[:], in0=po[:], in1=W('pred_b2', 128, 2), op=OP.add)
            nc.sync.dma_start(out=dout[n0:n0 + nreal, :], in_=ot[:nreal, :])

    nc.compile()
    return nc


def _make_runner(nc):
    """Build (once) a cached jitted shard_map wrapper around the compiled
    Bass module — same lowering as bass2jax.run_bass_via_pjrt, but the jit
    object is reused across calls so warm calls skip retrace/recompile."""
    import jax
    import jax.core as jcore
    from jax.experimental.shard_map import shard_map
    from jax.sharding import Mesh, PartitionSpec
    from concourse import bass2jax, mybir
    bass2jax.install_neuronx_cc_hook()

    partition_name = nc.partition_id_tensor.name if nc.partition_id_tensor else None
    in_names, out_names, out_avals, zero_shapes = [], [], [], []
    for alloc in nc.m.functions[0].allocations:
        if not isinstance(alloc, mybir.MemoryLocationSet):
            continue
        name = alloc.memorylocations[0].name
        if alloc.kind == "ExternalInput":
            if name != partition_name:
                in_names.append(name)
        elif alloc.kind == "ExternalOutput":
            shape = tuple(alloc.tensor_shape)
            dtype = mybir.dt.np(alloc.dtype)
            out_names.append(name)
            out_avals.append(jcore.ShapedArray(shape, dtype))
            zero_shapes.append((shape, dtype))
    n_params = len(in_names)
    n_outs = len(out_avals)
    all_in = list(in_names) + list(out_names)
    if partition_name is not None:
        all_in.append(partition_name)
    donate = tuple(range(n_params, n_params + n_outs))

    dbg_name = None
    if nc.dbg_addr is not None:
        assert not nc.dbg_callbacks
        dbg_name = nc.dbg_addr.name

    def _body(*args):
        operands = list(args)
        if partition_name is not None:
            operands.append(bass2jax.partition_id_tensor())
        outs = bass2jax._bass_exec_p.bind(
            *operands, out_avals=tuple(out_avals), in_names=tuple(all_in),
            out_names=tuple(out_names), lowering_input_output_aliases=(),
            sim_require_finite=True, sim_require_nnan=True, nc=nc)
        return tuple(outs)

    devices = jax.devices()[:NCORES]
    mesh = Mesh(np.asarray(devices), ("core",))
    in_specs = (PartitionSpec("core"),) * (n_params + n_outs)
    out_specs = (PartitionSpec("core"),) * n_outs
    fn = jax.jit(shard_map(_body, mesh=mesh, in_specs=in_specs,
                           out_specs=out_specs, check_rep=False),
                 donate_argnums=donate, keep_unused=True)
    return dict(fn=fn, in_names=in_names, out_names=out_names,
                out_avals=out_avals, zero_shapes=zero_shapes, dbg_name=dbg_name)


def _execute(runner, in_maps):
    n = len(in_maps)
    if runner['dbg_name'] is not None:
        z = np.zeros((1, 2), np.uint32)
        in_maps = [{**m, runner['dbg_name']: z} for m in in_maps]
    concat_in = [np.concatenate([np.asarray(in_maps[c][nm]) for c in range(n)], axis=0)
                 for nm in runner['in_names']]
    zeros = [np.zeros((n * s[0], *s[1:]), dt) for (s, dt) in runner['zero_shapes']]
    outs = runner['fn'](*concat_in, *zeros)
    return [{nm: np.asarray(outs[i]).reshape(n, *runner['out_avals'][i].shape)[c]
             for i, nm in enumerate(runner['out_names'])} for c in range(n)]


def _in_maps(x, pp, blob):
    b96, b128 = blob
    xb = np.ascontiguousarray(
        np.clip(np.rint(x * XSCALE), -127, 127).astype(np.int8))
    maps = []
    for c in range(NCORES):
        maps.append({'x': xb[c * NLOC:(c + 1) * NLOC],
                     'ea8': pp['ea8'][c],
                     'dcw': pp['dcw'][c],
                     'gw': pp['gw'][c],
                     'wb96': b96,
                     'wb128': b128})
    return maps


def kernel(**inputs):
    x = np.asarray(inputs['x'], np.float32)
    ei = np.asarray(inputs['edge_index'])
    ea = np.asarray(inputs['edge_attr'], np.float32)
    pp = _host_prep(x, ei, ea)
    if _CACHE.get('NSLOT') != pp['NSLOT']:
        _CACHE['nc'] = build_kernel(pp)
        _CACHE['runner'] = _make_runner(_CACHE['nc'])
        _CACHE['NSLOT'] = pp['NSLOT']
    blob = _wblob(inputs)
    res = _execute(_CACHE['runner'], _in_maps(x, pp, blob))
    out = np.concatenate([res[c]['out'] for c in range(NCORES)], axis=0)
    return out.astype(np.float32)


# revision 49
# speedup vs baseline: 21.3834x; 1.1292x over previous
"""BiLevelGAT (2-branch x 3-layer GATv2, N=50000, E=500000, D=96) on 8 TRN2 cores.

Sharding: nodes + incoming edges partitioned by dst; per-layer AllGather of a
bf16 per-node table [hl_loc 96|1|w_loc|w_glob|pad29|hl_glob 96|1|w_glob|pad30]
(512B rows) gathered per edge by src.

Math: lrelu(x) = 0.6x+0.4|x| splits the GATv2 logit into linear terms (per-src
w=exp(0.6*att.hl) folded into the softmax weight; per-dst term cancels in
softmax; per-edge ea term computed on device from a compact [9, NSLOT] edge
table: rows 0-7 ea, row 8 dst-col) plus 0.4*att.|m| computed on device.
Softmax max-subtraction skipped (logits O(1), fp32 safe).

The per-edge one-hot "R" matrix (dst selector + ea rows used as matmul rhs to
form m = hr[dst] + ea@We) is built on device per section: a rank-1 broadcast
matmul of the dst-col row followed by an is_equal against a partition iota.
Inputs per core are ~5MB (x bf16, edge table bf16, gather idx, dst cols,
one packed weight blob) so the host->device transfer over the axon tunnel
stays small; the jitted shard_map wrapper is cached across calls.
"""
import sys
sys.path.insert(0, '/opt/trn_rl_repo')
import numpy as np
import ml_dtypes

BF16 = ml_dtypes.bfloat16

N, E, D, EDIM, L, DENSE, OUT = 50000, 500000, 96, 8, 3, 256, 2
NCORES = 8
NLOC = N // NCORES            # 6250
WIN, HALF = 96, 48
NWIN = (NLOC + WIN - 1) // WIN  # 66
NPAD = NWIN * WIN
NCH = (NPAD + 127) // 128     # chunks of 128 (PASS A / table)
SPLIT = 32768
TROW = 256
KMAX = 6

_CACHE = {}

# ---- packed weight blobs ----
# wbh [96, WBH] bf16: big matrices (Wl/Wr/att/We/fusion/pred_W1, V).
#   We_{l,b} stacked six-up in two 96-col groups at row bases 0/32/64
#   (any access must start at a partition base that is a multiple of 32);
#   V_l at row base 32*l (matmul rhs).
# wbf [96, WBF] f32: 96-row biases.
# wb128 [128, WB128] f32: 128-row biases/mats + iota constants.
WOFF = {}   # name -> (blob, row0, col0)
_cols = {'h': 0, 'f': 0, '128': 0}
def _al(blob, name, width, row=0):
    WOFF[name] = (blob, row, _cols[blob])
    _cols[blob] += width
for _l in range(L):
    for _b in range(2):
        _al('h', f'Wl_{_l}_{_b}', 96)
        _al('h', f'Wr_{_l}_{_b}', 96)
        _al('h', f'att_{_l}_{_b}', 1)
        _al('f', f'bb_{_l}_{_b}', 1)
for _i in range(6):
    _l, _b = divmod(_i, 2)
    WOFF[f'We_{_l}_{_b}'] = ('h', 32 * (_i % 3), _cols['h'] + 96 * (_i // 3))
_cols['h'] += 192
for _l in range(L):
    WOFF[f'V_{_l}'] = ('h', 32 * _l, _cols['h'])              # shared 2 cols
_cols['h'] += 2
_al('h', 'fusion_Wt', 96)
_al('h', 'fusion_Wb', 96)
_al('f', 'fusion_b', 1)
_al('h', 'pred_W1a', 128)
_al('h', 'pred_W1b', 128)
_al('128', 'pred_b1a', 1)
_al('128', 'pred_b1b', 1)
_al('128', 'pred_W2a', 2)
_al('128', 'pred_W2b', 2)
_al('128', 'pred_b2', 2)
_al('128', 'iotaf', 48)               # all 128 rows = arange(48)
_al('128', 'iota128', 1)              # rows = arange(128)
WBH, WBF, WB128 = _cols['h'], _cols['f'], _cols['128']
EASCALE = 32.0
XSCALE = 32.0


def _host_prep(x, edge_index, edge_attr):
    src = edge_index[0].astype(np.int64)
    dst = edge_index[1].astype(np.int64)
    mean_ea = edge_attr.mean(0).astype(np.float32)
    loop = np.arange(N, dtype=np.int64)
    src_a = np.concatenate([src, loop])
    dst_a = np.concatenate([dst, loop])
    ea_a = np.concatenate([edge_attr.astype(np.float32),
                           np.broadcast_to(mean_ea, (N, EDIM))], 0)

    owner = dst_a // NLOC
    dloc = dst_a - owner * NLOC
    win = dloc // WIN
    half = (dloc % WIN) // HALF
    stream = (src_a >= SPLIT).astype(np.int64)

    per_core = []
    secs = np.zeros((NCORES, NWIN, 2, 2), np.int64)
    for c in range(NCORES):
        m = owner == c
        s_c, d_c, e_c = src_a[m], dloc[m], ea_a[m]
        w_c, h_c, st_c = win[m], half[m], stream[m]
        sec = ((w_c * 2 + h_c) * 2 + st_c)
        order = np.argsort(sec * NLOC + d_c, kind='stable')
        s_c, d_c, e_c, sec = s_c[order], d_c[order], e_c[order], sec[order]
        st_c = st_c[order]
        per_core.append((s_c, d_c, e_c, sec, st_c))
        secs[c] = np.bincount(sec, minlength=NWIN * 4).reshape(NWIN, 2, 2)

    K = np.maximum((secs.max(0) + 127) // 128, 1)       # [NWIN, 2, 2]
    assert K.max() <= KMAX
    Kf = K.reshape(-1)
    sec_slot = np.zeros(NWIN * 4 + 1, np.int64)
    np.cumsum(Kf * 128, out=sec_slot[1:])
    NSLOT = int(sec_slot[-1])

    gidx = np.zeros((NCORES, NSLOT), np.int16)
    eav = np.zeros((NCORES, 9, NSLOT), np.float32)
    # invalid slots: dstcol=147 misses the 96-wide one-hot AND (after -48h)
    # the 48-wide es window -> no contribution
    eav[:, 8, :] = 147.0

    for c in range(NCORES):
        s_c, d_c, e_c, sec, st_c = per_core[c]
        counts = np.bincount(sec, minlength=NWIN * 4)
        starts = np.concatenate([[0], np.cumsum(counts)])[:-1]
        pos = np.arange(len(s_c)) - starts[sec]
        slot = sec_slot[sec] + pos
        gidx[c, slot] = (s_c - st_c * SPLIT).astype(np.int16)
        eav[c, 0:8, slot] = e_c  # advanced idx puts slot axis first: (nedge, 8)
        eav[c, 8, slot] = (d_c % WIN).astype(np.float32)

    gw = np.ascontiguousarray(
        gidx.reshape(NCORES, -1, 16).transpose(0, 2, 1))     # [NCORES, 16, NSLOT//16]

    ea_q = np.clip(np.rint(eav[:, 0:8, :] * EASCALE), -127, 127).astype(np.int8)
    return dict(K=K, Kf=Kf, sec_slot=sec_slot, NSLOT=NSLOT, NSEC=NWIN * 4,
                gw=gw, ea8=ea_q,
                dcw=eav[:, 8, :].astype(np.uint8).reshape(NCORES, 1, NSLOT))


def _wblob(w):
    bh = np.zeros((96, WBH), BF16)
    bf = np.zeros((96, WBF), np.float32)
    b128 = np.zeros((128, WB128), np.float32)
    blobs = {'h': bh, 'f': bf, '128': b128}
    def put(name, arr):
        a = np.asarray(arr, np.float32)
        if a.ndim == 1:
            a = a.reshape(-1, 1)
        bl, r0, c0 = WOFF[name]
        dst = blobs[bl]
        dst[r0:r0 + a.shape[0], c0:c0 + a.shape[1]] = a.astype(dst.dtype)
    for l in range(L):
        V = np.zeros((8, 2), np.float32)
        # x ships as int8 * XSCALE; fold 1/XSCALE into the layer-0 weights
        xs = XSCALE if l == 0 else 1.0
        for b, p in enumerate(['local', 'global']):
            put(f'Wl_{l}_{b}', np.asarray(w[f'{p}_Wl'][l], np.float32) / xs)
            put(f'Wr_{l}_{b}', np.asarray(w[f'{p}_Wr'][l], np.float32) / xs)
            put(f'att_{l}_{b}', w[f'{p}_att'][l])
            put(f'bb_{l}_{b}', w[f'{p}_b'][l])
            # ea ships as int8 * EASCALE; fold 1/EASCALE into We and V
            put(f'We_{l}_{b}', np.asarray(w[f'{p}_We'][l], np.float32) / EASCALE)
            V[:, b] = (0.6 / EASCALE) * (np.asarray(w[f'{p}_We'][l], np.float32)
                                         @ np.asarray(w[f'{p}_att'][l], np.float32))
        put(f'V_{l}', V)
    put('fusion_Wt', w['fusion_W'][:96])
    put('fusion_Wb', w['fusion_W'][96:])
    put('fusion_b', w['fusion_b'])
    put('pred_W1a', w['pred_W1'][:, :128])
    put('pred_W1b', w['pred_W1'][:, 128:])
    put('pred_b1a', w['pred_b1'][:128])
    put('pred_b1b', w['pred_b1'][128:])
    put('pred_W2a', w['pred_W2'][:128])
    put('pred_W2b', w['pred_W2'][128:])
    put('pred_b2', np.broadcast_to(np.asarray(w['pred_b2']).reshape(1, 2), (128, 2)))
    put('iotaf', np.broadcast_to(np.arange(48, dtype=np.float32), (128, 48)))
    put('iota128', np.arange(128, dtype=np.float32))
    return bh, bf, b128


def build_kernel(pp):
    import os as _os
    SKIP_EDGE = _os.environ.get('SKIP_EDGE', '0') == '1'
    SKIP_GATHER = _os.environ.get('SKIP_GATHER', '0') == '1'
    from concourse import mybir, bacc
    import concourse.tile as tile
    Kf, sec_slot, NSLOT = pp['Kf'], pp['sec_slot'], pp['NSLOT']
    f32, bf16, i16 = mybir.dt.float32, mybir.dt.bfloat16, mybir.dt.int16
    AF = mybir.ActivationFunctionType
    OP = mybir.AluOpType

    i8, u8 = mybir.dt.int8, mybir.dt.uint8
    nc = bacc.Bacc("TRN2", target_bir_lowering=False, debug=False, num_devices=NCORES)
    dx = nc.dram_tensor("x", [NLOC, D], i8, kind="ExternalInput")
    dea8 = nc.dram_tensor("ea8", [8, NSLOT], i8, kind="ExternalInput")
    ddcw = nc.dram_tensor("dcw", [1, NSLOT], u8, kind="ExternalInput")
    dgw = nc.dram_tensor("gw", [16, NSLOT // 16], i16, kind="ExternalInput")
    dwbh = nc.dram_tensor("wbh", [96, WBH], bf16, kind="ExternalInput")
    dwbf = nc.dram_tensor("wbf", [96, WBF], f32, kind="ExternalInput")
    dwb128 = nc.dram_tensor("wb128", [128, WB128], f32, kind="ExternalInput")
    dout = nc.dram_tensor("out", [NLOC, OUT], f32, kind="ExternalOutput")

    tab_slice = nc.dram_tensor("tab_slice", [NLOC, TROW], bf16)
    tab_sh = nc.dram_tensor("tab_sh", [N, TROW], bf16, addr_space="Shared")
    tab = nc.dram_tensor("tab", [N, TROW], bf16)

    def wo(name, rows=96, width=None):
        w_ = width if width is not None else 96
        return (WOFF[name], WOFF[name] + w_, rows)

    with tile.TileContext(nc) as tc:
      with (tc.tile_pool(name="const", bufs=1) as cp,
            tc.tile_pool(name="hp", bufs=1) as hp,
            tc.tile_pool(name="wp", bufs=1) as wp,
            tc.tile_pool(name="sp", bufs=3) as sp,
            tc.tile_pool(name="gpool", bufs=2) as gpl,
            tc.tile_pool(name="ps", bufs=2, space="PSUM") as psp,
            tc.tile_pool(name="psA", bufs=2, space="PSUM") as psA,
            tc.tile_pool(name="psagg", bufs=1, space="PSUM") as psG):

        ident = cp.tile([128, 128], bf16)
        nc.sync.dma_start(out=ident[:], in_=nc.inline_tensor(np.eye(128, dtype=BF16), name="idb").ap())
        identf = cp.tile([128, 128], f32)
        nc.sync.dma_start(out=identf[:], in_=nc.inline_tensor(np.eye(128, dtype=np.float32), name="idf").ap())
        wbh_t = cp.tile([96, WBH], bf16)
        nc.sync.dma_start(out=wbh_t[:], in_=dwbh[:])
        wbf_t = cp.tile([96, WBF], f32)
        nc.sync.dma_start(out=wbf_t[:], in_=dwbf[:])
        wb128_t = cp.tile([128, WB128], f32)
        nc.sync.dma_start(out=wb128_t[:], in_=dwb128[:])
        gw_t = cp.tile([128, NSLOT // 16], i16)
        for k in range(8):
            nc.sync.dma_start(out=gw_t[16 * k:16 * (k + 1), :], in_=dgw[:])

        blobs_t = {'h': wbh_t, 'f': wbf_t, '128': wb128_t}
        def W(name, rows=96, width=96):
            bl, r0, c0 = WOFF[name]
            return blobs_t[bl][r0:r0 + rows, c0:c0 + width]

        iota_t = wb128_t[:, WOFF['iotaf'][2]:WOFF['iotaf'][2] + 48]

        one1 = cp.tile([1, 96], f32)
        nc.vector.memset(one1[:], 1.0)
        ones1 = cp.tile([1, 128], bf16)
        nc.vector.memset(ones1[:], 1.0)
        att04 = {}
        for l in range(L):
            for b in range(2):
                att04[(l, b)] = cp.tile([96, 1], bf16, tag=f"att04_{l}_{b}", name=f"att04_{l}_{b}")
                nc.vector.tensor_scalar(out=att04[(l, b)][:], in0=W(f'att_{l}_{b}', 96, 1),
                                        scalar1=0.4, scalar2=None, op0=OP.mult)
        Vt = {}
        for l in range(L):
            Vt[l] = cp.tile([8, 2], bf16, tag=f"V_{l}", name=f"V_{l}")
            nc.vector.tensor_copy(out=Vt[l][:], in_=W(f'V_{l}', 8, 2))

        # h_T feature-major [96, NPAD] (cols beyond NLOC are pad)
        h_T = [hp.tile([96, NCH * 128], bf16, tag=f"h{b}", name=f"h{b}") for b in range(2)]
        for ch in range(NCH):
            n0 = ch * 128
            nreal = max(0, min(NLOC - n0, 128))
            xin8 = sp.tile([128, 128], i8, tag="xin8")
            nc.vector.memset(xin8[:], 0)
            if nreal > 0:
                nc.sync.dma_start(out=xin8[:nreal, :96], in_=dx[n0:n0 + nreal, :])
            xin = sp.tile([128, 128], bf16, tag="xin")
            nc.vector.tensor_copy(out=xin[:], in_=xin8[:])
            pt = psA.tile([128, 128], bf16, tag="pbig")
            nc.tensor.transpose(out=pt[:], in_=xin[:], identity=ident[:])
            for b in range(2):
                nc.vector.tensor_copy(out=h_T[b][:, n0:n0 + 128], in_=pt[:96, :])

        hw_T = [wp.tile([96, NCH * 128], bf16, tag=f"hw{b}", name=f"hw{b}") for b in range(2)]

        for l in range(L):
            # ---------- PASS A ----------
            for b in range(2):
                for cs in range(0, NCH * 128, 512):
                    ce = min(cs + 512, NCH * 128)
                    w_ = ce - cs
                    pl = psA.tile([96, 512], f32, tag="pbig")
                    nc.tensor.matmul(out=pl[:, :w_], lhsT=W(f'Wl_{l}_{b}'),
                                     rhs=h_T[b][:, cs:ce], start=True, stop=True)
                    nc.vector.tensor_copy(out=hw_T[b][:, cs:ce], in_=pl[:, :w_])
            # table slice + allgather
            for ch in range(NCH):
                n0 = ch * 128
                nreal = max(0, min(NLOC - n0, 128))
                if nreal == 0:
                    continue
                stg = sp.tile([128, TROW], bf16, tag="stg")
                nc.vector.memset(stg[:], 0.0)
                for b in range(2):
                    pt = psA.tile([128, 128], bf16, tag="pbig")
                    nc.tensor.transpose(out=pt[:, :96], in_=hw_T[b][:, n0:n0 + 128],
                                        identity=ident[:96, :96])
                    nc.vector.tensor_copy(out=stg[:, b * 128:b * 128 + 96], in_=pt[:, :96])
                    # w = exp(0.6*att.hl) for this chunk; ones at ext row 32
                    pphi = psA.tile([1, 128], f32, tag="pbig")
                    nc.tensor.matmul(out=pphi[:], lhsT=W(f'att_{l}_{b}', 96, 1),
                                     rhs=hw_T[b][:, n0:n0 + 128], start=True, stop=True)
                    ext = sp.tile([64, 128], f32, tag="ext")
                    nc.scalar.activation(out=ext[0:1, :], in_=pphi[:], func=AF.Exp, scale=0.6)
                    nc.vector.memset(ext[32:33, :], 1.0)
                    pt2 = psA.tile([128, 64], f32, tag="pbig")
                    nc.tensor.transpose(out=pt2[:], in_=ext[:], identity=identf[:64, :64])
                    nc.vector.tensor_copy(out=stg[:, b * 128 + 96:b * 128 + 97], in_=pt2[:, 32:33])
                    nc.vector.tensor_copy(out=stg[:, b * 128 + 97:b * 128 + 98], in_=pt2[:, 0:1])
                nc.vector.tensor_copy(out=stg[:, 98:99], in_=stg[:, 225:226])
                nc.sync.dma_start(out=tab_slice[n0:n0 + nreal, :], in_=stg[:nreal, :])
            nc.gpsimd.collective_compute(
                "AllGather", mybir.AluOpType.bypass,
                replica_groups=[list(range(NCORES))],
                ins=[tab_slice[:]], outs=[tab_sh[:]],
            )
            nc.sync.dma_start(out=tab[:], in_=tab_sh[:])

            # ---------- edge phase ----------
            for w in range(0 if not SKIP_EDGE else NWIN, NWIN):
                aggp = {}
                first = {b: True for b in range(2)}
                nagg = {b: 0 for b in range(2)}
                tot = {b: sum(int(Kf[(w * 2 + h) * 2 + s]) for h in range(2) for s in range(2))
                       for b in range(2)}
                for b in range(2):
                    aggp[b] = psG.tile([97, WIN], f32, tag=f"agg{b}", name=f"agg{b}")
                # base lhsT per branch for this window (hr = h @ Wr computed here)
                basel = {}
                for b in range(2):
                    phr = psA.tile([96, WIN], f32, tag="pbig")
                    nc.tensor.matmul(out=phr[:], lhsT=W(f'Wr_{l}_{b}'),
                                     rhs=h_T[b][:, w * WIN:(w + 1) * WIN],
                                     start=True, stop=True)
                    hrs = sp.tile([96, WIN], f32, tag="hrs")
                    nc.vector.tensor_copy(out=hrs[:], in_=phr[:])
                    pt = psA.tile([WIN, 96], f32, tag="pbig")
                    nc.tensor.transpose(out=pt[:], in_=hrs[:], identity=identf[:96, :96])
                    bl = sp.tile([128, 96], bf16, tag=f"basel{b}", name=f"basel{b}")
                    nc.vector.memset(bl[:], 0.0)
                    nc.vector.tensor_copy(out=bl[:WIN, :], in_=pt[:])
                    nc.vector.tensor_copy(out=bl[WIN:WIN + 8, :], in_=W(f'We_{l}_{b}', 8, 96))
                    basel[b] = bl
                for h in range(2):
                    for s in range(2):
                        si = (w * 2 + h) * 2 + s
                        Ks = int(Kf[si])
                        sl0 = int(sec_slot[si])
                        nsl = Ks * 128
                        g = gpl.tile([128, KMAX, TROW], bf16, tag="gath")
                        if SKIP_GATHER:
                            nc.vector.memset(g[:, :Ks, :], 0.0)
                        else:
                            nc.gpsimd.dma_gather(
                                out_ap=g[:, :Ks, :],
                                in_ap=tab[SPLIT:, :] if s else tab[:SPLIT, :],
                                idxs_ap=gw_t[:, sl0 // 16:(sl0 + nsl) // 16],
                                num_idxs=nsl, num_idxs_reg=nsl, elem_size=TROW)
                        # compact edge table slice (i8/u8 on the wire -> bf16)
                        ea8s = sp.tile([8, KMAX * 128], i8, tag="ea8s")
                        nc.sync.dma_start(out=ea8s[:, :nsl], in_=dea8[:, sl0:sl0 + nsl])
                        eavs = sp.tile([8, KMAX * 128], bf16, tag="eavs")
                        nc.vector.tensor_copy(out=eavs[:, :nsl], in_=ea8s[:, :nsl])
                        dc8s = sp.tile([1, KMAX * 128], u8, tag="dc8s")
                        nc.sync.dma_start(out=dc8s[:, :nsl], in_=ddcw[:, sl0:sl0 + nsl])
                        dcw = sp.tile([1, KMAX * 128], bf16, tag="dcw")
                        nc.vector.tensor_copy(out=dcw[:, :nsl], in_=dc8s[:, :nsl])
                        # build Rt on device: rows 0-95 one-hot(dstcol), 96-103
                        # ea. is_equal covers ALL 128 rows (rows 96-127 can
                        # never match dstcol<=147... rows 96-103 overwritten by
                        # ea below, 104-127 exact zeros) so no partition of Rt
                        # is left uninitialized -- the mps matmul reads all 128
                        # partitions and 0 * NaN-garbage = NaN.
                        Rt = sp.tile([128, KMAX * 128], bf16, tag="Rt")
                        for c0 in range(0, nsl, 512):
                            cw = min(512, nsl - c0)
                            pbc = psA.tile([128, 512], f32, tag="pbig")
                            nc.tensor.matmul(out=pbc[:, :cw], lhsT=ones1[:],
                                             rhs=dcw[0:1, c0:c0 + cw], start=True, stop=True)
                            nc.vector.tensor_scalar(out=Rt[:, c0:c0 + cw],
                                                    in0=pbc[:, :cw],
                                                    scalar1=W('iota128', 128, 1),
                                                    scalar2=None, op0=OP.is_equal)
                            nc.vector.tensor_copy(out=Rt[96:104, c0:c0 + cw],
                                                  in_=eavs[0:8, c0:c0 + cw])
                        # per-slot dst col within half-window: transpose dcw
                        # blocks to partitions, -48h (invalid 147 -> 147/99)
                        dcsec = sp.tile([128, KMAX], f32, tag="dcs")
                        for j in range(Ks):
                            pt1 = psA.tile([128, 1], bf16, tag="pbig")
                            nc.tensor.transpose(out=pt1[:],
                                                in_=dcw[0:1, j * 128:(j + 1) * 128],
                                                identity=ident[:1, :1])
                            nc.vector.tensor_scalar(out=dcsec[:, j:j + 1], in0=pt1[:],
                                                    scalar1=float(-48 * h), scalar2=None,
                                                    op0=OP.add)
                        lgp = psp.tile([128, 16], f32, tag="lgp", bufs=1)
                        for j0 in range(0, Ks, 4):
                            jw = min(4, Ks - j0)
                            for b in range(2):
                                mps = psp.tile([96, 512], f32, tag="mps")
                                nc.tensor.matmul(out=mps[:, :jw * 128], lhsT=basel[b][:],
                                                 rhs=Rt[:, j0 * 128:(j0 + jw) * 128],
                                                 start=True, stop=False)
                                for dj in range(jw):
                                    j = j0 + dj
                                    nc.tensor.matmul(out=mps[:, dj * 128:(dj + 1) * 128],
                                                     lhsT=g[:, j, b * 128:b * 128 + 96],
                                                     rhs=ident[:], start=False,
                                                     stop=(dj == jw - 1),
                                                     skip_group_check=True)
                                am = sp.tile([96, 512], bf16, tag="am")
                                nc.scalar.activation(out=am[:, :jw * 128],
                                                     in_=mps[:, :jw * 128], func=AF.Abs)
                                for dj in range(jw):
                                    j = j0 + dj
                                    nc.tensor.matmul(out=lgp[:, 2 * j + b:2 * j + b + 1],
                                                     lhsT=am[:, dj * 128:(dj + 1) * 128],
                                                     rhs=att04[(l, b)][:],
                                                     start=(j == 0 and b == 0), stop=False,
                                                     skip_group_check=True)
                        # += 0.6*ea.(We@att) per branch (cols 2j|2j+1), on device
                        for j in range(Ks):
                            nc.tensor.matmul(out=lgp[:, 2 * j:2 * j + 2],
                                             lhsT=eavs[0:8, j * 128:(j + 1) * 128],
                                             rhs=Vt[l][:], start=False,
                                             stop=(j == Ks - 1), skip_group_check=True)
                        exw = sp.tile([128, 16], f32, tag="exw")
                        nc.scalar.activation(out=exw[:, :2 * Ks], in_=lgp[:, :2 * Ks],
                                             func=AF.Exp)
                        nc.vector.tensor_tensor(
                            out=exw[:, :2 * Ks].rearrange("p (j b) -> p j b", b=2),
                            in0=exw[:, :2 * Ks].rearrange("p (j b) -> p j b", b=2),
                            in1=g[:, :Ks, 97:99], op=OP.mult)
                        for j in range(Ks):
                            for b in range(2):
                                es = sp.tile([128, HALF], bf16, tag="es")
                                nc.vector.tensor_scalar(
                                    out=es[:], in0=iota_t, scalar1=dcsec[:, j:j + 1],
                                    scalar2=exw[:, 2 * j + b:2 * j + b + 1],
                                    op0=OP.is_equal, op1=OP.mult)
                                nagg[b] += 1
                                nc.tensor.matmul(out=aggp[b][:, h * HALF:(h + 1) * HALF],
                                                 lhsT=g[:, j, b * 128:b * 128 + 97],
                                                 rhs=es[:],
                                                 start=first[b], stop=(nagg[b] == tot[b]),
                                                 skip_group_check=True)
                                first[b] = False
                # finalize window -> h_T
                for b in range(2):
                    num = sp.tile([96, WIN], f32, tag="num")
                    den = sp.tile([1, WIN], f32, tag="den")
                    nc.vector.tensor_copy(out=num[:], in_=aggp[b][:96, :])
                    nc.vector.tensor_scalar(out=den[:], in0=aggp[b][96:97, :],
                                            scalar1=1e-30, scalar2=None, op0=OP.add)
                    rec = sp.tile([1, WIN], f32, tag="rec")
                    nc.vector.reciprocal(out=rec[:], in_=den[:])
                    pb = psp.tile([96, WIN], f32, tag="mps")
                    nc.tensor.matmul(out=pb[:], lhsT=one1[:], rhs=rec[:], start=True, stop=True)
                    tdiv = sp.tile([96, WIN], f32, tag="tdiv")
                    nc.vector.tensor_tensor(out=tdiv[:], in0=num[:], in1=pb[:], op=OP.mult)
                    lin = sp.tile([96, WIN], f32, tag="lin")
                    nc.scalar.activation(out=lin[:], in_=tdiv[:], func=AF.Identity,
                                         bias=W(f'bb_{l}_{b}', 96, 1))
                    ab = sp.tile([96, WIN], f32, tag="ab")
                    nc.scalar.activation(out=ab[:], in_=tdiv[:], func=AF.Abs,
                                         bias=W(f'bb_{l}_{b}', 96, 1))
                    nc.vector.tensor_scalar(out=lin[:], in0=lin[:], scalar1=0.505,
                                            scalar2=None, op0=OP.mult)
                    nc.vector.tensor_scalar(out=ab[:], in0=ab[:], scalar1=0.495,
                                            scalar2=None, op0=OP.mult)
                    nc.vector.tensor_tensor(out=h_T[b][:, w * WIN:(w + 1) * WIN],
                                            in0=lin[:], in1=ab[:], op=OP.add)

        # ---------- head ----------
        hid_T = [wp.tile([128, NCH * 128], f32, tag=f"hw{p}", name=f"hid{p}") for p in range(2)]
        for cs in range(0, NCH * 128, 512):
            ce = min(cs + 512, NCH * 128)
            w_ = ce - cs
            pf = psA.tile([96, 512], f32, tag="pbig")
            nc.tensor.matmul(out=pf[:, :w_], lhsT=W('fusion_Wt'),
                             rhs=h_T[0][:, cs:ce], start=True, stop=False)
            nc.tensor.matmul(out=pf[:, :w_], lhsT=W('fusion_Wb'),
                             rhs=h_T[1][:, cs:ce], start=False, stop=True)
            fus = sp.tile([96, 512], bf16, tag="fus")
            lin = sp.tile([96, 512], f32, tag="flin")
            nc.scalar.activation(out=lin[:, :w_], in_=pf[:, :w_], func=AF.Identity,
                                 bias=W('fusion_b', 96, 1))
            ab = sp.tile([96, 512], f32, tag="fab")
            nc.scalar.activation(out=ab[:, :w_], in_=pf[:, :w_], func=AF.Abs,
                                 bias=W('fusion_b', 96, 1))
            nc.vector.tensor_scalar(out=lin[:, :w_], in0=lin[:, :w_], scalar1=0.505,
                                    scalar2=None, op0=OP.mult)
            nc.vector.tensor_scalar(out=ab[:, :w_], in0=ab[:, :w_], scalar1=0.495,
                                    scalar2=None, op0=OP.mult)
            nc.vector.tensor_tensor(out=fus[:, :w_], in0=lin[:, :w_], in1=ab[:, :w_],
                                    op=OP.add)
            for p, (wk, bk) in enumerate([('pred_W1a', 'pred_b1a'), ('pred_W1b', 'pred_b1b')]):
                ph = psA.tile([128, 512], f32, tag="pbig")
                nc.tensor.matmul(out=ph[:, :w_], lhsT=W(wk, 96, 128), rhs=fus[:, :w_],
                                 start=True, stop=True)
                l2 = sp.tile([128, 512], f32, tag=f"l2{p}")
                a2 = sp.tile([128, 512], f32, tag=f"a2{p}")
                nc.scalar.activation(out=l2[:, :w_], in_=ph[:, :w_], func=AF.Identity,
                                     bias=W(bk, 128, 1))
                nc.scalar.activation(out=a2[:, :w_], in_=ph[:, :w_], func=AF.Abs,
                                     bias=W(bk, 128, 1))
                nc.vector.tensor_scalar(out=l2[:, :w_], in0=l2[:, :w_], scalar1=0.505,
                                        scalar2=None, op0=OP.mult)
                nc.vector.tensor_scalar(out=a2[:, :w_], in0=a2[:, :w_], scalar1=0.495,
                                        scalar2=None, op0=OP.mult)
                nc.vector.tensor_tensor(out=hid_T[p][:, cs:ce], in0=l2[:, :w_],
                                        in1=a2[:, :w_], op=OP.add)
        for ch in range(NCH):
            n0 = ch * 128
            nreal = max(0, min(NLOC - n0, 128))
            if nreal == 0:
                continue
            po = psp.tile([128, 2], f32, tag="mps")
            nc.tensor.matmul(out=po[:], lhsT=hid_T[0][:, n0:n0 + 128],
                             rhs=W('pred_W2a', 128, 2), start=True, stop=False)
            nc.tensor.matmul(out=po[:], lhsT=hid_T[1][:, n0:n0 + 128],
                             rhs=W('pred_W2b', 128, 2), start=False, stop=True)
            ot = sp.tile([128, 2], f32, tag="ot")
            nc.vector.tensor_tensor(out=ot[:], in0=po[:], in1=W('pred_b2', 128, 2), op=OP.add)
            nc.sync.dma_start(out=dout[n0:n0 + nreal, :], in_=ot[:nreal, :])

    nc.compile()
    return nc


def _make_runner(nc):
    """Build (once) a cached jitted shard_map wrapper around the compiled
    Bass module — same lowering as bass2jax.run_bass_via_pjrt, but the jit
    object is reused across calls so warm calls skip retrace/recompile."""
    import jax
    import jax.core as jcore
    from jax.experimental.shard_map import shard_map
    from jax.sharding import Mesh, PartitionSpec
    from concourse import bass2jax, mybir
    bass2jax.install_neuronx_cc_hook()

    partition_name = nc.partition_id_tensor.name if nc.partition_id_tensor else None
    in_names, out_names, out_avals, zero_shapes = [], [], [], []
    for alloc in nc.m.functions[0].allocations:
        if not isinstance(alloc, mybir.MemoryLocationSet):
            continue
        name = alloc.memorylocations[0].name
        if alloc.kind == "ExternalInput":
            if name != partition_name:
                in_names.append(name)
        elif alloc.kind == "ExternalOutput":
            shape = tuple(alloc.tensor_shape)
            dtype = mybir.dt.np(alloc.dtype)
            out_names.append(name)
            out_avals.append(jcore.ShapedArray(shape, dtype))
            zero_shapes.append((shape, dtype))
    n_params = len(in_names)
    n_outs = len(out_avals)
    all_in = list(in_names) + list(out_names)
    if partition_name is not None:
        all_in.append(partition_name)
    donate = tuple(range(n_params, n_params + n_outs))

    dbg_name = None
    if nc.dbg_addr is not None:
        assert not nc.dbg_callbacks
        dbg_name = nc.dbg_addr.name

    def _body(*args):
        operands = list(args)
        if partition_name is not None:
            operands.append(bass2jax.partition_id_tensor())
        outs = bass2jax._bass_exec_p.bind(
            *operands, out_avals=tuple(out_avals), in_names=tuple(all_in),
            out_names=tuple(out_names), lowering_input_output_aliases=(),
            sim_require_finite=True, sim_require_nnan=True, nc=nc)
        return tuple(outs)

    devices = jax.devices()[:NCORES]
    mesh = Mesh(np.asarray(devices), ("core",))
    in_specs = (PartitionSpec("core"),) * (n_params + n_outs)
    out_specs = (PartitionSpec("core"),) * n_outs
    fn = jax.jit(shard_map(_body, mesh=mesh, in_specs=in_specs,
                           out_specs=out_specs, check_rep=False),
                 donate_argnums=donate, keep_unused=True)
    return dict(fn=fn, in_names=in_names, out_names=out_names,
                out_avals=out_avals, zero_shapes=zero_shapes, dbg_name=dbg_name)


def _execute(runner, in_maps):
    n = len(in_maps)
    if runner['dbg_name'] is not None:
        z = np.zeros((1, 2), np.uint32)
        in_maps = [{**m, runner['dbg_name']: z} for m in in_maps]
    concat_in = [np.concatenate([np.asarray(in_maps[c][nm]) for c in range(n)], axis=0)
                 for nm in runner['in_names']]
    zeros = [np.zeros((n * s[0], *s[1:]), dt) for (s, dt) in runner['zero_shapes']]
    outs = runner['fn'](*concat_in, *zeros)
    return [{nm: np.asarray(outs[i]).reshape(n, *runner['out_avals'][i].shape)[c]
             for i, nm in enumerate(runner['out_names'])} for c in range(n)]


def _in_maps(x, pp, blob):
    bh, bf, b128 = blob
    xb = np.ascontiguousarray(
        np.clip(np.rint(x * XSCALE), -127, 127).astype(np.int8))
    maps = []
    for c in range(NCORES):
        maps.append({'x': xb[c * NLOC:(c + 1) * NLOC],
                     'ea8': pp['ea8'][c],
                     'dcw': pp['dcw'][c],
                     'gw': pp['gw'][c],
                     'wbh': bh,
                     'wbf': bf,
                     'wb128': b128})
    return maps


def kernel(**inputs):
    x = np.asarray(inputs['x'], np.float32)
    ei = np.asarray(inputs['edge_index'])
    ea = np.asarray(inputs['edge_attr'], np.float32)
    pp = _host_prep(x, ei, ea)
    if _CACHE.get('NSLOT') != pp['NSLOT']:
        _CACHE['nc'] = build_kernel(pp)
        _CACHE['runner'] = _make_runner(_CACHE['nc'])
        _CACHE['NSLOT'] = pp['NSLOT']
    blob = _wblob(inputs)
    res = _execute(_CACHE['runner'], _in_maps(x, pp, blob))
    out = np.concatenate([res[c]['out'] for c in range(NCORES)], axis=0)
    return out.astype(np.float32)


# revision 51
# speedup vs baseline: 22.0113x; 1.0294x over previous
"""BiLevelGAT (2-branch x 3-layer GATv2, N=50000, E=500000, D=96) on 8 TRN2 cores.

Sharding: nodes + incoming edges partitioned by dst; per-layer AllGather of a
bf16 per-node table [hl_loc 96|1|w_loc|w_glob|pad29|hl_glob 96|1|w_glob|pad30]
(512B rows) gathered per edge by src.

Math: lrelu(x) = 0.6x+0.4|x| splits the GATv2 logit into linear terms (per-src
w=exp(0.6*att.hl) folded into the softmax weight; per-dst term cancels in
softmax; per-edge ea term = on-device matmul of the edge table against
V=0.6*We@att) plus 0.4*att.|m| computed on device. Softmax max-subtraction
skipped (logits O(1), fp32 safe).

End-to-end wall time is dominated by host->device transfer over the axon
tunnel (~45MB/s), so inputs are aggressively compacted (~2MB/core):
 - x as int8*32 (1/32 folded into the layer-0 Wl/Wr), edge_attr as int8*32
   (1/32 folded into We and V); both add ~5e-3 rel err vs the 2e-2 gate.
 - per-edge one-hot "R" (dst selector + ea rows used as matmul rhs to form
   m = hr[dst] + ea@We) built on device per section: rank-1 broadcast matmul
   of the dst-col row, then is_equal against a partition iota covering ALL
   128 partitions (a partially-initialized Rt causes 0*NaN-garbage = NaN on
   a per-process-random basis -- caught by CoreSim's uninit checker).
 - per-slot dst cols for the scatter one-hot derived on device by per-block
   PE transposes of the dst-col row (invalid slots carry 147 -> miss both
   one-hots).
 - weights packed into 3 blobs (bf16 matrices / f32 96-row biases / f32
   128-row tail + iotas); gather indices shipped once [16, NSLOT/16] and
   replicated to 128 partitions on device.
The jitted shard_map wrapper is cached across calls (fresh jit per call
would retrace + recompile the XLA wrapper each time).
"""
import os
import sys
sys.path.insert(0, '/opt/trn_rl_repo')
os.environ.setdefault('NEURON_RT_RESET_CORES', '1')
import numpy as np
import ml_dtypes

BF16 = ml_dtypes.bfloat16

N, E, D, EDIM, L, DENSE, OUT = 50000, 500000, 96, 8, 3, 256, 2
NCORES = 8
NLOC = N // NCORES            # 6250
WIN, HALF = 96, 48
NWIN = (NLOC + WIN - 1) // WIN  # 66
NPAD = NWIN * WIN
NCH = (NPAD + 127) // 128     # chunks of 128 (PASS A / table)
SPLIT = 32768
TROW = 256
KMAX = 6

_CACHE = {}

# ---- packed weight blobs ----
# wbh [96, WBH] bf16: big matrices (Wl/Wr/att/We/fusion/pred_W1, V).
#   We_{l,b} stacked six-up in two 96-col groups at row bases 0/32/64
#   (any access must start at a partition base that is a multiple of 32);
#   V_l at row base 32*l (matmul rhs).
# wbf [96, WBF] f32: 96-row biases.
# wb128 [128, WB128] f32: 128-row biases/mats + iota constants.
WOFF = {}   # name -> (blob, row0, col0)
_cols = {'h': 0, 'f': 0, '128': 0}
def _al(blob, name, width, row=0):
    WOFF[name] = (blob, row, _cols[blob])
    _cols[blob] += width
for _l in range(L):
    for _b in range(2):
        _al('h', f'Wl_{_l}_{_b}', 96)
        _al('h', f'Wr_{_l}_{_b}', 96)
        _al('h', f'att_{_l}_{_b}', 1)
        _al('f', f'bb_{_l}_{_b}', 1)
for _i in range(6):
    _l, _b = divmod(_i, 2)
    WOFF[f'We_{_l}_{_b}'] = ('h', 32 * (_i % 3), _cols['h'] + 96 * (_i // 3))
_cols['h'] += 192
for _l in range(L):
    WOFF[f'V_{_l}'] = ('h', 32 * _l, _cols['h'])              # shared 2 cols
_cols['h'] += 2
_al('h', 'fusion_Wt', 96)
_al('h', 'fusion_Wb', 96)
_al('f', 'fusion_b', 1)
_al('h', 'pred_W1a', 128)
_al('h', 'pred_W1b', 128)
_al('128', 'pred_b1a', 1)
_al('128', 'pred_b1b', 1)
_al('128', 'pred_W2a', 2)
_al('128', 'pred_W2b', 2)
_al('128', 'pred_b2', 2)
_al('128', 'iotaf', 48)               # all 128 rows = arange(48)
_al('128', 'iota128', 1)              # rows = arange(128)
WBH, WBF, WB128 = _cols['h'], _cols['f'], _cols['128']
EASCALE = 32.0
XSCALE = 32.0


def _host_prep(x, edge_index, edge_attr):
    src = edge_index[0].astype(np.int64)
    dst = edge_index[1].astype(np.int64)
    mean_ea = edge_attr.mean(0).astype(np.float32)
    loop = np.arange(N, dtype=np.int64)
    src_a = np.concatenate([src, loop])
    dst_a = np.concatenate([dst, loop])
    ea_a = np.concatenate([edge_attr.astype(np.float32),
                           np.broadcast_to(mean_ea, (N, EDIM))], 0)

    owner = dst_a // NLOC
    dloc = dst_a - owner * NLOC
    win = dloc // WIN
    half = (dloc % WIN) // HALF
    stream = (src_a >= SPLIT).astype(np.int64)

    per_core = []
    secs = np.zeros((NCORES, NWIN, 2, 2), np.int64)
    for c in range(NCORES):
        m = owner == c
        s_c, d_c, e_c = src_a[m], dloc[m], ea_a[m]
        w_c, h_c, st_c = win[m], half[m], stream[m]
        sec = ((w_c * 2 + h_c) * 2 + st_c)
        order = np.argsort(sec * NLOC + d_c, kind='stable')
        s_c, d_c, e_c, sec = s_c[order], d_c[order], e_c[order], sec[order]
        st_c = st_c[order]
        per_core.append((s_c, d_c, e_c, sec, st_c))
        secs[c] = np.bincount(sec, minlength=NWIN * 4).reshape(NWIN, 2, 2)

    K = np.maximum((secs.max(0) + 127) // 128, 1)       # [NWIN, 2, 2]
    assert K.max() <= KMAX
    Kf = K.reshape(-1)
    sec_slot = np.zeros(NWIN * 4 + 1, np.int64)
    np.cumsum(Kf * 128, out=sec_slot[1:])
    NSLOT = int(sec_slot[-1])

    gidx = np.zeros((NCORES, NSLOT), np.int16)
    eav = np.zeros((NCORES, 9, NSLOT), np.float32)
    # invalid slots: dstcol=147 misses the 96-wide one-hot AND (after -48h)
    # the 48-wide es window -> no contribution
    eav[:, 8, :] = 147.0

    for c in range(NCORES):
        s_c, d_c, e_c, sec, st_c = per_core[c]
        counts = np.bincount(sec, minlength=NWIN * 4)
        starts = np.concatenate([[0], np.cumsum(counts)])[:-1]
        pos = np.arange(len(s_c)) - starts[sec]
        slot = sec_slot[sec] + pos
        gidx[c, slot] = (s_c - st_c * SPLIT).astype(np.int16)
        eav[c, 0:8, slot] = e_c  # advanced idx puts slot axis first: (nedge, 8)
        eav[c, 8, slot] = (d_c % WIN).astype(np.float32)

    gw = np.ascontiguousarray(
        gidx.reshape(NCORES, -1, 16).transpose(0, 2, 1))     # [NCORES, 16, NSLOT//16]

    ea_q = np.clip(np.rint(eav[:, 0:8, :] * EASCALE), -127, 127).astype(np.int8)
    return dict(K=K, Kf=Kf, sec_slot=sec_slot, NSLOT=NSLOT, NSEC=NWIN * 4,
                gw=gw, ea8=ea_q,
                dcw=eav[:, 8, :].astype(np.uint8).reshape(NCORES, 1, NSLOT))


def _wblob(w):
    bh = np.zeros((96, WBH), BF16)
    bf = np.zeros((96, WBF), np.float32)
    b128 = np.zeros((128, WB128), np.float32)
    blobs = {'h': bh, 'f': bf, '128': b128}
    def put(name, arr):
        a = np.asarray(arr, np.float32)
        if a.ndim == 1:
            a = a.reshape(-1, 1)
        bl, r0, c0 = WOFF[name]
        dst = blobs[bl]
        dst[r0:r0 + a.shape[0], c0:c0 + a.shape[1]] = a.astype(dst.dtype)
    for l in range(L):
        V = np.zeros((8, 2), np.float32)
        # x ships as int8 * XSCALE; fold 1/XSCALE into the layer-0 weights
        xs = XSCALE if l == 0 else 1.0
        for b, p in enumerate(['local', 'global']):
            put(f'Wl_{l}_{b}', np.asarray(w[f'{p}_Wl'][l], np.float32) / xs)
            put(f'Wr_{l}_{b}', np.asarray(w[f'{p}_Wr'][l], np.float32) / xs)
            put(f'att_{l}_{b}', w[f'{p}_att'][l])
            put(f'bb_{l}_{b}', w[f'{p}_b'][l])
            # ea ships as int8 * EASCALE; fold 1/EASCALE into We and V
            put(f'We_{l}_{b}', np.asarray(w[f'{p}_We'][l], np.float32) / EASCALE)
            V[:, b] = (0.6 / EASCALE) * (np.asarray(w[f'{p}_We'][l], np.float32)
                                         @ np.asarray(w[f'{p}_att'][l], np.float32))
        put(f'V_{l}', V)
    put('fusion_Wt', w['fusion_W'][:96])
    put('fusion_Wb', w['fusion_W'][96:])
    put('fusion_b', w['fusion_b'])
    put('pred_W1a', w['pred_W1'][:, :128])
    put('pred_W1b', w['pred_W1'][:, 128:])
    put('pred_b1a', w['pred_b1'][:128])
    put('pred_b1b', w['pred_b1'][128:])
    put('pred_W2a', w['pred_W2'][:128])
    put('pred_W2b', w['pred_W2'][128:])
    put('pred_b2', np.broadcast_to(np.asarray(w['pred_b2']).reshape(1, 2), (128, 2)))
    put('iotaf', np.broadcast_to(np.arange(48, dtype=np.float32), (128, 48)))
    put('iota128', np.arange(128, dtype=np.float32))
    return bh, bf, b128


def build_kernel(pp):
    import os as _os
    SKIP_EDGE = _os.environ.get('SKIP_EDGE', '0') == '1'
    SKIP_GATHER = _os.environ.get('SKIP_GATHER', '0') == '1'
    from concourse import mybir, bacc
    import concourse.tile as tile
    Kf, sec_slot, NSLOT = pp['Kf'], pp['sec_slot'], pp['NSLOT']
    f32, bf16, i16 = mybir.dt.float32, mybir.dt.bfloat16, mybir.dt.int16
    AF = mybir.ActivationFunctionType
    OP = mybir.AluOpType

    i8, u8 = mybir.dt.int8, mybir.dt.uint8
    nc = bacc.Bacc("TRN2", target_bir_lowering=False, debug=False, num_devices=NCORES)
    dx = nc.dram_tensor("x", [NLOC, D], i8, kind="ExternalInput")
    dea8 = nc.dram_tensor("ea8", [8, NSLOT], i8, kind="ExternalInput")
    ddcw = nc.dram_tensor("dcw", [1, NSLOT], u8, kind="ExternalInput")
    dgw = nc.dram_tensor("gw", [16, NSLOT // 16], i16, kind="ExternalInput")
    dwbh = nc.dram_tensor("wbh", [96, WBH], bf16, kind="ExternalInput")
    dwbf = nc.dram_tensor("wbf", [96, WBF], f32, kind="ExternalInput")
    dwb128 = nc.dram_tensor("wb128", [128, WB128], f32, kind="ExternalInput")
    dout = nc.dram_tensor("out", [NLOC, OUT], f32, kind="ExternalOutput")

    tab_slice = nc.dram_tensor("tab_slice", [NLOC, TROW], bf16)
    tab_sh = nc.dram_tensor("tab_sh", [N, TROW], bf16, addr_space="Shared")
    tab = nc.dram_tensor("tab", [N, TROW], bf16)

    def wo(name, rows=96, width=None):
        w_ = width if width is not None else 96
        return (WOFF[name], WOFF[name] + w_, rows)

    with tile.TileContext(nc) as tc:
      with (tc.tile_pool(name="const", bufs=1) as cp,
            tc.tile_pool(name="hp", bufs=1) as hp,
            tc.tile_pool(name="wp", bufs=1) as wp,
            tc.tile_pool(name="sp", bufs=3) as sp,
            tc.tile_pool(name="gpool", bufs=2) as gpl,
            tc.tile_pool(name="ps", bufs=2, space="PSUM") as psp,
            tc.tile_pool(name="psA", bufs=2, space="PSUM") as psA,
            tc.tile_pool(name="psagg", bufs=1, space="PSUM") as psG):

        ident = cp.tile([128, 128], bf16)
        nc.sync.dma_start(out=ident[:], in_=nc.inline_tensor(np.eye(128, dtype=BF16), name="idb").ap())
        identf = cp.tile([128, 128], f32)
        nc.sync.dma_start(out=identf[:], in_=nc.inline_tensor(np.eye(128, dtype=np.float32), name="idf").ap())
        wbh_t = cp.tile([96, WBH], bf16)
        nc.sync.dma_start(out=wbh_t[:], in_=dwbh[:])
        wbf_t = cp.tile([96, WBF], f32)
        nc.sync.dma_start(out=wbf_t[:], in_=dwbf[:])
        wb128_t = cp.tile([128, WB128], f32)
        nc.sync.dma_start(out=wb128_t[:], in_=dwb128[:])
        gw_t = cp.tile([128, NSLOT // 16], i16)
        for k in range(8):
            nc.sync.dma_start(out=gw_t[16 * k:16 * (k + 1), :], in_=dgw[:])

        blobs_t = {'h': wbh_t, 'f': wbf_t, '128': wb128_t}
        def W(name, rows=96, width=96):
            bl, r0, c0 = WOFF[name]
            return blobs_t[bl][r0:r0 + rows, c0:c0 + width]

        iota_t = wb128_t[:, WOFF['iotaf'][2]:WOFF['iotaf'][2] + 48]

        one1 = cp.tile([1, 96], f32)
        nc.vector.memset(one1[:], 1.0)
        ones1 = cp.tile([1, 128], bf16)
        nc.vector.memset(ones1[:], 1.0)
        att04 = {}
        for l in range(L):
            for b in range(2):
                att04[(l, b)] = cp.tile([96, 1], bf16, tag=f"att04_{l}_{b}", name=f"att04_{l}_{b}")
                nc.vector.tensor_scalar(out=att04[(l, b)][:], in0=W(f'att_{l}_{b}', 96, 1),
                                        scalar1=0.4, scalar2=None, op0=OP.mult)
        Vt = {}
        for l in range(L):
            Vt[l] = cp.tile([8, 2], bf16, tag=f"V_{l}", name=f"V_{l}")
            nc.vector.tensor_copy(out=Vt[l][:], in_=W(f'V_{l}', 8, 2))

        # h_T feature-major [96, NPAD] (cols beyond NLOC are pad)
        h_T = [hp.tile([96, NCH * 128], bf16, tag=f"h{b}", name=f"h{b}") for b in range(2)]
        for ch in range(NCH):
            n0 = ch * 128
            nreal = max(0, min(NLOC - n0, 128))
            xin8 = sp.tile([128, 128], i8, tag="xin8")
            nc.vector.memset(xin8[:], 0)
            if nreal > 0:
                nc.sync.dma_start(out=xin8[:nreal, :96], in_=dx[n0:n0 + nreal, :])
            xin = sp.tile([128, 128], bf16, tag="xin")
            nc.vector.tensor_copy(out=xin[:], in_=xin8[:])
            pt = psA.tile([128, 128], bf16, tag="pbig")
            nc.tensor.transpose(out=pt[:], in_=xin[:], identity=ident[:])
            for b in range(2):
                nc.vector.tensor_copy(out=h_T[b][:, n0:n0 + 128], in_=pt[:96, :])

        hw_T = [wp.tile([96, NCH * 128], bf16, tag=f"hw{b}", name=f"hw{b}") for b in range(2)]

        for l in range(L):
            # ---------- PASS A ----------
            for b in range(2):
                for cs in range(0, NCH * 128, 512):
                    ce = min(cs + 512, NCH * 128)
                    w_ = ce - cs
                    pl = psA.tile([96, 512], f32, tag="pbig")
                    nc.tensor.matmul(out=pl[:, :w_], lhsT=W(f'Wl_{l}_{b}'),
                                     rhs=h_T[b][:, cs:ce], start=True, stop=True)
                    nc.vector.tensor_copy(out=hw_T[b][:, cs:ce], in_=pl[:, :w_])
            # table slice + allgather
            for ch in range(NCH):
                n0 = ch * 128
                nreal = max(0, min(NLOC - n0, 128))
                if nreal == 0:
                    continue
                stg = sp.tile([128, TROW], bf16, tag="stg")
                nc.vector.memset(stg[:], 0.0)
                for b in range(2):
                    pt = psA.tile([128, 128], bf16, tag="pbig")
                    nc.tensor.transpose(out=pt[:, :96], in_=hw_T[b][:, n0:n0 + 128],
                                        identity=ident[:96, :96])
                    nc.vector.tensor_copy(out=stg[:, b * 128:b * 128 + 96], in_=pt[:, :96])
                    # w = exp(0.6*att.hl) for this chunk; ones at ext row 32
                    pphi = psA.tile([1, 128], f32, tag="pbig")
                    nc.tensor.matmul(out=pphi[:], lhsT=W(f'att_{l}_{b}', 96, 1),
                                     rhs=hw_T[b][:, n0:n0 + 128], start=True, stop=True)
                    ext = sp.tile([64, 128], f32, tag="ext")
                    nc.scalar.activation(out=ext[0:1, :], in_=pphi[:], func=AF.Exp, scale=0.6)
                    nc.vector.memset(ext[32:33, :], 1.0)
                    pt2 = psA.tile([128, 64], f32, tag="pbig")
                    nc.tensor.transpose(out=pt2[:], in_=ext[:], identity=identf[:64, :64])
                    nc.vector.tensor_copy(out=stg[:, b * 128 + 96:b * 128 + 97], in_=pt2[:, 32:33])
                    nc.vector.tensor_copy(out=stg[:, b * 128 + 97:b * 128 + 98], in_=pt2[:, 0:1])
                nc.vector.tensor_copy(out=stg[:, 98:99], in_=stg[:, 225:226])
                nc.sync.dma_start(out=tab_slice[n0:n0 + nreal, :], in_=stg[:nreal, :])
            nc.gpsimd.collective_compute(
                "AllGather", mybir.AluOpType.bypass,
                replica_groups=[list(range(NCORES))],
                ins=[tab_slice[:]], outs=[tab_sh[:]],
            )
            nc.sync.dma_start(out=tab[:], in_=tab_sh[:])

            # ---------- edge phase ----------
            for w in range(0 if not SKIP_EDGE else NWIN, NWIN):
                aggp = {}
                first = {b: True for b in range(2)}
                nagg = {b: 0 for b in range(2)}
                tot = {b: sum(int(Kf[(w * 2 + h) * 2 + s]) for h in range(2) for s in range(2))
                       for b in range(2)}
                for b in range(2):
                    aggp[b] = psG.tile([97, WIN], f32, tag=f"agg{b}", name=f"agg{b}")
                # base lhsT per branch for this window (hr = h @ Wr computed here)
                basel = {}
                for b in range(2):
                    phr = psA.tile([96, WIN], f32, tag="pbig")
                    nc.tensor.matmul(out=phr[:], lhsT=W(f'Wr_{l}_{b}'),
                                     rhs=h_T[b][:, w * WIN:(w + 1) * WIN],
                                     start=True, stop=True)
                    hrs = sp.tile([96, WIN], f32, tag="hrs")
                    nc.vector.tensor_copy(out=hrs[:], in_=phr[:])
                    pt = psA.tile([WIN, 96], f32, tag="pbig")
                    nc.tensor.transpose(out=pt[:], in_=hrs[:], identity=identf[:96, :96])
                    bl = sp.tile([128, 96], bf16, tag=f"basel{b}", name=f"basel{b}")
                    nc.vector.memset(bl[:], 0.0)
                    nc.vector.tensor_copy(out=bl[:WIN, :], in_=pt[:])
                    nc.vector.tensor_copy(out=bl[WIN:WIN + 8, :], in_=W(f'We_{l}_{b}', 8, 96))
                    basel[b] = bl
                for h in range(2):
                    for s in range(2):
                        si = (w * 2 + h) * 2 + s
                        Ks = int(Kf[si])
                        sl0 = int(sec_slot[si])
                        nsl = Ks * 128
                        g = gpl.tile([128, KMAX, TROW], bf16, tag="gath")
                        if SKIP_GATHER:
                            nc.vector.memset(g[:, :Ks, :], 0.0)
                        else:
                            nc.gpsimd.dma_gather(
                                out_ap=g[:, :Ks, :],
                                in_ap=tab[SPLIT:, :] if s else tab[:SPLIT, :],
                                idxs_ap=gw_t[:, sl0 // 16:(sl0 + nsl) // 16],
                                num_idxs=nsl, num_idxs_reg=nsl, elem_size=TROW)
                        # compact edge table slice (i8/u8 on the wire -> bf16)
                        ea8s = sp.tile([8, KMAX * 128], i8, tag="ea8s")
                        nc.sync.dma_start(out=ea8s[:, :nsl], in_=dea8[:, sl0:sl0 + nsl])
                        eavs = sp.tile([8, KMAX * 128], bf16, tag="eavs")
                        nc.vector.tensor_copy(out=eavs[:, :nsl], in_=ea8s[:, :nsl])
                        dc8s = sp.tile([1, KMAX * 128], u8, tag="dc8s")
                        nc.sync.dma_start(out=dc8s[:, :nsl], in_=ddcw[:, sl0:sl0 + nsl])
                        dcw = sp.tile([1, KMAX * 128], bf16, tag="dcw")
                        nc.vector.tensor_copy(out=dcw[:, :nsl], in_=dc8s[:, :nsl])
                        # build Rt on device: rows 0-95 one-hot(dstcol), 96-103
                        # ea. is_equal covers ALL 128 rows (rows 96-127 can
                        # never match dstcol<=147... rows 96-103 overwritten by
                        # ea below, 104-127 exact zeros) so no partition of Rt
                        # is left uninitialized -- the mps matmul reads all 128
                        # partitions and 0 * NaN-garbage = NaN.
                        Rt = sp.tile([128, KMAX * 128], bf16, tag="Rt")
                        for c0 in range(0, nsl, 512):
                            cw = min(512, nsl - c0)
                            pbc = psA.tile([128, 512], f32, tag="pbig")
                            nc.tensor.matmul(out=pbc[:, :cw], lhsT=ones1[:],
                                             rhs=dcw[0:1, c0:c0 + cw], start=True, stop=True)
                            nc.vector.tensor_scalar(out=Rt[:, c0:c0 + cw],
                                                    in0=pbc[:, :cw],
                                                    scalar1=W('iota128', 128, 1),
                                                    scalar2=None, op0=OP.is_equal)
                            nc.vector.tensor_copy(out=Rt[96:104, c0:c0 + cw],
                                                  in_=eavs[0:8, c0:c0 + cw])
                        # per-slot dst col within half-window: transpose dcw
                        # blocks to partitions, -48h (invalid 147 -> 147/99)
                        dcsec = sp.tile([128, KMAX], f32, tag="dcs")
                        for j in range(Ks):
                            pt1 = psA.tile([128, 1], bf16, tag="pbig")
                            nc.tensor.transpose(out=pt1[:],
                                                in_=dcw[0:1, j * 128:(j + 1) * 128],
                                                identity=ident[:1, :1])
                            nc.vector.tensor_scalar(out=dcsec[:, j:j + 1], in0=pt1[:],
                                                    scalar1=float(-48 * h), scalar2=None,
                                                    op0=OP.add)
                        lgp = psp.tile([128, 16], f32, tag="lgp", bufs=1)
                        for j0 in range(0, Ks, 4):
                            jw = min(4, Ks - j0)
                            for b in range(2):
                                mps = psp.tile([96, 512], f32, tag="mps")
                                nc.tensor.matmul(out=mps[:, :jw * 128], lhsT=basel[b][:],
                                                 rhs=Rt[:, j0 * 128:(j0 + jw) * 128],
                                                 start=True, stop=False)
                                for dj in range(jw):
                                    j = j0 + dj
                                    nc.tensor.matmul(out=mps[:, dj * 128:(dj + 1) * 128],
                                                     lhsT=g[:, j, b * 128:b * 128 + 96],
                                                     rhs=ident[:], start=False,
                                                     stop=(dj == jw - 1),
                                                     skip_group_check=True)
                                am = sp.tile([96, 512], bf16, tag="am")
                                nc.scalar.activation(out=am[:, :jw * 128],
                                                     in_=mps[:, :jw * 128], func=AF.Abs)
                                for dj in range(jw):
                                    j = j0 + dj
                                    nc.tensor.matmul(out=lgp[:, 2 * j + b:2 * j + b + 1],
                                                     lhsT=am[:, dj * 128:(dj + 1) * 128],
                                                     rhs=att04[(l, b)][:],
                                                     start=(j == 0 and b == 0), stop=False,
                                                     skip_group_check=True)
                        # += 0.6*ea.(We@att) per branch (cols 2j|2j+1), on device
                        for j in range(Ks):
                            nc.tensor.matmul(out=lgp[:, 2 * j:2 * j + 2],
                                             lhsT=eavs[0:8, j * 128:(j + 1) * 128],
                                             rhs=Vt[l][:], start=False,
                                             stop=(j == Ks - 1), skip_group_check=True)
                        exw = sp.tile([128, 16], f32, tag="exw")
                        nc.scalar.activation(out=exw[:, :2 * Ks], in_=lgp[:, :2 * Ks],
                                             func=AF.Exp)
                        nc.vector.tensor_tensor(
                            out=exw[:, :2 * Ks].rearrange("p (j b) -> p j b", b=2),
                            in0=exw[:, :2 * Ks].rearrange("p (j b) -> p j b", b=2),
                            in1=g[:, :Ks, 97:99], op=OP.mult)
                        for j in range(Ks):
                            for b in range(2):
                                es = sp.tile([128, HALF], bf16, tag="es")
                                nc.vector.tensor_scalar(
                                    out=es[:], in0=iota_t, scalar1=dcsec[:, j:j + 1],
                                    scalar2=exw[:, 2 * j + b:2 * j + b + 1],
                                    op0=OP.is_equal, op1=OP.mult)
                                nagg[b] += 1
                                nc.tensor.matmul(out=aggp[b][:, h * HALF:(h + 1) * HALF],
                                                 lhsT=g[:, j, b * 128:b * 128 + 97],
                                                 rhs=es[:],
                                                 start=first[b], stop=(nagg[b] == tot[b]),
                                                 skip_group_check=True)
                                first[b] = False
                # finalize window -> h_T
                for b in range(2):
                    num = sp.tile([96, WIN], f32, tag="num")
                    den = sp.tile([1, WIN], f32, tag="den")
                    nc.vector.tensor_copy(out=num[:], in_=aggp[b][:96, :])
                    nc.vector.tensor_scalar(out=den[:], in0=aggp[b][96:97, :],
                                            scalar1=1e-30, scalar2=None, op0=OP.add)
                    rec = sp.tile([1, WIN], f32, tag="rec")
                    nc.vector.reciprocal(out=rec[:], in_=den[:])
                    pb = psp.tile([96, WIN], f32, tag="mps")
                    nc.tensor.matmul(out=pb[:], lhsT=one1[:], rhs=rec[:], start=True, stop=True)
                    tdiv = sp.tile([96, WIN], f32, tag="tdiv")
                    nc.vector.tensor_tensor(out=tdiv[:], in0=num[:], in1=pb[:], op=OP.mult)
                    lin = sp.tile([96, WIN], f32, tag="lin")
                    nc.scalar.activation(out=lin[:], in_=tdiv[:], func=AF.Identity,
                                         bias=W(f'bb_{l}_{b}', 96, 1))
                    ab = sp.tile([96, WIN], f32, tag="ab")
                    nc.scalar.activation(out=ab[:], in_=tdiv[:], func=AF.Abs,
                                         bias=W(f'bb_{l}_{b}', 96, 1))
                    nc.vector.tensor_scalar(out=lin[:], in0=lin[:], scalar1=0.505,
                                            scalar2=None, op0=OP.mult)
                    nc.vector.tensor_scalar(out=ab[:], in0=ab[:], scalar1=0.495,
                                            scalar2=None, op0=OP.mult)
                    nc.vector.tensor_tensor(out=h_T[b][:, w * WIN:(w + 1) * WIN],
                                            in0=lin[:], in1=ab[:], op=OP.add)

        # ---------- head ----------
        hid_T = [wp.tile([128, NCH * 128], f32, tag=f"hw{p}", name=f"hid{p}") for p in range(2)]
        for cs in range(0, NCH * 128, 512):
            ce = min(cs + 512, NCH * 128)
            w_ = ce - cs
            pf = psA.tile([96, 512], f32, tag="pbig")
            nc.tensor.matmul(out=pf[:, :w_], lhsT=W('fusion_Wt'),
                             rhs=h_T[0][:, cs:ce], start=True, stop=False)
            nc.tensor.matmul(out=pf[:, :w_], lhsT=W('fusion_Wb'),
                             rhs=h_T[1][:, cs:ce], start=False, stop=True)
            fus = sp.tile([96, 512], bf16, tag="fus")
            lin = sp.tile([96, 512], f32, tag="flin")
            nc.scalar.activation(out=lin[:, :w_], in_=pf[:, :w_], func=AF.Identity,
                                 bias=W('fusion_b', 96, 1))
            ab = sp.tile([96, 512], f32, tag="fab")
            nc.scalar.activation(out=ab[:, :w_], in_=pf[:, :w_], func=AF.Abs,
                                 bias=W('fusion_b', 96, 1))
            nc.vector.tensor_scalar(out=lin[:, :w_], in0=lin[:, :w_], scalar1=0.505,
                                    scalar2=None, op0=OP.mult)
            nc.vector.tensor_scalar(out=ab[:, :w_], in0=ab[:, :w_], scalar1=0.495,
                                    scalar2=None, op0=OP.mult)
            nc.vector.tensor_tensor(out=fus[:, :w_], in0=lin[:, :w_], in1=ab[:, :w_],
                                    op=OP.add)
            for p, (wk, bk) in enumerate([('pred_W1a', 'pred_b1a'), ('pred_W1b', 'pred_b1b')]):
                ph = psA.tile([128, 512], f32, tag="pbig")
                nc.tensor.matmul(out=ph[:, :w_], lhsT=W(wk, 96, 128), rhs=fus[:, :w_],
                                 start=True, stop=True)
                l2 = sp.tile([128, 512], f32, tag=f"l2{p}")
                a2 = sp.tile([128, 512], f32, tag=f"a2{p}")
                nc.scalar.activation(out=l2[:, :w_], in_=ph[:, :w_], func=AF.Identity,
                                     bias=W(bk, 128, 1))
                nc.scalar.activation(out=a2[:, :w_], in_=ph[:, :w_], func=AF.Abs,
                                     bias=W(bk, 128, 1))
                nc.vector.tensor_scalar(out=l2[:, :w_], in0=l2[:, :w_], scalar1=0.505,
                                        scalar2=None, op0=OP.mult)
                nc.vector.tensor_scalar(out=a2[:, :w_], in0=a2[:, :w_], scalar1=0.495,
                                        scalar2=None, op0=OP.mult)
                nc.vector.tensor_tensor(out=hid_T[p][:, cs:ce], in0=l2[:, :w_],
                                        in1=a2[:, :w_], op=OP.add)
        for ch in range(NCH):
            n0 = ch * 128
            nreal = max(0, min(NLOC - n0, 128))
            if nreal == 0:
                continue
            po = psp.tile([128, 2], f32, tag="mps")
            nc.tensor.matmul(out=po[:], lhsT=hid_T[0][:, n0:n0 + 128],
                             rhs=W('pred_W2a', 128, 2), start=True, stop=False)
            nc.tensor.matmul(out=po[:], lhsT=hid_T[1][:, n0:n0 + 128],
                             rhs=W('pred_W2b', 128, 2), start=False, stop=True)
            ot = sp.tile([128, 2], f32, tag="ot")
            nc.vector.tensor_tensor(out=ot[:], in0=po[:], in1=W('pred_b2', 128, 2), op=OP.add)
            nc.sync.dma_start(out=dout[n0:n0 + nreal, :], in_=ot[:nreal, :])

    nc.compile()
    return nc


def _make_runner(nc):
    """Build (once) a cached jitted shard_map wrapper around the compiled
    Bass module — same lowering as bass2jax.run_bass_via_pjrt, but the jit
    object is reused across calls so warm calls skip retrace/recompile."""
    import jax
    import jax.core as jcore
    from jax.experimental.shard_map import shard_map
    from jax.sharding import Mesh, PartitionSpec
    from concourse import bass2jax, mybir
    bass2jax.install_neuronx_cc_hook()

    partition_name = nc.partition_id_tensor.name if nc.partition_id_tensor else None
    in_names, out_names, out_avals, zero_shapes = [], [], [], []
    for alloc in nc.m.functions[0].allocations:
        if not isinstance(alloc, mybir.MemoryLocationSet):
            continue
        name = alloc.memorylocations[0].name
        if alloc.kind == "ExternalInput":
            if name != partition_name:
                in_names.append(name)
        elif alloc.kind == "ExternalOutput":
            shape = tuple(alloc.tensor_shape)
            dtype = mybir.dt.np(alloc.dtype)
            out_names.append(name)
            out_avals.append(jcore.ShapedArray(shape, dtype))
            zero_shapes.append((shape, dtype))
    n_params = len(in_names)
    n_outs = len(out_avals)
    all_in = list(in_names) + list(out_names)
    if partition_name is not None:
        all_in.append(partition_name)
    donate = tuple(range(n_params, n_params + n_outs))

    dbg_name = None
    if nc.dbg_addr is not None:
        assert not nc.dbg_callbacks
        dbg_name = nc.dbg_addr.name

    def _body(*args):
        operands = list(args)
        if partition_name is not None:
            operands.append(bass2jax.partition_id_tensor())
        outs = bass2jax._bass_exec_p.bind(
            *operands, out_avals=tuple(out_avals), in_names=tuple(all_in),
            out_names=tuple(out_names), lowering_input_output_aliases=(),
            sim_require_finite=True, sim_require_nnan=True, nc=nc)
        return tuple(outs)

    devices = jax.devices()[:NCORES]
    mesh = Mesh(np.asarray(devices), ("core",))
    in_specs = (PartitionSpec("core"),) * (n_params + n_outs)
    out_specs = (PartitionSpec("core"),) * n_outs
    fn = jax.jit(shard_map(_body, mesh=mesh, in_specs=in_specs,
                           out_specs=out_specs, check_rep=False),
                 donate_argnums=donate, keep_unused=True)
    return dict(fn=fn, in_names=in_names, out_names=out_names,
                out_avals=out_avals, zero_shapes=zero_shapes, dbg_name=dbg_name)


def _execute(runner, in_maps):
    n = len(in_maps)
    if runner['dbg_name'] is not None:
        z = np.zeros((1, 2), np.uint32)
        in_maps = [{**m, runner['dbg_name']: z} for m in in_maps]
    concat_in = [np.concatenate([np.asarray(in_maps[c][nm]) for c in range(n)], axis=0)
                 for nm in runner['in_names']]
    zeros = [np.zeros((n * s[0], *s[1:]), dt) for (s, dt) in runner['zero_shapes']]
    outs = runner['fn'](*concat_in, *zeros)
    return [{nm: np.asarray(outs[i]).reshape(n, *runner['out_avals'][i].shape)[c]
             for i, nm in enumerate(runner['out_names'])} for c in range(n)]


def _in_maps(x, pp, blob):
    bh, bf, b128 = blob
    xb = np.ascontiguousarray(
        np.clip(np.rint(x * XSCALE), -127, 127).astype(np.int8))
    maps = []
    for c in range(NCORES):
        maps.append({'x': xb[c * NLOC:(c + 1) * NLOC],
                     'ea8': pp['ea8'][c],
                     'dcw': pp['dcw'][c],
                     'gw': pp['gw'][c],
                     'wbh': bh,
                     'wbf': bf,
                     'wb128': b128})
    return maps


def kernel(**inputs):
    x = np.asarray(inputs['x'], np.float32)
    ei = np.asarray(inputs['edge_index'])
    ea = np.asarray(inputs['edge_attr'], np.float32)
    pp = _host_prep(x, ei, ea)
    if _CACHE.get('NSLOT') != pp['NSLOT']:
        _CACHE['nc'] = build_kernel(pp)
        _CACHE['runner'] = _make_runner(_CACHE['nc'])
        _CACHE['NSLOT'] = pp['NSLOT']
    blob = _wblob(inputs)
    res = _execute(_CACHE['runner'], _in_maps(x, pp, blob))
    out = np.concatenate([res[c]['out'] for c in range(NCORES)], axis=0)
    return out.astype(np.float32)


# revision 56
# speedup vs baseline: 25.6621x; 1.1659x over previous
"""BiLevelGAT (2-branch x 3-layer GATv2, N=50000, E=500000, D=96) on 8 TRN2 cores.

Sharding: nodes + incoming edges partitioned by dst; per-layer AllGather of a
bf16 per-node table [hl_loc 96|1|w_loc|w_glob|pad29|hl_glob 96|1|w_glob|pad30]
(512B rows) gathered per edge by src.

Math: lrelu(x) = 0.6x+0.4|x| splits the GATv2 logit into linear terms (per-src
w=exp(0.6*att.hl) folded into the softmax weight; per-dst term cancels in
softmax; per-edge ea term = on-device matmul of the edge table against
V=0.6*We@att) plus 0.4*att.|m| computed on device. Softmax max-subtraction
skipped (logits O(1), fp32 safe).

End-to-end wall time is dominated by host->device transfer over the axon
tunnel (~45MB/s), so inputs are aggressively compacted (~2MB/core):
 - x as int8*32 (1/32 folded into the layer-0 Wl/Wr), edge_attr as int8*32
   (1/32 folded into We and V); both add ~5e-3 rel err vs the 2e-2 gate.
 - per-edge one-hot "R" (dst selector + ea rows used as matmul rhs to form
   m = hr[dst] + ea@We) built on device per section: rank-1 broadcast matmul
   of the dst-col row, then is_equal against a partition iota covering ALL
   128 partitions (a partially-initialized Rt causes 0*NaN-garbage = NaN on
   a per-process-random basis -- caught by CoreSim's uninit checker).
 - per-slot dst cols for the scatter one-hot derived on device by per-block
   PE transposes of the dst-col row (invalid slots carry 147 -> miss both
   one-hots).
 - weights packed into 3 blobs (bf16 matrices / f32 96-row biases / f32
   128-row tail + iotas); gather indices shipped once [16, NSLOT/16] and
   replicated to 128 partitions on device.
The jitted shard_map wrapper is cached across calls (fresh jit per call
would retrace + recompile the XLA wrapper each time).
"""
import os
import sys
sys.path.insert(0, '/opt/trn_rl_repo')
os.environ.setdefault('NEURON_RT_RESET_CORES', '1')
import numpy as np
import ml_dtypes

BF16 = ml_dtypes.bfloat16

N, E, D, EDIM, L, DENSE, OUT = 50000, 500000, 96, 8, 3, 256, 2
NCORES = 8
NLOC = N // NCORES            # 6250
WIN, HALF = 96, 48
NWIN = (NLOC + WIN - 1) // WIN  # 66
NPAD = NWIN * WIN
NCH = (NPAD + 127) // 128     # chunks of 128 (PASS A / table)
SPLIT = 32768
TROW = 256
KMAX = 6

_CACHE = {}

# ---- packed weight blobs ----
# wbh [96, WBH] bf16: big matrices (Wl/Wr/att/We/fusion/pred_W1, V).
#   We_{l,b} stacked six-up in two 96-col groups at row bases 0/32/64
#   (any access must start at a partition base that is a multiple of 32);
#   V_l at row base 32*l (matmul rhs).
# wbf [96, WBF] f32: 96-row biases.
# wb128 [128, WB128] f32: 128-row biases/mats + iota constants.
WOFF = {}   # name -> (blob, row0, col0)
_cols = {'h': 0, 'f': 0, '128': 0}
def _al(blob, name, width, row=0):
    WOFF[name] = (blob, row, _cols[blob])
    _cols[blob] += width
for _l in range(L):
    for _b in range(2):
        _al('h', f'Wl_{_l}_{_b}', 96)
        _al('h', f'Wr_{_l}_{_b}', 96)
        _al('h', f'att_{_l}_{_b}', 1)
        _al('f', f'bb_{_l}_{_b}', 1)
for _i in range(6):
    _l, _b = divmod(_i, 2)
    WOFF[f'We_{_l}_{_b}'] = ('h', 32 * (_i % 3), _cols['h'] + 96 * (_i // 3))
_cols['h'] += 192
for _l in range(L):
    WOFF[f'V_{_l}'] = ('h', 32 * _l, _cols['h'])              # shared 2 cols
_cols['h'] += 2
_al('h', 'fusion_Wt', 96)
_al('h', 'fusion_Wb', 96)
_al('f', 'fusion_b', 1)
_al('h', 'pred_W1a', 128)
_al('h', 'pred_W1b', 128)
_al('128', 'pred_b1a', 1)
_al('128', 'pred_b1b', 1)
_al('128', 'pred_W2a', 2)
_al('128', 'pred_W2b', 2)
_al('128', 'pred_b2', 2)
_al('128', 'iotaf', 48)               # all 128 rows = arange(48)
_al('128', 'iota128', 1)              # rows = arange(128)
WBH, WBF, WB128 = _cols['h'], _cols['f'], _cols['128']
EASCALE = 32.0
XSCALE = 32.0


def _host_prep(x, edge_index, edge_attr):
    src = edge_index[0].astype(np.int64)
    dst = edge_index[1].astype(np.int64)
    mean_ea = edge_attr.mean(0).astype(np.float32)
    loop = np.arange(N, dtype=np.int64)
    src_a = np.concatenate([src, loop])
    dst_a = np.concatenate([dst, loop])
    ea_a = np.concatenate([edge_attr.astype(np.float32),
                           np.broadcast_to(mean_ea, (N, EDIM))], 0)

    owner = dst_a // NLOC
    dloc = dst_a - owner * NLOC
    win = dloc // WIN
    half = (dloc % WIN) // HALF
    stream = (src_a >= SPLIT).astype(np.int64)

    per_core = []
    secs = np.zeros((NCORES, NWIN, 2, 2), np.int64)
    for c in range(NCORES):
        m = owner == c
        s_c, d_c, e_c = src_a[m], dloc[m], ea_a[m]
        w_c, h_c, st_c = win[m], half[m], stream[m]
        sec = ((w_c * 2 + h_c) * 2 + st_c)
        order = np.argsort(sec * NLOC + d_c, kind='stable')
        s_c, d_c, e_c, sec = s_c[order], d_c[order], e_c[order], sec[order]
        st_c = st_c[order]
        per_core.append((s_c, d_c, e_c, sec, st_c))
        secs[c] = np.bincount(sec, minlength=NWIN * 4).reshape(NWIN, 2, 2)

    K = np.maximum((secs.max(0) + 127) // 128, 1)       # [NWIN, 2, 2]
    assert K.max() <= KMAX
    Kf = K.reshape(-1)
    sec_slot = np.zeros(NWIN * 4 + 1, np.int64)
    np.cumsum(Kf * 128, out=sec_slot[1:])
    NSLOT = int(sec_slot[-1])

    gidx = np.zeros((NCORES, NSLOT), np.int16)
    eav = np.zeros((NCORES, 9, NSLOT), np.float32)
    # invalid slots: dstcol=147 misses the 96-wide one-hot AND (after -48h)
    # the 48-wide es window -> no contribution
    eav[:, 8, :] = 147.0

    for c in range(NCORES):
        s_c, d_c, e_c, sec, st_c = per_core[c]
        counts = np.bincount(sec, minlength=NWIN * 4)
        starts = np.concatenate([[0], np.cumsum(counts)])[:-1]
        pos = np.arange(len(s_c)) - starts[sec]
        slot = sec_slot[sec] + pos
        gidx[c, slot] = (s_c - st_c * SPLIT).astype(np.int16)
        eav[c, 0:8, slot] = e_c  # advanced idx puts slot axis first: (nedge, 8)
        eav[c, 8, slot] = (d_c % WIN).astype(np.float32)

    gw = np.ascontiguousarray(
        gidx.reshape(NCORES, -1, 16).transpose(0, 2, 1))     # [NCORES, 16, NSLOT//16]

    ea_q = np.clip(np.rint(eav[:, 0:8, :] * EASCALE), -127, 127).astype(np.int8)
    return dict(K=K, Kf=Kf, sec_slot=sec_slot, NSLOT=NSLOT, NSEC=NWIN * 4,
                gw=gw, ea8=ea_q,
                dcw=eav[:, 8, :].astype(np.uint8).reshape(NCORES, 1, NSLOT))


def _wblob(w):
    bh = np.zeros((96, WBH), BF16)
    bf = np.zeros((96, WBF), np.float32)
    b128 = np.zeros((128, WB128), np.float32)
    blobs = {'h': bh, 'f': bf, '128': b128}
    def put(name, arr):
        a = np.asarray(arr, np.float32)
        if a.ndim == 1:
            a = a.reshape(-1, 1)
        bl, r0, c0 = WOFF[name]
        dst = blobs[bl]
        dst[r0:r0 + a.shape[0], c0:c0 + a.shape[1]] = a.astype(dst.dtype)
    for l in range(L):
        V = np.zeros((8, 2), np.float32)
        # x ships as int8 * XSCALE; fold 1/XSCALE into the layer-0 weights
        xs = XSCALE if l == 0 else 1.0
        for b, p in enumerate(['local', 'global']):
            put(f'Wl_{l}_{b}', np.asarray(w[f'{p}_Wl'][l], np.float32) / xs)
            put(f'Wr_{l}_{b}', np.asarray(w[f'{p}_Wr'][l], np.float32) / xs)
            put(f'att_{l}_{b}', w[f'{p}_att'][l])
            put(f'bb_{l}_{b}', w[f'{p}_b'][l])
            # ea ships as int8 * EASCALE; fold 1/EASCALE into We and V
            put(f'We_{l}_{b}', np.asarray(w[f'{p}_We'][l], np.float32) / EASCALE)
            V[:, b] = (0.6 / EASCALE) * (np.asarray(w[f'{p}_We'][l], np.float32)
                                         @ np.asarray(w[f'{p}_att'][l], np.float32))
        put(f'V_{l}', V)
    put('fusion_Wt', w['fusion_W'][:96])
    put('fusion_Wb', w['fusion_W'][96:])
    put('fusion_b', w['fusion_b'])
    put('pred_W1a', w['pred_W1'][:, :128])
    put('pred_W1b', w['pred_W1'][:, 128:])
    put('pred_b1a', w['pred_b1'][:128])
    put('pred_b1b', w['pred_b1'][128:])
    put('pred_W2a', w['pred_W2'][:128])
    put('pred_W2b', w['pred_W2'][128:])
    put('pred_b2', np.broadcast_to(np.asarray(w['pred_b2']).reshape(1, 2), (128, 2)))
    put('iotaf', np.broadcast_to(np.arange(48, dtype=np.float32), (128, 48)))
    put('iota128', np.arange(128, dtype=np.float32))
    return bh, bf, b128


def build_kernel(pp):
    import os as _os
    SKIP_EDGE = _os.environ.get('SKIP_EDGE', '0') == '1'
    SKIP_GATHER = _os.environ.get('SKIP_GATHER', '0') == '1'
    from concourse import mybir, bacc
    import concourse.tile as tile
    Kf, sec_slot, NSLOT = pp['Kf'], pp['sec_slot'], pp['NSLOT']
    f32, bf16, i16 = mybir.dt.float32, mybir.dt.bfloat16, mybir.dt.int16
    AF = mybir.ActivationFunctionType
    OP = mybir.AluOpType

    i8, u8 = mybir.dt.int8, mybir.dt.uint8
    nc = bacc.Bacc("TRN2", target_bir_lowering=False, debug=False, num_devices=NCORES)
    dx = nc.dram_tensor("x", [NLOC, D], i8, kind="ExternalInput")
    dea8 = nc.dram_tensor("ea8", [8, NSLOT], i8, kind="ExternalInput")
    ddcw = nc.dram_tensor("dcw", [1, NSLOT], u8, kind="ExternalInput")
    dgw = nc.dram_tensor("gw", [16, NSLOT // 16], i16, kind="ExternalInput")
    # weight blobs ship 1/8 per core (identical content host-side) and are
    # reassembled on device with an AllGather -- saves 7/8 of the blob bytes
    # on the transfer-bound axon tunnel
    dwbh = nc.dram_tensor("wbh", [96 // NCORES, WBH], bf16, kind="ExternalInput")
    wbh_loc = nc.dram_tensor("wbh_loc", [96 // NCORES, WBH], bf16)
    wbh_sh = nc.dram_tensor("wbh_sh", [96, WBH], bf16, addr_space="Shared")
    dwbf = nc.dram_tensor("wbf", [96, WBF], f32, kind="ExternalInput")
    dwb128 = nc.dram_tensor("wb128", [128 // NCORES, WB128], f32, kind="ExternalInput")
    wb128_loc = nc.dram_tensor("wb128_loc", [128 // NCORES, WB128], f32)
    wb128_sh = nc.dram_tensor("wb128_sh", [128, WB128], f32, addr_space="Shared")
    dout = nc.dram_tensor("out", [NLOC, OUT], f32, kind="ExternalOutput")

    tab_slice = nc.dram_tensor("tab_slice", [NLOC, TROW], bf16)
    tab_sh = nc.dram_tensor("tab_sh", [N, TROW], bf16, addr_space="Shared")
    tab = nc.dram_tensor("tab", [N, TROW], bf16)

    def wo(name, rows=96, width=None):
        w_ = width if width is not None else 96
        return (WOFF[name], WOFF[name] + w_, rows)

    with tile.TileContext(nc) as tc:
      with (tc.tile_pool(name="const", bufs=1) as cp,
            tc.tile_pool(name="hp", bufs=1) as hp,
            tc.tile_pool(name="wp", bufs=1) as wp,
            tc.tile_pool(name="sp", bufs=3) as sp,
            tc.tile_pool(name="gpool", bufs=2) as gpl,
            tc.tile_pool(name="ps", bufs=2, space="PSUM") as psp,
            tc.tile_pool(name="psA", bufs=2, space="PSUM") as psA,
            tc.tile_pool(name="psagg", bufs=1, space="PSUM") as psG):

        ident = cp.tile([128, 128], bf16)
        nc.sync.dma_start(out=ident[:], in_=nc.inline_tensor(np.eye(128, dtype=BF16), name="idb").ap())
        identf = cp.tile([128, 128], f32)
        nc.sync.dma_start(out=identf[:], in_=nc.inline_tensor(np.eye(128, dtype=np.float32), name="idf").ap())
        nc.sync.dma_start(out=wbh_loc[:], in_=dwbh[:])
        nc.sync.dma_start(out=wb128_loc[:], in_=dwb128[:])
        nc.gpsimd.collective_compute(
            "AllGather", mybir.AluOpType.bypass,
            replica_groups=[list(range(NCORES))],
            ins=[wbh_loc[:]], outs=[wbh_sh[:]])
        nc.gpsimd.collective_compute(
            "AllGather", mybir.AluOpType.bypass,
            replica_groups=[list(range(NCORES))],
            ins=[wb128_loc[:]], outs=[wb128_sh[:]])
        wbh_t = cp.tile([96, WBH], bf16)
        nc.sync.dma_start(out=wbh_t[:], in_=wbh_sh[:])
        wbf_t = cp.tile([96, WBF], f32)
        nc.sync.dma_start(out=wbf_t[:], in_=dwbf[:])
        wb128_t = cp.tile([128, WB128], f32)
        nc.sync.dma_start(out=wb128_t[:], in_=wb128_sh[:])
        gw_t = cp.tile([128, NSLOT // 16], i16)
        for k in range(8):
            nc.sync.dma_start(out=gw_t[16 * k:16 * (k + 1), :], in_=dgw[:])

        blobs_t = {'h': wbh_t, 'f': wbf_t, '128': wb128_t}
        def W(name, rows=96, width=96):
            bl, r0, c0 = WOFF[name]
            return blobs_t[bl][r0:r0 + rows, c0:c0 + width]

        iota_t = wb128_t[:, WOFF['iotaf'][2]:WOFF['iotaf'][2] + 48]

        one1 = cp.tile([1, 96], f32)
        nc.vector.memset(one1[:], 1.0)
        ones1 = cp.tile([1, 128], bf16)
        nc.vector.memset(ones1[:], 1.0)
        att04 = {}
        for l in range(L):
            for b in range(2):
                att04[(l, b)] = cp.tile([96, 1], bf16, tag=f"att04_{l}_{b}", name=f"att04_{l}_{b}")
                nc.vector.tensor_scalar(out=att04[(l, b)][:], in0=W(f'att_{l}_{b}', 96, 1),
                                        scalar1=0.4, scalar2=None, op0=OP.mult)
        Vt = {}
        for l in range(L):
            Vt[l] = cp.tile([8, 2], bf16, tag=f"V_{l}", name=f"V_{l}")
            nc.vector.tensor_copy(out=Vt[l][:], in_=W(f'V_{l}', 8, 2))

        # h_T feature-major [96, NPAD] (cols beyond NLOC are pad)
        h_T = [hp.tile([96, NCH * 128], bf16, tag=f"h{b}", name=f"h{b}") for b in range(2)]
        for ch in range(NCH):
            n0 = ch * 128
            nreal = max(0, min(NLOC - n0, 128))
            xin8 = sp.tile([128, 128], i8, tag="xin8")
            nc.vector.memset(xin8[:], 0)
            if nreal > 0:
                nc.sync.dma_start(out=xin8[:nreal, :96], in_=dx[n0:n0 + nreal, :])
            xin = sp.tile([128, 128], bf16, tag="xin")
            nc.vector.tensor_copy(out=xin[:], in_=xin8[:])
            pt = psA.tile([128, 128], bf16, tag="pbig")
            nc.tensor.transpose(out=pt[:], in_=xin[:], identity=ident[:])
            for b in range(2):
                nc.vector.tensor_copy(out=h_T[b][:, n0:n0 + 128], in_=pt[:96, :])

        hw_T = [wp.tile([96, NCH * 128], bf16, tag=f"hw{b}", name=f"hw{b}") for b in range(2)]

        for l in range(L):
            # ---------- PASS A ----------
            for b in range(2):
                for cs in range(0, NCH * 128, 512):
                    ce = min(cs + 512, NCH * 128)
                    w_ = ce - cs
                    pl = psA.tile([96, 512], f32, tag="pbig")
                    nc.tensor.matmul(out=pl[:, :w_], lhsT=W(f'Wl_{l}_{b}'),
                                     rhs=h_T[b][:, cs:ce], start=True, stop=True)
                    nc.vector.tensor_copy(out=hw_T[b][:, cs:ce], in_=pl[:, :w_])
            # table slice + allgather
            for ch in range(NCH):
                n0 = ch * 128
                nreal = max(0, min(NLOC - n0, 128))
                if nreal == 0:
                    continue
                stg = sp.tile([128, TROW], bf16, tag="stg")
                nc.vector.memset(stg[:], 0.0)
                for b in range(2):
                    pt = psA.tile([128, 128], bf16, tag="pbig")
                    nc.tensor.transpose(out=pt[:, :96], in_=hw_T[b][:, n0:n0 + 128],
                                        identity=ident[:96, :96])
                    nc.vector.tensor_copy(out=stg[:, b * 128:b * 128 + 96], in_=pt[:, :96])
                    # w = exp(0.6*att.hl) for this chunk; ones at ext row 32
                    pphi = psA.tile([1, 128], f32, tag="pbig")
                    nc.tensor.matmul(out=pphi[:], lhsT=W(f'att_{l}_{b}', 96, 1),
                                     rhs=hw_T[b][:, n0:n0 + 128], start=True, stop=True)
                    ext = sp.tile([64, 128], f32, tag="ext")
                    nc.scalar.activation(out=ext[0:1, :], in_=pphi[:], func=AF.Exp, scale=0.6)
                    nc.vector.memset(ext[32:33, :], 1.0)
                    pt2 = psA.tile([128, 64], f32, tag="pbig")
                    nc.tensor.transpose(out=pt2[:], in_=ext[:], identity=identf[:64, :64])
                    nc.vector.tensor_copy(out=stg[:, b * 128 + 96:b * 128 + 97], in_=pt2[:, 32:33])
                    nc.vector.tensor_copy(out=stg[:, b * 128 + 97:b * 128 + 98], in_=pt2[:, 0:1])
                nc.vector.tensor_copy(out=stg[:, 98:99], in_=stg[:, 225:226])
                nc.sync.dma_start(out=tab_slice[n0:n0 + nreal, :], in_=stg[:nreal, :])
            nc.gpsimd.collective_compute(
                "AllGather", mybir.AluOpType.bypass,
                replica_groups=[list(range(NCORES))],
                ins=[tab_slice[:]], outs=[tab_sh[:]],
            )
            nc.sync.dma_start(out=tab[:], in_=tab_sh[:])

            # ---------- edge phase ----------
            for w in range(0 if not SKIP_EDGE else NWIN, NWIN):
                aggp = {}
                first = {b: True for b in range(2)}
                nagg = {b: 0 for b in range(2)}
                tot = {b: sum(int(Kf[(w * 2 + h) * 2 + s]) for h in range(2) for s in range(2))
                       for b in range(2)}
                for b in range(2):
                    aggp[b] = psG.tile([97, WIN], f32, tag=f"agg{b}", name=f"agg{b}")
                # base lhsT per branch for this window (hr = h @ Wr computed here)
                basel = {}
                for b in range(2):
                    phr = psA.tile([96, WIN], f32, tag="pbig")
                    nc.tensor.matmul(out=phr[:], lhsT=W(f'Wr_{l}_{b}'),
                                     rhs=h_T[b][:, w * WIN:(w + 1) * WIN],
                                     start=True, stop=True)
                    hrs = sp.tile([96, WIN], f32, tag="hrs")
                    nc.vector.tensor_copy(out=hrs[:], in_=phr[:])
                    pt = psA.tile([WIN, 96], f32, tag="pbig")
                    nc.tensor.transpose(out=pt[:], in_=hrs[:], identity=identf[:96, :96])
                    bl = sp.tile([128, 96], bf16, tag=f"basel{b}", name=f"basel{b}")
                    nc.vector.memset(bl[:], 0.0)
                    nc.vector.tensor_copy(out=bl[:WIN, :], in_=pt[:])
                    nc.vector.tensor_copy(out=bl[WIN:WIN + 8, :], in_=W(f'We_{l}_{b}', 8, 96))
                    basel[b] = bl
                for h in range(2):
                    for s in range(2):
                        si = (w * 2 + h) * 2 + s
                        Ks = int(Kf[si])
                        sl0 = int(sec_slot[si])
                        nsl = Ks * 128
                        g = gpl.tile([128, KMAX, TROW], bf16, tag="gath")
                        if SKIP_GATHER:
                            nc.vector.memset(g[:, :Ks, :], 0.0)
                        else:
                            nc.gpsimd.dma_gather(
                                out_ap=g[:, :Ks, :],
                                in_ap=tab[SPLIT:, :] if s else tab[:SPLIT, :],
                                idxs_ap=gw_t[:, sl0 // 16:(sl0 + nsl) // 16],
                                num_idxs=nsl, num_idxs_reg=nsl, elem_size=TROW)
                        # compact edge table slice (i8/u8 on the wire -> bf16)
                        ea8s = sp.tile([8, KMAX * 128], i8, tag="ea8s")
                        nc.sync.dma_start(out=ea8s[:, :nsl], in_=dea8[:, sl0:sl0 + nsl])
                        eavs = sp.tile([8, KMAX * 128], bf16, tag="eavs")
                        nc.vector.tensor_copy(out=eavs[:, :nsl], in_=ea8s[:, :nsl])
                        dc8s = sp.tile([1, KMAX * 128], u8, tag="dc8s")
                        nc.sync.dma_start(out=dc8s[:, :nsl], in_=ddcw[:, sl0:sl0 + nsl])
                        dcw = sp.tile([1, KMAX * 128], bf16, tag="dcw")
                        nc.vector.tensor_copy(out=dcw[:, :nsl], in_=dc8s[:, :nsl])
                        # build Rt on device: rows 0-95 one-hot(dstcol), 96-103
                        # ea. is_equal covers ALL 128 rows (rows 96-127 can
                        # never match dstcol<=147... rows 96-103 overwritten by
                        # ea below, 104-127 exact zeros) so no partition of Rt
                        # is left uninitialized -- the mps matmul reads all 128
                        # partitions and 0 * NaN-garbage = NaN.
                        Rt = sp.tile([128, KMAX * 128], bf16, tag="Rt")
                        for c0 in range(0, nsl, 512):
                            cw = min(512, nsl - c0)
                            pbc = psA.tile([128, 512], f32, tag="pbig")
                            nc.tensor.matmul(out=pbc[:, :cw], lhsT=ones1[:],
                                             rhs=dcw[0:1, c0:c0 + cw], start=True, stop=True)
                            nc.vector.tensor_scalar(out=Rt[:, c0:c0 + cw],
                                                    in0=pbc[:, :cw],
                                                    scalar1=W('iota128', 128, 1),
                                                    scalar2=None, op0=OP.is_equal)
                            nc.vector.tensor_copy(out=Rt[96:104, c0:c0 + cw],
                                                  in_=eavs[0:8, c0:c0 + cw])
                        # per-slot dst col within half-window: transpose dcw
                        # blocks to partitions, -48h (invalid 147 -> 147/99)
                        dcsec = sp.tile([128, KMAX], f32, tag="dcs")
                        for j in range(Ks):
                            pt1 = psA.tile([128, 1], bf16, tag="pbig")
                            nc.tensor.transpose(out=pt1[:],
                                                in_=dcw[0:1, j * 128:(j + 1) * 128],
                                                identity=ident[:1, :1])
                            nc.vector.tensor_scalar(out=dcsec[:, j:j + 1], in0=pt1[:],
                                                    scalar1=float(-48 * h), scalar2=None,
                                                    op0=OP.add)
                        lgp = psp.tile([128, 16], f32, tag="lgp", bufs=1)
                        for j0 in range(0, Ks, 4):
                            jw = min(4, Ks - j0)
                            for b in range(2):
                                mps = psp.tile([96, 512], f32, tag="mps")
                                nc.tensor.matmul(out=mps[:, :jw * 128], lhsT=basel[b][:],
                                                 rhs=Rt[:, j0 * 128:(j0 + jw) * 128],
                                                 start=True, stop=False)
                                for dj in range(jw):
                                    j = j0 + dj
                                    nc.tensor.matmul(out=mps[:, dj * 128:(dj + 1) * 128],
                                                     lhsT=g[:, j, b * 128:b * 128 + 96],
                                                     rhs=ident[:], start=False,
                                                     stop=(dj == jw - 1),
                                                     skip_group_check=True)
                                am = sp.tile([96, 512], bf16, tag="am")
                                nc.scalar.activation(out=am[:, :jw * 128],
                                                     in_=mps[:, :jw * 128], func=AF.Abs)
                                for dj in range(jw):
                                    j = j0 + dj
                                    nc.tensor.matmul(out=lgp[:, 2 * j + b:2 * j + b + 1],
                                                     lhsT=am[:, dj * 128:(dj + 1) * 128],
                                                     rhs=att04[(l, b)][:],
                                                     start=(j == 0 and b == 0), stop=False,
                                                     skip_group_check=True)
                        # += 0.6*ea.(We@att) per branch (cols 2j|2j+1), on device
                        for j in range(Ks):
                            nc.tensor.matmul(out=lgp[:, 2 * j:2 * j + 2],
                                             lhsT=eavs[0:8, j * 128:(j + 1) * 128],
                                             rhs=Vt[l][:], start=False,
                                             stop=(j == Ks - 1), skip_group_check=True)
                        exw = sp.tile([128, 16], f32, tag="exw")
                        nc.scalar.activation(out=exw[:, :2 * Ks], in_=lgp[:, :2 * Ks],
                                             func=AF.Exp)
                        nc.vector.tensor_tensor(
                            out=exw[:, :2 * Ks].rearrange("p (j b) -> p j b", b=2),
                            in0=exw[:, :2 * Ks].rearrange("p (j b) -> p j b", b=2),
                            in1=g[:, :Ks, 97:99], op=OP.mult)
                        for j in range(Ks):
                            for b in range(2):
                                es = sp.tile([128, HALF], bf16, tag="es")
                                nc.vector.tensor_scalar(
                                    out=es[:], in0=iota_t, scalar1=dcsec[:, j:j + 1],
                                    scalar2=exw[:, 2 * j + b:2 * j + b + 1],
                                    op0=OP.is_equal, op1=OP.mult)
                                nagg[b] += 1
                                nc.tensor.matmul(out=aggp[b][:, h * HALF:(h + 1) * HALF],
                                                 lhsT=g[:, j, b * 128:b * 128 + 97],
                                                 rhs=es[:],
                                                 start=first[b], stop=(nagg[b] == tot[b]),
                                                 skip_group_check=True)
                                first[b] = False
                # finalize window -> h_T
                for b in range(2):
                    num = sp.tile([96, WIN], f32, tag="num")
                    den = sp.tile([1, WIN], f32, tag="den")
                    nc.vector.tensor_copy(out=num[:], in_=aggp[b][:96, :])
                    nc.vector.tensor_scalar(out=den[:], in0=aggp[b][96:97, :],
                                            scalar1=1e-30, scalar2=None, op0=OP.add)
                    rec = sp.tile([1, WIN], f32, tag="rec")
                    nc.vector.reciprocal(out=rec[:], in_=den[:])
                    pb = psp.tile([96, WIN], f32, tag="mps")
                    nc.tensor.matmul(out=pb[:], lhsT=one1[:], rhs=rec[:], start=True, stop=True)
                    tdiv = sp.tile([96, WIN], f32, tag="tdiv")
                    nc.vector.tensor_tensor(out=tdiv[:], in0=num[:], in1=pb[:], op=OP.mult)
                    lin = sp.tile([96, WIN], f32, tag="lin")
                    nc.scalar.activation(out=lin[:], in_=tdiv[:], func=AF.Identity,
                                         bias=W(f'bb_{l}_{b}', 96, 1))
                    ab = sp.tile([96, WIN], f32, tag="ab")
                    nc.scalar.activation(out=ab[:], in_=tdiv[:], func=AF.Abs,
                                         bias=W(f'bb_{l}_{b}', 96, 1))
                    nc.vector.tensor_scalar(out=lin[:], in0=lin[:], scalar1=0.505,
                                            scalar2=None, op0=OP.mult)
                    nc.vector.tensor_scalar(out=ab[:], in0=ab[:], scalar1=0.495,
                                            scalar2=None, op0=OP.mult)
                    nc.vector.tensor_tensor(out=h_T[b][:, w * WIN:(w + 1) * WIN],
                                            in0=lin[:], in1=ab[:], op=OP.add)

        # ---------- head ----------
        hid_T = [wp.tile([128, NCH * 128], f32, tag=f"hw{p}", name=f"hid{p}") for p in range(2)]
        for cs in range(0, NCH * 128, 512):
            ce = min(cs + 512, NCH * 128)
            w_ = ce - cs
            pf = psA.tile([96, 512], f32, tag="pbig")
            nc.tensor.matmul(out=pf[:, :w_], lhsT=W('fusion_Wt'),
                             rhs=h_T[0][:, cs:ce], start=True, stop=False)
            nc.tensor.matmul(out=pf[:, :w_], lhsT=W('fusion_Wb'),
                             rhs=h_T[1][:, cs:ce], start=False, stop=True)
            fus = sp.tile([96, 512], bf16, tag="fus")
            lin = sp.tile([96, 512], f32, tag="flin")
            nc.scalar.activation(out=lin[:, :w_], in_=pf[:, :w_], func=AF.Identity,
                                 bias=W('fusion_b', 96, 1))
            ab = sp.tile([96, 512], f32, tag="fab")
            nc.scalar.activation(out=ab[:, :w_], in_=pf[:, :w_], func=AF.Abs,
                                 bias=W('fusion_b', 96, 1))
            nc.vector.tensor_scalar(out=lin[:, :w_], in0=lin[:, :w_], scalar1=0.505,
                                    scalar2=None, op0=OP.mult)
            nc.vector.tensor_scalar(out=ab[:, :w_], in0=ab[:, :w_], scalar1=0.495,
                                    scalar2=None, op0=OP.mult)
            nc.vector.tensor_tensor(out=fus[:, :w_], in0=lin[:, :w_], in1=ab[:, :w_],
                                    op=OP.add)
            for p, (wk, bk) in enumerate([('pred_W1a', 'pred_b1a'), ('pred_W1b', 'pred_b1b')]):
                ph = psA.tile([128, 512], f32, tag="pbig")
                nc.tensor.matmul(out=ph[:, :w_], lhsT=W(wk, 96, 128), rhs=fus[:, :w_],
                                 start=True, stop=True)
                l2 = sp.tile([128, 512], f32, tag=f"l2{p}")
                a2 = sp.tile([128, 512], f32, tag=f"a2{p}")
                nc.scalar.activation(out=l2[:, :w_], in_=ph[:, :w_], func=AF.Identity,
                                     bias=W(bk, 128, 1))
                nc.scalar.activation(out=a2[:, :w_], in_=ph[:, :w_], func=AF.Abs,
                                     bias=W(bk, 128, 1))
                nc.vector.tensor_scalar(out=l2[:, :w_], in0=l2[:, :w_], scalar1=0.505,
                                        scalar2=None, op0=OP.mult)
                nc.vector.tensor_scalar(out=a2[:, :w_], in0=a2[:, :w_], scalar1=0.495,
                                        scalar2=None, op0=OP.mult)
                nc.vector.tensor_tensor(out=hid_T[p][:, cs:ce], in0=l2[:, :w_],
                                        in1=a2[:, :w_], op=OP.add)
        for ch in range(NCH):
            n0 = ch * 128
            nreal = max(0, min(NLOC - n0, 128))
            if nreal == 0:
                continue
            po = psp.tile([128, 2], f32, tag="mps")
            nc.tensor.matmul(out=po[:], lhsT=hid_T[0][:, n0:n0 + 128],
                             rhs=W('pred_W2a', 128, 2), start=True, stop=False)
            nc.tensor.matmul(out=po[:], lhsT=hid_T[1][:, n0:n0 + 128],
                             rhs=W('pred_W2b', 128, 2), start=False, stop=True)
            ot = sp.tile([128, 2], f32, tag="ot")
            nc.vector.tensor_tensor(out=ot[:], in0=po[:], in1=W('pred_b2', 128, 2), op=OP.add)
            nc.sync.dma_start(out=dout[n0:n0 + nreal, :], in_=ot[:nreal, :])

    nc.compile()
    return nc


def _make_runner(nc):
    """Build (once) a cached jitted shard_map wrapper around the compiled
    Bass module — same lowering as bass2jax.run_bass_via_pjrt, but the jit
    object is reused across calls so warm calls skip retrace/recompile."""
    import jax
    import jax.core as jcore
    from jax.experimental.shard_map import shard_map
    from jax.sharding import Mesh, PartitionSpec
    from concourse import bass2jax, mybir
    bass2jax.install_neuronx_cc_hook()

    partition_name = nc.partition_id_tensor.name if nc.partition_id_tensor else None
    in_names, out_names, out_avals, zero_shapes = [], [], [], []
    for alloc in nc.m.functions[0].allocations:
        if not isinstance(alloc, mybir.MemoryLocationSet):
            continue
        name = alloc.memorylocations[0].name
        if alloc.kind == "ExternalInput":
            if name != partition_name:
                in_names.append(name)
        elif alloc.kind == "ExternalOutput":
            shape = tuple(alloc.tensor_shape)
            dtype = mybir.dt.np(alloc.dtype)
            out_names.append(name)
            out_avals.append(jcore.ShapedArray(shape, dtype))
            zero_shapes.append((shape, dtype))
    n_params = len(in_names)
    n_outs = len(out_avals)
    all_in = list(in_names) + list(out_names)
    if partition_name is not None:
        all_in.append(partition_name)
    donate = tuple(range(n_params, n_params + n_outs))

    dbg_name = None
    if nc.dbg_addr is not None:
        assert not nc.dbg_callbacks
        dbg_name = nc.dbg_addr.name

    def _body(*args):
        operands = list(args)
        if partition_name is not None:
            operands.append(bass2jax.partition_id_tensor())
        outs = bass2jax._bass_exec_p.bind(
            *operands, out_avals=tuple(out_avals), in_names=tuple(all_in),
            out_names=tuple(out_names), lowering_input_output_aliases=(),
            sim_require_finite=True, sim_require_nnan=True, nc=nc)
        return tuple(outs)

    devices = jax.devices()[:NCORES]
    mesh = Mesh(np.asarray(devices), ("core",))
    in_specs = (PartitionSpec("core"),) * (n_params + n_outs)
    out_specs = (PartitionSpec("core"),) * n_outs
    fn = jax.jit(shard_map(_body, mesh=mesh, in_specs=in_specs,
                           out_specs=out_specs, check_rep=False),
                 donate_argnums=donate, keep_unused=True)
    return dict(fn=fn, in_names=in_names, out_names=out_names,
                out_avals=out_avals, zero_shapes=zero_shapes, dbg_name=dbg_name)


def _execute(runner, in_maps):
    n = len(in_maps)
    if runner['dbg_name'] is not None:
        z = np.zeros((1, 2), np.uint32)
        in_maps = [{**m, runner['dbg_name']: z} for m in in_maps]
    concat_in = [np.concatenate([np.asarray(in_maps[c][nm]) for c in range(n)], axis=0)
                 for nm in runner['in_names']]
    zeros = [np.zeros((n * s[0], *s[1:]), dt) for (s, dt) in runner['zero_shapes']]
    outs = runner['fn'](*concat_in, *zeros)
    return [{nm: np.asarray(outs[i]).reshape(n, *runner['out_avals'][i].shape)[c]
             for i, nm in enumerate(runner['out_names'])} for c in range(n)]


def _in_maps(x, pp, blob):
    bh, bf, b128 = blob
    xb = np.ascontiguousarray(
        np.clip(np.rint(x * XSCALE), -127, 127).astype(np.int8))
    maps = []
    for c in range(NCORES):
        maps.append({'x': xb[c * NLOC:(c + 1) * NLOC],
                     'ea8': pp['ea8'][c],
                     'dcw': pp['dcw'][c],
                     'gw': pp['gw'][c],
                     'wbh': np.ascontiguousarray(bh[c * 12:(c + 1) * 12]),
                     'wbf': bf,
                     'wb128': np.ascontiguousarray(b128[c * 16:(c + 1) * 16])})
    return maps


def kernel(**inputs):
    x = np.asarray(inputs['x'], np.float32)
    ei = np.asarray(inputs['edge_index'])
    ea = np.asarray(inputs['edge_attr'], np.float32)
    pp = _host_prep(x, ei, ea)
    if _CACHE.get('NSLOT') != pp['NSLOT']:
        _CACHE['nc'] = build_kernel(pp)
        _CACHE['runner'] = _make_runner(_CACHE['nc'])
        _CACHE['NSLOT'] = pp['NSLOT']
    blob = _wblob(inputs)
    res = _execute(_CACHE['runner'], _in_maps(x, pp, blob))
    out = np.concatenate([res[c]['out'] for c in range(NCORES)], axis=0)
    return out.astype(np.float32)


# revision 58
# speedup vs baseline: 27.2096x; 1.0603x over previous
"""BiLevelGAT (2-branch x 3-layer GATv2, N=50000, E=500000, D=96) on 8 TRN2 cores.

Sharding: nodes + incoming edges partitioned by dst; per-layer AllGather of a
bf16 per-node table [hl_loc 96|1|w_loc|w_glob|pad29|hl_glob 96|1|w_glob|pad30]
(512B rows) gathered per edge by src.

Math: lrelu(x) = 0.6x+0.4|x| splits the GATv2 logit into linear terms (per-src
w=exp(0.6*att.hl) folded into the softmax weight; per-dst term cancels in
softmax; per-edge ea term = on-device matmul of the edge table against
V=0.6*We@att) plus 0.4*att.|m| computed on device. Softmax max-subtraction
skipped (logits O(1), fp32 safe).

End-to-end wall time is dominated by host->device transfer over the axon
tunnel (~45MB/s), so inputs are aggressively compacted (~2MB/core):
 - x as int8*32 (1/32 folded into the layer-0 Wl/Wr), edge_attr as int8*32
   (1/32 folded into We and V); both add ~5e-3 rel err vs the 2e-2 gate.
 - per-edge one-hot "R" (dst selector + ea rows used as matmul rhs to form
   m = hr[dst] + ea@We) built on device per section: rank-1 broadcast matmul
   of the dst-col row, then is_equal against a partition iota covering ALL
   128 partitions (a partially-initialized Rt causes 0*NaN-garbage = NaN on
   a per-process-random basis -- caught by CoreSim's uninit checker).
 - per-slot dst cols for the scatter one-hot derived on device by per-block
   PE transposes of the dst-col row (invalid slots carry 147 -> miss both
   one-hots).
 - weights packed into 3 blobs (bf16 matrices / f32 96-row biases / f32
   128-row tail + iotas); gather indices shipped once [16, NSLOT/16] and
   replicated to 128 partitions on device.
The jitted shard_map wrapper is cached across calls (fresh jit per call
would retrace + recompile the XLA wrapper each time).
"""
import os
import sys
sys.path.insert(0, '/opt/trn_rl_repo')
os.environ.setdefault('NEURON_RT_RESET_CORES', '1')
import numpy as np
import ml_dtypes

BF16 = ml_dtypes.bfloat16

N, E, D, EDIM, L, DENSE, OUT = 50000, 500000, 96, 8, 3, 256, 2
NCORES = 8
NLOC = N // NCORES            # 6250
WIN, HALF = 96, 48
NWIN = (NLOC + WIN - 1) // WIN  # 66
NPAD = NWIN * WIN
NCH = (NPAD + 127) // 128     # chunks of 128 (PASS A / table)
SPLIT = 32768
TROW = 256
KMAX = 6

_CACHE = {}

# ---- packed weight blobs ----
# wbh [96, WBH] bf16: big matrices (Wl/Wr/att/We/fusion/pred_W1, V).
#   We_{l,b} stacked six-up in two 96-col groups at row bases 0/32/64
#   (any access must start at a partition base that is a multiple of 32);
#   V_l at row base 32*l (matmul rhs).
# wbf [96, WBF] f32: 96-row biases.
# wb128 [128, WB128] f32: 128-row biases/mats + iota constants.
WOFF = {}   # name -> (blob, row0, col0)
_cols = {'h': 0, 'f': 0, '128': 0}
def _al(blob, name, width, row=0):
    WOFF[name] = (blob, row, _cols[blob])
    _cols[blob] += width
for _l in range(L):
    for _b in range(2):
        _al('h', f'Wl_{_l}_{_b}', 96)
        _al('h', f'Wr_{_l}_{_b}', 96)
        _al('h', f'att_{_l}_{_b}', 1)
        _al('f', f'bb_{_l}_{_b}', 1)
for _i in range(6):
    _l, _b = divmod(_i, 2)
    WOFF[f'We_{_l}_{_b}'] = ('h', 32 * (_i % 3), _cols['h'] + 96 * (_i // 3))
_cols['h'] += 192
for _l in range(L):
    WOFF[f'V_{_l}'] = ('h', 32 * _l, _cols['h'])              # shared 2 cols
_cols['h'] += 2
_al('h', 'fusion_Wt', 96)
_al('h', 'fusion_Wb', 96)
_al('f', 'fusion_b', 1)
_al('h', 'pred_W1a', 128)
_al('h', 'pred_W1b', 128)
_al('128', 'pred_b1a', 1)
_al('128', 'pred_b1b', 1)
_al('128', 'pred_W2a', 2)
_al('128', 'pred_W2b', 2)
_al('128', 'pred_b2', 2)
_al('128', 'iotaf', 48)               # all 128 rows = arange(48)
_al('128', 'iota128', 1)              # rows = arange(128)
WBH, WBF, WB128 = _cols['h'], _cols['f'], _cols['128']
EASCALE = 32.0
XSCALE = 32.0


def _host_prep(x, edge_index, edge_attr):
    src = edge_index[0].astype(np.int64)
    dst = edge_index[1].astype(np.int64)
    mean_ea = edge_attr.mean(0).astype(np.float32)
    loop = np.arange(N, dtype=np.int64)
    src_a = np.concatenate([src, loop])
    dst_a = np.concatenate([dst, loop])
    ea_a = np.concatenate([edge_attr.astype(np.float32),
                           np.broadcast_to(mean_ea, (N, EDIM))], 0)

    owner = dst_a // NLOC
    dloc = dst_a - owner * NLOC
    win = dloc // WIN
    half = (dloc % WIN) // HALF
    stream = (src_a >= SPLIT).astype(np.int64)

    per_core = []
    secs = np.zeros((NCORES, NWIN, 2, 2), np.int64)
    for c in range(NCORES):
        m = owner == c
        s_c, d_c, e_c = src_a[m], dloc[m], ea_a[m]
        w_c, h_c, st_c = win[m], half[m], stream[m]
        sec = ((w_c * 2 + h_c) * 2 + st_c)
        order = np.argsort(sec * NLOC + d_c, kind='stable')
        s_c, d_c, e_c, sec = s_c[order], d_c[order], e_c[order], sec[order]
        st_c = st_c[order]
        per_core.append((s_c, d_c, e_c, sec, st_c))
        secs[c] = np.bincount(sec, minlength=NWIN * 4).reshape(NWIN, 2, 2)

    K = np.maximum((secs.max(0) + 127) // 128, 1)       # [NWIN, 2, 2]
    assert K.max() <= KMAX
    Kf = K.reshape(-1)
    sec_slot = np.zeros(NWIN * 4 + 1, np.int64)
    np.cumsum(Kf * 128, out=sec_slot[1:])
    NSLOT = int(sec_slot[-1])

    gidx = np.zeros((NCORES, NSLOT), np.int16)
    eav = np.zeros((NCORES, 9, NSLOT), np.float32)
    # invalid slots: dstcol=147 misses the 96-wide one-hot AND (after -48h)
    # the 48-wide es window -> no contribution
    eav[:, 8, :] = 147.0

    for c in range(NCORES):
        s_c, d_c, e_c, sec, st_c = per_core[c]
        counts = np.bincount(sec, minlength=NWIN * 4)
        starts = np.concatenate([[0], np.cumsum(counts)])[:-1]
        pos = np.arange(len(s_c)) - starts[sec]
        slot = sec_slot[sec] + pos
        gidx[c, slot] = (s_c - st_c * SPLIT).astype(np.int16)
        eav[c, 0:8, slot] = e_c  # advanced idx puts slot axis first: (nedge, 8)
        eav[c, 8, slot] = (d_c % WIN).astype(np.float32)

    gw = np.ascontiguousarray(
        gidx.reshape(NCORES, -1, 16).transpose(0, 2, 1))     # [NCORES, 16, NSLOT//16]

    ea_q = np.clip(np.rint(eav[:, 0:8, :] * EASCALE), -127, 127).astype(np.int8)
    return dict(K=K, Kf=Kf, sec_slot=sec_slot, NSLOT=NSLOT, NSEC=NWIN * 4,
                gw=gw, ea8=ea_q,
                dcw=eav[:, 8, :].astype(np.uint8).reshape(NCORES, 1, NSLOT))


def _wblob(w):
    bh = np.zeros((96, WBH), BF16)
    bf = np.zeros((96, WBF), np.float32)
    b128 = np.zeros((128, WB128), np.float32)
    blobs = {'h': bh, 'f': bf, '128': b128}
    def put(name, arr):
        a = np.asarray(arr, np.float32)
        if a.ndim == 1:
            a = a.reshape(-1, 1)
        bl, r0, c0 = WOFF[name]
        dst = blobs[bl]
        dst[r0:r0 + a.shape[0], c0:c0 + a.shape[1]] = a.astype(dst.dtype)
    for l in range(L):
        V = np.zeros((8, 2), np.float32)
        # x ships as int8 * XSCALE; fold 1/XSCALE into the layer-0 weights
        xs = XSCALE if l == 0 else 1.0
        for b, p in enumerate(['local', 'global']):
            put(f'Wl_{l}_{b}', np.asarray(w[f'{p}_Wl'][l], np.float32) / xs)
            put(f'Wr_{l}_{b}', np.asarray(w[f'{p}_Wr'][l], np.float32) / xs)
            put(f'att_{l}_{b}', w[f'{p}_att'][l])
            put(f'bb_{l}_{b}', w[f'{p}_b'][l])
            # ea ships as int8 * EASCALE; fold 1/EASCALE into We and V
            put(f'We_{l}_{b}', np.asarray(w[f'{p}_We'][l], np.float32) / EASCALE)
            V[:, b] = (0.6 / EASCALE) * (np.asarray(w[f'{p}_We'][l], np.float32)
                                         @ np.asarray(w[f'{p}_att'][l], np.float32))
        put(f'V_{l}', V)
    put('fusion_Wt', w['fusion_W'][:96])
    put('fusion_Wb', w['fusion_W'][96:])
    put('fusion_b', w['fusion_b'])
    put('pred_W1a', w['pred_W1'][:, :128])
    put('pred_W1b', w['pred_W1'][:, 128:])
    put('pred_b1a', w['pred_b1'][:128])
    put('pred_b1b', w['pred_b1'][128:])
    put('pred_W2a', w['pred_W2'][:128])
    put('pred_W2b', w['pred_W2'][128:])
    put('pred_b2', np.broadcast_to(np.asarray(w['pred_b2']).reshape(1, 2), (128, 2)))
    put('iotaf', np.broadcast_to(np.arange(48, dtype=np.float32), (128, 48)))
    put('iota128', np.arange(128, dtype=np.float32))
    return bh, bf, b128


def build_kernel(pp):
    import os as _os
    SKIP_EDGE = _os.environ.get('SKIP_EDGE', '0') == '1'
    SKIP_GATHER = _os.environ.get('SKIP_GATHER', '0') == '1'
    from concourse import mybir, bacc
    import concourse.tile as tile
    Kf, sec_slot, NSLOT = pp['Kf'], pp['sec_slot'], pp['NSLOT']
    f32, bf16, i16 = mybir.dt.float32, mybir.dt.bfloat16, mybir.dt.int16
    AF = mybir.ActivationFunctionType
    OP = mybir.AluOpType

    i8, u8 = mybir.dt.int8, mybir.dt.uint8
    nc = bacc.Bacc("TRN2", target_bir_lowering=False, debug=False, num_devices=NCORES)
    dx = nc.dram_tensor("x", [NLOC, D], i8, kind="ExternalInput")
    dea8 = nc.dram_tensor("ea8", [8, NSLOT], i8, kind="ExternalInput")
    ddcw = nc.dram_tensor("dcw", [1, NSLOT], u8, kind="ExternalInput")
    dgw = nc.dram_tensor("gw", [16, NSLOT // 16], i16, kind="ExternalInput")
    # weight blobs ship 1/8 per core (identical content host-side) and are
    # reassembled on device with an AllGather -- saves 7/8 of the blob bytes
    # on the transfer-bound axon tunnel
    dwbh = nc.dram_tensor("wbh", [96 // NCORES, WBH], bf16, kind="ExternalInput")
    wbh_loc = nc.dram_tensor("wbh_loc", [96 // NCORES, WBH], bf16)
    wbh_sh = nc.dram_tensor("wbh_sh", [96, WBH], bf16, addr_space="Shared")
    dwbf = nc.dram_tensor("wbf", [96, WBF], f32, kind="ExternalInput")
    dwb128 = nc.dram_tensor("wb128", [128 // NCORES, WB128], f32, kind="ExternalInput")
    wb128_loc = nc.dram_tensor("wb128_loc", [128 // NCORES, WB128], f32)
    wb128_sh = nc.dram_tensor("wb128_sh", [128, WB128], f32, addr_space="Shared")
    dout = nc.dram_tensor("out", [NLOC, OUT], bf16, kind="ExternalOutput")

    tab_slice = nc.dram_tensor("tab_slice", [NLOC, TROW], bf16)
    tab_sh = nc.dram_tensor("tab_sh", [N, TROW], bf16, addr_space="Shared")
    tab = nc.dram_tensor("tab", [N, TROW], bf16)

    def wo(name, rows=96, width=None):
        w_ = width if width is not None else 96
        return (WOFF[name], WOFF[name] + w_, rows)

    with tile.TileContext(nc) as tc:
      with (tc.tile_pool(name="const", bufs=1) as cp,
            tc.tile_pool(name="hp", bufs=1) as hp,
            tc.tile_pool(name="wp", bufs=1) as wp,
            tc.tile_pool(name="sp", bufs=3) as sp,
            tc.tile_pool(name="gpool", bufs=2) as gpl,
            tc.tile_pool(name="ps", bufs=2, space="PSUM") as psp,
            tc.tile_pool(name="psA", bufs=2, space="PSUM") as psA,
            tc.tile_pool(name="psagg", bufs=1, space="PSUM") as psG):

        ident = cp.tile([128, 128], bf16)
        nc.sync.dma_start(out=ident[:], in_=nc.inline_tensor(np.eye(128, dtype=BF16), name="idb").ap())
        identf = cp.tile([128, 128], f32)
        nc.sync.dma_start(out=identf[:], in_=nc.inline_tensor(np.eye(128, dtype=np.float32), name="idf").ap())
        nc.sync.dma_start(out=wbh_loc[:], in_=dwbh[:])
        nc.sync.dma_start(out=wb128_loc[:], in_=dwb128[:])
        nc.gpsimd.collective_compute(
            "AllGather", mybir.AluOpType.bypass,
            replica_groups=[list(range(NCORES))],
            ins=[wbh_loc[:]], outs=[wbh_sh[:]])
        nc.gpsimd.collective_compute(
            "AllGather", mybir.AluOpType.bypass,
            replica_groups=[list(range(NCORES))],
            ins=[wb128_loc[:]], outs=[wb128_sh[:]])
        wbh_t = cp.tile([96, WBH], bf16)
        nc.sync.dma_start(out=wbh_t[:], in_=wbh_sh[:])
        wbf_t = cp.tile([96, WBF], f32)
        nc.sync.dma_start(out=wbf_t[:], in_=dwbf[:])
        wb128_t = cp.tile([128, WB128], f32)
        nc.sync.dma_start(out=wb128_t[:], in_=wb128_sh[:])
        gw_t = cp.tile([128, NSLOT // 16], i16)
        for k in range(8):
            nc.sync.dma_start(out=gw_t[16 * k:16 * (k + 1), :], in_=dgw[:])

        blobs_t = {'h': wbh_t, 'f': wbf_t, '128': wb128_t}
        def W(name, rows=96, width=96):
            bl, r0, c0 = WOFF[name]
            return blobs_t[bl][r0:r0 + rows, c0:c0 + width]

        iota_t = wb128_t[:, WOFF['iotaf'][2]:WOFF['iotaf'][2] + 48]

        one1 = cp.tile([1, 96], f32)
        nc.vector.memset(one1[:], 1.0)
        ones1 = cp.tile([1, 128], bf16)
        nc.vector.memset(ones1[:], 1.0)
        att04 = {}
        for l in range(L):
            for b in range(2):
                att04[(l, b)] = cp.tile([96, 1], bf16, tag=f"att04_{l}_{b}", name=f"att04_{l}_{b}")
                nc.vector.tensor_scalar(out=att04[(l, b)][:], in0=W(f'att_{l}_{b}', 96, 1),
                                        scalar1=0.4, scalar2=None, op0=OP.mult)
        Vt = {}
        for l in range(L):
            Vt[l] = cp.tile([8, 2], bf16, tag=f"V_{l}", name=f"V_{l}")
            nc.vector.tensor_copy(out=Vt[l][:], in_=W(f'V_{l}', 8, 2))

        # h_T feature-major [96, NPAD] (cols beyond NLOC are pad)
        h_T = [hp.tile([96, NCH * 128], bf16, tag=f"h{b}", name=f"h{b}") for b in range(2)]
        for ch in range(NCH):
            n0 = ch * 128
            nreal = max(0, min(NLOC - n0, 128))
            xin8 = sp.tile([128, 128], i8, tag="xin8")
            nc.vector.memset(xin8[:], 0)
            if nreal > 0:
                nc.sync.dma_start(out=xin8[:nreal, :96], in_=dx[n0:n0 + nreal, :])
            xin = sp.tile([128, 128], bf16, tag="xin")
            nc.vector.tensor_copy(out=xin[:], in_=xin8[:])
            pt = psA.tile([128, 128], bf16, tag="pbig")
            nc.tensor.transpose(out=pt[:], in_=xin[:], identity=ident[:])
            for b in range(2):
                nc.vector.tensor_copy(out=h_T[b][:, n0:n0 + 128], in_=pt[:96, :])

        hw_T = [wp.tile([96, NCH * 128], bf16, tag=f"hw{b}", name=f"hw{b}") for b in range(2)]

        for l in range(L):
            # ---------- PASS A ----------
            for b in range(2):
                for cs in range(0, NCH * 128, 512):
                    ce = min(cs + 512, NCH * 128)
                    w_ = ce - cs
                    pl = psA.tile([96, 512], f32, tag="pbig")
                    nc.tensor.matmul(out=pl[:, :w_], lhsT=W(f'Wl_{l}_{b}'),
                                     rhs=h_T[b][:, cs:ce], start=True, stop=True)
                    nc.vector.tensor_copy(out=hw_T[b][:, cs:ce], in_=pl[:, :w_])
            # table slice + allgather
            for ch in range(NCH):
                n0 = ch * 128
                nreal = max(0, min(NLOC - n0, 128))
                if nreal == 0:
                    continue
                stg = sp.tile([128, TROW], bf16, tag="stg")
                nc.vector.memset(stg[:], 0.0)
                for b in range(2):
                    pt = psA.tile([128, 128], bf16, tag="pbig")
                    nc.tensor.transpose(out=pt[:, :96], in_=hw_T[b][:, n0:n0 + 128],
                                        identity=ident[:96, :96])
                    nc.vector.tensor_copy(out=stg[:, b * 128:b * 128 + 96], in_=pt[:, :96])
                    # w = exp(0.6*att.hl) for this chunk; ones at ext row 32
                    pphi = psA.tile([1, 128], f32, tag="pbig")
                    nc.tensor.matmul(out=pphi[:], lhsT=W(f'att_{l}_{b}', 96, 1),
                                     rhs=hw_T[b][:, n0:n0 + 128], start=True, stop=True)
                    ext = sp.tile([64, 128], f32, tag="ext")
                    nc.scalar.activation(out=ext[0:1, :], in_=pphi[:], func=AF.Exp, scale=0.6)
                    nc.vector.memset(ext[32:33, :], 1.0)
                    pt2 = psA.tile([128, 64], f32, tag="pbig")
                    nc.tensor.transpose(out=pt2[:], in_=ext[:], identity=identf[:64, :64])
                    nc.vector.tensor_copy(out=stg[:, b * 128 + 96:b * 128 + 97], in_=pt2[:, 32:33])
                    nc.vector.tensor_copy(out=stg[:, b * 128 + 97:b * 128 + 98], in_=pt2[:, 0:1])
                nc.vector.tensor_copy(out=stg[:, 98:99], in_=stg[:, 225:226])
                nc.sync.dma_start(out=tab_slice[n0:n0 + nreal, :], in_=stg[:nreal, :])
            nc.gpsimd.collective_compute(
                "AllGather", mybir.AluOpType.bypass,
                replica_groups=[list(range(NCORES))],
                ins=[tab_slice[:]], outs=[tab_sh[:]],
            )
            nc.sync.dma_start(out=tab[:], in_=tab_sh[:])

            # ---------- edge phase ----------
            for w in range(0 if not SKIP_EDGE else NWIN, NWIN):
                aggp = {}
                first = {b: True for b in range(2)}
                nagg = {b: 0 for b in range(2)}
                tot = {b: sum(int(Kf[(w * 2 + h) * 2 + s]) for h in range(2) for s in range(2))
                       for b in range(2)}
                for b in range(2):
                    aggp[b] = psG.tile([97, WIN], f32, tag=f"agg{b}", name=f"agg{b}")
                # base lhsT per branch for this window (hr = h @ Wr computed here)
                basel = {}
                for b in range(2):
                    phr = psA.tile([96, WIN], f32, tag="pbig")
                    nc.tensor.matmul(out=phr[:], lhsT=W(f'Wr_{l}_{b}'),
                                     rhs=h_T[b][:, w * WIN:(w + 1) * WIN],
                                     start=True, stop=True)
                    hrs = sp.tile([96, WIN], f32, tag="hrs")
                    nc.vector.tensor_copy(out=hrs[:], in_=phr[:])
                    pt = psA.tile([WIN, 96], f32, tag="pbig")
                    nc.tensor.transpose(out=pt[:], in_=hrs[:], identity=identf[:96, :96])
                    bl = sp.tile([128, 96], bf16, tag=f"basel{b}", name=f"basel{b}")
                    nc.vector.memset(bl[:], 0.0)
                    nc.vector.tensor_copy(out=bl[:WIN, :], in_=pt[:])
                    nc.vector.tensor_copy(out=bl[WIN:WIN + 8, :], in_=W(f'We_{l}_{b}', 8, 96))
                    basel[b] = bl
                for h in range(2):
                    for s in range(2):
                        si = (w * 2 + h) * 2 + s
                        Ks = int(Kf[si])
                        sl0 = int(sec_slot[si])
                        nsl = Ks * 128
                        g = gpl.tile([128, KMAX, TROW], bf16, tag="gath")
                        if SKIP_GATHER:
                            nc.vector.memset(g[:, :Ks, :], 0.0)
                        else:
                            nc.gpsimd.dma_gather(
                                out_ap=g[:, :Ks, :],
                                in_ap=tab[SPLIT:, :] if s else tab[:SPLIT, :],
                                idxs_ap=gw_t[:, sl0 // 16:(sl0 + nsl) // 16],
                                num_idxs=nsl, num_idxs_reg=nsl, elem_size=TROW)
                        # compact edge table slice (i8/u8 on the wire -> bf16)
                        ea8s = sp.tile([8, KMAX * 128], i8, tag="ea8s")
                        nc.sync.dma_start(out=ea8s[:, :nsl], in_=dea8[:, sl0:sl0 + nsl])
                        eavs = sp.tile([8, KMAX * 128], bf16, tag="eavs")
                        nc.vector.tensor_copy(out=eavs[:, :nsl], in_=ea8s[:, :nsl])
                        dc8s = sp.tile([1, KMAX * 128], u8, tag="dc8s")
                        nc.sync.dma_start(out=dc8s[:, :nsl], in_=ddcw[:, sl0:sl0 + nsl])
                        dcw = sp.tile([1, KMAX * 128], bf16, tag="dcw")
                        nc.vector.tensor_copy(out=dcw[:, :nsl], in_=dc8s[:, :nsl])
                        # build Rt on device: rows 0-95 one-hot(dstcol), 96-103
                        # ea. is_equal covers ALL 128 rows (rows 96-127 can
                        # never match dstcol<=147... rows 96-103 overwritten by
                        # ea below, 104-127 exact zeros) so no partition of Rt
                        # is left uninitialized -- the mps matmul reads all 128
                        # partitions and 0 * NaN-garbage = NaN.
                        Rt = sp.tile([128, KMAX * 128], bf16, tag="Rt")
                        for c0 in range(0, nsl, 512):
                            cw = min(512, nsl - c0)
                            pbc = psA.tile([128, 512], f32, tag="pbig")
                            nc.tensor.matmul(out=pbc[:, :cw], lhsT=ones1[:],
                                             rhs=dcw[0:1, c0:c0 + cw], start=True, stop=True)
                            nc.vector.tensor_scalar(out=Rt[:, c0:c0 + cw],
                                                    in0=pbc[:, :cw],
                                                    scalar1=W('iota128', 128, 1),
                                                    scalar2=None, op0=OP.is_equal)
                            nc.vector.tensor_copy(out=Rt[96:104, c0:c0 + cw],
                                                  in_=eavs[0:8, c0:c0 + cw])
                        # per-slot dst col within half-window: transpose dcw
                        # blocks to partitions, -48h (invalid 147 -> 147/99)
                        dcsec = sp.tile([128, KMAX], f32, tag="dcs")
                        for j in range(Ks):
                            pt1 = psA.tile([128, 1], bf16, tag="pbig")
                            nc.tensor.transpose(out=pt1[:],
                                                in_=dcw[0:1, j * 128:(j + 1) * 128],
                                                identity=ident[:1, :1])
                            nc.vector.tensor_scalar(out=dcsec[:, j:j + 1], in0=pt1[:],
                                                    scalar1=float(-48 * h), scalar2=None,
                                                    op0=OP.add)
                        lgp = psp.tile([128, 16], f32, tag="lgp", bufs=1)
                        for j0 in range(0, Ks, 4):
                            jw = min(4, Ks - j0)
                            for b in range(2):
                                mps = psp.tile([96, 512], f32, tag="mps")
                                nc.tensor.matmul(out=mps[:, :jw * 128], lhsT=basel[b][:],
                                                 rhs=Rt[:, j0 * 128:(j0 + jw) * 128],
                                                 start=True, stop=False)
                                for dj in range(jw):
                                    j = j0 + dj
                                    nc.tensor.matmul(out=mps[:, dj * 128:(dj + 1) * 128],
                                                     lhsT=g[:, j, b * 128:b * 128 + 96],
                                                     rhs=ident[:], start=False,
                                                     stop=(dj == jw - 1),
                                                     skip_group_check=True)
                                am = sp.tile([96, 512], bf16, tag="am")
                                nc.scalar.activation(out=am[:, :jw * 128],
                                                     in_=mps[:, :jw * 128], func=AF.Abs)
                                for dj in range(jw):
                                    j = j0 + dj
                                    nc.tensor.matmul(out=lgp[:, 2 * j + b:2 * j + b + 1],
                                                     lhsT=am[:, dj * 128:(dj + 1) * 128],
                                                     rhs=att04[(l, b)][:],
                                                     start=(j == 0 and b == 0), stop=False,
                                                     skip_group_check=True)
                        # += 0.6*ea.(We@att) per branch (cols 2j|2j+1), on device
                        for j in range(Ks):
                            nc.tensor.matmul(out=lgp[:, 2 * j:2 * j + 2],
                                             lhsT=eavs[0:8, j * 128:(j + 1) * 128],
                                             rhs=Vt[l][:], start=False,
                                             stop=(j == Ks - 1), skip_group_check=True)
                        exw = sp.tile([128, 16], f32, tag="exw")
                        nc.scalar.activation(out=exw[:, :2 * Ks], in_=lgp[:, :2 * Ks],
                                             func=AF.Exp)
                        nc.vector.tensor_tensor(
                            out=exw[:, :2 * Ks].rearrange("p (j b) -> p j b", b=2),
                            in0=exw[:, :2 * Ks].rearrange("p (j b) -> p j b", b=2),
                            in1=g[:, :Ks, 97:99], op=OP.mult)
                        for j in range(Ks):
                            for b in range(2):
                                es = sp.tile([128, HALF], bf16, tag="es")
                                nc.vector.tensor_scalar(
                                    out=es[:], in0=iota_t, scalar1=dcsec[:, j:j + 1],
                                    scalar2=exw[:, 2 * j + b:2 * j + b + 1],
                                    op0=OP.is_equal, op1=OP.mult)
                                nagg[b] += 1
                                nc.tensor.matmul(out=aggp[b][:, h * HALF:(h + 1) * HALF],
                                                 lhsT=g[:, j, b * 128:b * 128 + 97],
                                                 rhs=es[:],
                                                 start=first[b], stop=(nagg[b] == tot[b]),
                                                 skip_group_check=True)
                                first[b] = False
                # finalize window -> h_T
                for b in range(2):
                    num = sp.tile([96, WIN], f32, tag="num")
                    den = sp.tile([1, WIN], f32, tag="den")
                    nc.vector.tensor_copy(out=num[:], in_=aggp[b][:96, :])
                    nc.vector.tensor_scalar(out=den[:], in0=aggp[b][96:97, :],
                                            scalar1=1e-30, scalar2=None, op0=OP.add)
                    rec = sp.tile([1, WIN], f32, tag="rec")
                    nc.vector.reciprocal(out=rec[:], in_=den[:])
                    pb = psp.tile([96, WIN], f32, tag="mps")
                    nc.tensor.matmul(out=pb[:], lhsT=one1[:], rhs=rec[:], start=True, stop=True)
                    tdiv = sp.tile([96, WIN], f32, tag="tdiv")
                    nc.vector.tensor_tensor(out=tdiv[:], in0=num[:], in1=pb[:], op=OP.mult)
                    lin = sp.tile([96, WIN], f32, tag="lin")
                    nc.scalar.activation(out=lin[:], in_=tdiv[:], func=AF.Identity,
                                         bias=W(f'bb_{l}_{b}', 96, 1))
                    ab = sp.tile([96, WIN], f32, tag="ab")
                    nc.scalar.activation(out=ab[:], in_=tdiv[:], func=AF.Abs,
                                         bias=W(f'bb_{l}_{b}', 96, 1))
                    nc.vector.tensor_scalar(out=lin[:], in0=lin[:], scalar1=0.505,
                                            scalar2=None, op0=OP.mult)
                    nc.vector.tensor_scalar(out=ab[:], in0=ab[:], scalar1=0.495,
                                            scalar2=None, op0=OP.mult)
                    nc.vector.tensor_tensor(out=h_T[b][:, w * WIN:(w + 1) * WIN],
                                            in0=lin[:], in1=ab[:], op=OP.add)

        # ---------- head ----------
        hid_T = [wp.tile([128, NCH * 128], f32, tag=f"hw{p}", name=f"hid{p}") for p in range(2)]
        for cs in range(0, NCH * 128, 512):
            ce = min(cs + 512, NCH * 128)
            w_ = ce - cs
            pf = psA.tile([96, 512], f32, tag="pbig")
            nc.tensor.matmul(out=pf[:, :w_], lhsT=W('fusion_Wt'),
                             rhs=h_T[0][:, cs:ce], start=True, stop=False)
            nc.tensor.matmul(out=pf[:, :w_], lhsT=W('fusion_Wb'),
                             rhs=h_T[1][:, cs:ce], start=False, stop=True)
            fus = sp.tile([96, 512], bf16, tag="fus")
            lin = sp.tile([96, 512], f32, tag="flin")
            nc.scalar.activation(out=lin[:, :w_], in_=pf[:, :w_], func=AF.Identity,
                                 bias=W('fusion_b', 96, 1))
            ab = sp.tile([96, 512], f32, tag="fab")
            nc.scalar.activation(out=ab[:, :w_], in_=pf[:, :w_], func=AF.Abs,
                                 bias=W('fusion_b', 96, 1))
            nc.vector.tensor_scalar(out=lin[:, :w_], in0=lin[:, :w_], scalar1=0.505,
                                    scalar2=None, op0=OP.mult)
            nc.vector.tensor_scalar(out=ab[:, :w_], in0=ab[:, :w_], scalar1=0.495,
                                    scalar2=None, op0=OP.mult)
            nc.vector.tensor_tensor(out=fus[:, :w_], in0=lin[:, :w_], in1=ab[:, :w_],
                                    op=OP.add)
            for p, (wk, bk) in enumerate([('pred_W1a', 'pred_b1a'), ('pred_W1b', 'pred_b1b')]):
                ph = psA.tile([128, 512], f32, tag="pbig")
                nc.tensor.matmul(out=ph[:, :w_], lhsT=W(wk, 96, 128), rhs=fus[:, :w_],
                                 start=True, stop=True)
                l2 = sp.tile([128, 512], f32, tag=f"l2{p}")
                a2 = sp.tile([128, 512], f32, tag=f"a2{p}")
                nc.scalar.activation(out=l2[:, :w_], in_=ph[:, :w_], func=AF.Identity,
                                     bias=W(bk, 128, 1))
                nc.scalar.activation(out=a2[:, :w_], in_=ph[:, :w_], func=AF.Abs,
                                     bias=W(bk, 128, 1))
                nc.vector.tensor_scalar(out=l2[:, :w_], in0=l2[:, :w_], scalar1=0.505,
                                        scalar2=None, op0=OP.mult)
                nc.vector.tensor_scalar(out=a2[:, :w_], in0=a2[:, :w_], scalar1=0.495,
                                        scalar2=None, op0=OP.mult)
                nc.vector.tensor_tensor(out=hid_T[p][:, cs:ce], in0=l2[:, :w_],
                                        in1=a2[:, :w_], op=OP.add)
        for ch in range(NCH):
            n0 = ch * 128
            nreal = max(0, min(NLOC - n0, 128))
            if nreal == 0:
                continue
            po = psp.tile([128, 2], f32, tag="mps")
            nc.tensor.matmul(out=po[:], lhsT=hid_T[0][:, n0:n0 + 128],
                             rhs=W('pred_W2a', 128, 2), start=True, stop=False)
            nc.tensor.matmul(out=po[:], lhsT=hid_T[1][:, n0:n0 + 128],
                             rhs=W('pred_W2b', 128, 2), start=False, stop=True)
            ot = sp.tile([128, 2], bf16, tag="ot")
            nc.vector.tensor_tensor(out=ot[:], in0=po[:], in1=W('pred_b2', 128, 2), op=OP.add)
            nc.sync.dma_start(out=dout[n0:n0 + nreal, :], in_=ot[:nreal, :])

    nc.compile()
    return nc


def _make_runner(nc):
    """Build (once) a cached jitted shard_map wrapper around the compiled
    Bass module — same lowering as bass2jax.run_bass_via_pjrt, but the jit
    object is reused across calls so warm calls skip retrace/recompile."""
    import jax
    import jax.core as jcore
    from jax.experimental.shard_map import shard_map
    from jax.sharding import Mesh, PartitionSpec
    from concourse import bass2jax, mybir
    bass2jax.install_neuronx_cc_hook()

    partition_name = nc.partition_id_tensor.name if nc.partition_id_tensor else None
    in_names, out_names, out_avals, zero_shapes = [], [], [], []
    for alloc in nc.m.functions[0].allocations:
        if not isinstance(alloc, mybir.MemoryLocationSet):
            continue
        name = alloc.memorylocations[0].name
        if alloc.kind == "ExternalInput":
            if name != partition_name:
                in_names.append(name)
        elif alloc.kind == "ExternalOutput":
            shape = tuple(alloc.tensor_shape)
            dtype = mybir.dt.np(alloc.dtype)
            out_names.append(name)
            out_avals.append(jcore.ShapedArray(shape, dtype))
            zero_shapes.append((shape, dtype))
    n_params = len(in_names)
    n_outs = len(out_avals)
    all_in = list(in_names) + list(out_names)
    if partition_name is not None:
        all_in.append(partition_name)
    donate = tuple(range(n_params, n_params + n_outs))

    dbg_name = None
    if nc.dbg_addr is not None:
        assert not nc.dbg_callbacks
        dbg_name = nc.dbg_addr.name

    def _body(*args):
        operands = list(args)
        if partition_name is not None:
            operands.append(bass2jax.partition_id_tensor())
        outs = bass2jax._bass_exec_p.bind(
            *operands, out_avals=tuple(out_avals), in_names=tuple(all_in),
            out_names=tuple(out_names), lowering_input_output_aliases=(),
            sim_require_finite=True, sim_require_nnan=True, nc=nc)
        return tuple(outs)

    devices = jax.devices()[:NCORES]
    mesh = Mesh(np.asarray(devices), ("core",))
    in_specs = (PartitionSpec("core"),) * (n_params + n_outs)
    out_specs = (PartitionSpec("core"),) * n_outs
    fn = jax.jit(shard_map(_body, mesh=mesh, in_specs=in_specs,
                           out_specs=out_specs, check_rep=False),
                 donate_argnums=donate, keep_unused=True)
    return dict(fn=fn, in_names=in_names, out_names=out_names,
                out_avals=out_avals, zero_shapes=zero_shapes, dbg_name=dbg_name)


def _execute(runner, in_maps):
    n = len(in_maps)
    if runner['dbg_name'] is not None:
        z = np.zeros((1, 2), np.uint32)
        in_maps = [{**m, runner['dbg_name']: z} for m in in_maps]
    concat_in = [np.concatenate([np.asarray(in_maps[c][nm]) for c in range(n)], axis=0)
                 for nm in runner['in_names']]
    zeros = [np.zeros((n * s[0], *s[1:]), dt) for (s, dt) in runner['zero_shapes']]
    outs = runner['fn'](*concat_in, *zeros)
    return [{nm: np.asarray(outs[i]).reshape(n, *runner['out_avals'][i].shape)[c]
             for i, nm in enumerate(runner['out_names'])} for c in range(n)]


def _in_maps(x, pp, blob):
    bh, bf, b128 = blob
    xb = np.ascontiguousarray(
        np.clip(np.rint(x * XSCALE), -127, 127).astype(np.int8))
    maps = []
    for c in range(NCORES):
        maps.append({'x': xb[c * NLOC:(c + 1) * NLOC],
                     'ea8': pp['ea8'][c],
                     'dcw': pp['dcw'][c],
                     'gw': pp['gw'][c],
                     'wbh': np.ascontiguousarray(bh[c * 12:(c + 1) * 12]),
                     'wbf': bf,
                     'wb128': np.ascontiguousarray(b128[c * 16:(c + 1) * 16])})
    return maps


def kernel(**inputs):
    x = np.asarray(inputs['x'], np.float32)
    ei = np.asarray(inputs['edge_index'])
    ea = np.asarray(inputs['edge_attr'], np.float32)
    pp = _host_prep(x, ei, ea)
    if _CACHE.get('NSLOT') != pp['NSLOT']:
        _CACHE['nc'] = build_kernel(pp)
        _CACHE['runner'] = _make_runner(_CACHE['nc'])
        _CACHE['NSLOT'] = pp['NSLOT']
    blob = _wblob(inputs)
    res = _execute(_CACHE['runner'], _in_maps(x, pp, blob))
    out = np.concatenate([res[c]['out'] for c in range(NCORES)], axis=0)
    return out.astype(np.float32)


# revision 67
# speedup vs baseline: 28.4915x; 1.0471x over previous
"""BiLevelGAT (2-branch x 3-layer GATv2, N=50000, E=500000, D=96) on 8 TRN2 cores.

Sharding: nodes + incoming edges partitioned by dst; per-layer AllGather of a
bf16 per-node table [hl_loc 96|1|w_loc|w_glob|pad29|hl_glob 96|1|w_glob|pad30]
(512B rows) gathered per edge by src.

Math: lrelu(x) = 0.6x+0.4|x| splits the GATv2 logit into linear terms (per-src
w=exp(0.6*att.hl) folded into the softmax weight; per-dst term cancels in
softmax; per-edge ea term = on-device matmul of the edge table against
V=0.6*We@att) plus 0.4*att.|m| computed on device. Softmax max-subtraction
skipped (logits O(1), fp32 safe).

End-to-end wall time is dominated by host->device transfer over the axon
tunnel (~45MB/s), so inputs are aggressively compacted (~2MB/core):
 - x as int8*32 (1/32 folded into the layer-0 Wl/Wr), edge_attr as int8*32
   (1/32 folded into We and V); both add ~5e-3 rel err vs the 2e-2 gate.
 - per-edge one-hot "R" (dst selector + ea rows used as matmul rhs to form
   m = hr[dst] + ea@We) built on device per section: rank-1 broadcast matmul
   of the dst-col row, then is_equal against a partition iota covering ALL
   128 partitions (a partially-initialized Rt causes 0*NaN-garbage = NaN on
   a per-process-random basis -- caught by CoreSim's uninit checker).
 - per-slot dst cols for the scatter one-hot derived on device by per-block
   PE transposes of the dst-col row (invalid slots carry 147 -> miss both
   one-hots).
 - weights packed into 3 blobs (bf16 matrices / f32 96-row biases / f32
   128-row tail + iotas); gather indices shipped once [16, NSLOT/16] and
   replicated to 128 partitions on device.
The jitted shard_map wrapper is cached across calls (fresh jit per call
would retrace + recompile the XLA wrapper each time).
"""
import os
import sys
sys.path.insert(0, '/opt/trn_rl_repo')
os.environ.setdefault('NEURON_RT_RESET_CORES', '1')
import numpy as np
import ml_dtypes

BF16 = ml_dtypes.bfloat16

N, E, D, EDIM, L, DENSE, OUT = 50000, 500000, 96, 8, 3, 256, 2
NCORES = 8
NLOC = N // NCORES            # 6250
WIN, HALF = 96, 48
NWIN = (NLOC + WIN - 1) // WIN  # 66
NPAD = NWIN * WIN
NCH = (NPAD + 127) // 128     # chunks of 128 (PASS A / table)
SPLIT = 32768
TROW = 256
KMAX = 8

_CACHE = {}

# ---- packed weight blobs ----
# wbh [96, WBH] bf16: big matrices (Wl/Wr/att/We/fusion/pred_W1, V).
#   We_{l,b} stacked six-up in two 96-col groups at row bases 0/32/64
#   (any access must start at a partition base that is a multiple of 32);
#   V_l at row base 32*l (matmul rhs).
# wbf [96, WBF] f32: 96-row biases.
# wb128 [128, WB128] f32: 128-row biases/mats + iota constants.
WOFF = {}   # name -> (blob, row0, col0)
_cols = {'h': 0, 'f': 0, '128': 0}
def _al(blob, name, width, row=0):
    WOFF[name] = (blob, row, _cols[blob])
    _cols[blob] += width
for _l in range(L):
    for _b in range(2):
        _al('h', f'Wl_{_l}_{_b}', 96)
        _al('h', f'Wr_{_l}_{_b}', 96)
        _al('h', f'att_{_l}_{_b}', 1)
        _al('f', f'bb_{_l}_{_b}', 1)
for _i in range(6):
    _l, _b = divmod(_i, 2)
    WOFF[f'We_{_l}_{_b}'] = ('h', 32 * (_i % 3), _cols['h'] + 96 * (_i // 3))
_cols['h'] += 192
for _l in range(L):
    WOFF[f'V_{_l}'] = ('h', 32 * _l, _cols['h'])              # shared 2 cols
_cols['h'] += 2
_al('h', 'fusion_Wt', 96)
_al('h', 'fusion_Wb', 96)
_al('f', 'fusion_b', 1)
_al('h', 'pred_W1a', 128)
_al('h', 'pred_W1b', 128)
_al('128', 'pred_b1a', 1)
_al('128', 'pred_b1b', 1)
_al('128', 'pred_W2a', 2)
_al('128', 'pred_W2b', 2)
_al('128', 'pred_b2', 2)
_al('128', 'iotaf', 96)               # all 128 rows = arange(96)
_al('128', 'iota128', 1)              # rows = arange(128)
WBH, WBF, WB128 = _cols['h'], _cols['f'], _cols['128']
EASCALE = 32.0
XSCALE = 32.0


def _host_prep(x, edge_index, edge_attr):
    src = edge_index[0].astype(np.int64)
    dst = edge_index[1].astype(np.int64)
    mean_ea = edge_attr.mean(0).astype(np.float32)
    loop = np.arange(N, dtype=np.int64)
    src_a = np.concatenate([src, loop])
    dst_a = np.concatenate([dst, loop])
    ea_a = np.concatenate([edge_attr.astype(np.float32),
                           np.broadcast_to(mean_ea, (N, EDIM))], 0)

    owner = dst_a // NLOC
    dloc = dst_a - owner * NLOC
    win = dloc // WIN
    stream = (src_a >= SPLIT).astype(np.int64)

    per_core = []
    secs = np.zeros((NCORES, NWIN, 2), np.int64)
    for c in range(NCORES):
        m = owner == c
        s_c, d_c, e_c = src_a[m], dloc[m], ea_a[m]
        w_c, st_c = win[m], stream[m]
        sec = (w_c * 2 + st_c)
        order = np.argsort(sec * NLOC + d_c, kind='stable')
        s_c, d_c, e_c, sec = s_c[order], d_c[order], e_c[order], sec[order]
        st_c = st_c[order]
        per_core.append((s_c, d_c, e_c, sec, st_c))
        secs[c] = np.bincount(sec, minlength=NWIN * 2).reshape(NWIN, 2)

    K = np.maximum((secs.max(0) + 127) // 128, 1)       # [NWIN, 2]
    assert K.max() <= KMAX
    Kf = K.reshape(-1)
    sec_slot = np.zeros(NWIN * 2 + 1, np.int64)
    np.cumsum(Kf * 128, out=sec_slot[1:])
    NSLOT = int(sec_slot[-1])

    gidx = np.zeros((NCORES, NSLOT), np.int16)
    eav = np.zeros((NCORES, 9, NSLOT), np.float32)
    # invalid slots: dstcol=147 misses both the 128-wide Rt one-hot and the
    # 96-wide es window -> no contribution
    eav[:, 8, :] = 147.0

    for c in range(NCORES):
        s_c, d_c, e_c, sec, st_c = per_core[c]
        counts = np.bincount(sec, minlength=NWIN * 2)
        starts = np.concatenate([[0], np.cumsum(counts)])[:-1]
        pos = np.arange(len(s_c)) - starts[sec]
        slot = sec_slot[sec] + pos
        gidx[c, slot] = (s_c - st_c * SPLIT).astype(np.int16)
        eav[c, 0:8, slot] = e_c  # advanced idx puts slot axis first: (nedge, 8)
        eav[c, 8, slot] = (d_c % WIN).astype(np.float32)

    gw = np.ascontiguousarray(
        gidx.reshape(NCORES, -1, 16).transpose(0, 2, 1))     # [NCORES, 16, NSLOT//16]

    ea_q = np.clip(np.rint(eav[:, 0:8, :] * EASCALE), -127, 127).astype(np.int8)
    return dict(K=K, Kf=Kf, sec_slot=sec_slot, NSLOT=NSLOT, NSEC=NWIN * 2,
                gw=gw, ea8=ea_q,
                dcw=eav[:, 8, :].astype(np.uint8).reshape(NCORES, 1, NSLOT))


def _wblob(w):
    bh = np.zeros((96, WBH), BF16)
    bf = np.zeros((96, WBF), np.float32)
    b128 = np.zeros((128, WB128), np.float32)
    blobs = {'h': bh, 'f': bf, '128': b128}
    def put(name, arr):
        a = np.asarray(arr, np.float32)
        if a.ndim == 1:
            a = a.reshape(-1, 1)
        bl, r0, c0 = WOFF[name]
        dst = blobs[bl]
        dst[r0:r0 + a.shape[0], c0:c0 + a.shape[1]] = a.astype(dst.dtype)
    for l in range(L):
        V = np.zeros((8, 2), np.float32)
        # x ships as int8 * XSCALE; fold 1/XSCALE into the layer-0 weights
        xs = XSCALE if l == 0 else 1.0
        for b, p in enumerate(['local', 'global']):
            put(f'Wl_{l}_{b}', np.asarray(w[f'{p}_Wl'][l], np.float32) / xs)
            put(f'Wr_{l}_{b}', np.asarray(w[f'{p}_Wr'][l], np.float32) / xs)
            put(f'att_{l}_{b}', w[f'{p}_att'][l])
            put(f'bb_{l}_{b}', w[f'{p}_b'][l])
            # ea ships as int8 * EASCALE; fold 1/EASCALE into We and V
            put(f'We_{l}_{b}', np.asarray(w[f'{p}_We'][l], np.float32) / EASCALE)
            V[:, b] = (0.6 / EASCALE) * (np.asarray(w[f'{p}_We'][l], np.float32)
                                         @ np.asarray(w[f'{p}_att'][l], np.float32))
        put(f'V_{l}', V)
    put('fusion_Wt', w['fusion_W'][:96])
    put('fusion_Wb', w['fusion_W'][96:])
    put('fusion_b', w['fusion_b'])
    put('pred_W1a', w['pred_W1'][:, :128])
    put('pred_W1b', w['pred_W1'][:, 128:])
    put('pred_b1a', w['pred_b1'][:128])
    put('pred_b1b', w['pred_b1'][128:])
    put('pred_W2a', w['pred_W2'][:128])
    put('pred_W2b', w['pred_W2'][128:])
    put('pred_b2', np.broadcast_to(np.asarray(w['pred_b2']).reshape(1, 2), (128, 2)))
    put('iotaf', np.broadcast_to(np.arange(96, dtype=np.float32), (128, 96)))
    put('iota128', np.arange(128, dtype=np.float32))
    return bh, bf, b128


def build_kernel(pp):
    import os as _os
    SKIP_EDGE = _os.environ.get('SKIP_EDGE', '0') == '1'
    SKIP_GATHER = _os.environ.get('SKIP_GATHER', '0') == '1'
    from concourse import mybir, bacc
    import concourse.tile as tile
    Kf, sec_slot, NSLOT = pp['Kf'], pp['sec_slot'], pp['NSLOT']
    f32, bf16, i16 = mybir.dt.float32, mybir.dt.bfloat16, mybir.dt.int16
    AF = mybir.ActivationFunctionType
    OP = mybir.AluOpType

    i8, u8 = mybir.dt.int8, mybir.dt.uint8
    nc = bacc.Bacc("TRN2", target_bir_lowering=False, debug=False, num_devices=NCORES)
    dx = nc.dram_tensor("x", [NLOC, D], i8, kind="ExternalInput")
    dea8 = nc.dram_tensor("ea8", [8, NSLOT], i8, kind="ExternalInput")
    ddcw = nc.dram_tensor("dcw", [1, NSLOT], u8, kind="ExternalInput")
    dgw = nc.dram_tensor("gw", [16, NSLOT // 16], i16, kind="ExternalInput")
    # weight blobs ship 1/8 per core (identical content host-side) and are
    # reassembled on device with an AllGather -- saves 7/8 of the blob bytes
    # on the transfer-bound axon tunnel
    dwbh = nc.dram_tensor("wbh", [96 // NCORES, WBH], bf16, kind="ExternalInput")
    wbh_loc = nc.dram_tensor("wbh_loc", [96 // NCORES, WBH], bf16)
    wbh_sh = nc.dram_tensor("wbh_sh", [96, WBH], bf16, addr_space="Shared")
    dwbf = nc.dram_tensor("wbf", [96, WBF], f32, kind="ExternalInput")
    dwb128 = nc.dram_tensor("wb128", [128 // NCORES, WB128], f32, kind="ExternalInput")
    wb128_loc = nc.dram_tensor("wb128_loc", [128 // NCORES, WB128], f32)
    wb128_sh = nc.dram_tensor("wb128_sh", [128, WB128], f32, addr_space="Shared")
    dout = nc.dram_tensor("out", [NLOC, OUT], bf16, kind="ExternalOutput")

    tab_slice = nc.dram_tensor("tab_slice", [NLOC, TROW], bf16)
    tab_sh = nc.dram_tensor("tab_sh", [N, TROW], bf16, addr_space="Shared")
    tab = nc.dram_tensor("tab", [N, TROW], bf16)

    def wo(name, rows=96, width=None):
        w_ = width if width is not None else 96
        return (WOFF[name], WOFF[name] + w_, rows)

    with tile.TileContext(nc) as tc:
      with (tc.tile_pool(name="const", bufs=1) as cp,
            tc.tile_pool(name="hp", bufs=1) as hp,
            tc.tile_pool(name="wp", bufs=1) as wp,
            tc.tile_pool(name="sp", bufs=3) as sp,
            tc.tile_pool(name="gpool", bufs=2) as gpl,
            tc.tile_pool(name="ps", bufs=2, space="PSUM") as psp,
            tc.tile_pool(name="psA", bufs=2, space="PSUM") as psA,
            tc.tile_pool(name="psagg", bufs=1, space="PSUM") as psG):

        ident = cp.tile([128, 128], bf16)
        nc.sync.dma_start(out=ident[:], in_=nc.inline_tensor(np.eye(128, dtype=BF16), name="idb").ap())
        identf = cp.tile([128, 128], f32)
        nc.sync.dma_start(out=identf[:], in_=nc.inline_tensor(np.eye(128, dtype=np.float32), name="idf").ap())
        nc.sync.dma_start(out=wbh_loc[:], in_=dwbh[:])
        nc.sync.dma_start(out=wb128_loc[:], in_=dwb128[:])
        nc.gpsimd.collective_compute(
            "AllGather", mybir.AluOpType.bypass,
            replica_groups=[list(range(NCORES))],
            ins=[wbh_loc[:]], outs=[wbh_sh[:]])
        nc.gpsimd.collective_compute(
            "AllGather", mybir.AluOpType.bypass,
            replica_groups=[list(range(NCORES))],
            ins=[wb128_loc[:]], outs=[wb128_sh[:]])
        wbh_t = cp.tile([96, WBH], bf16)
        nc.sync.dma_start(out=wbh_t[:], in_=wbh_sh[:])
        wbf_t = cp.tile([96, WBF], f32)
        nc.sync.dma_start(out=wbf_t[:], in_=dwbf[:])
        wb128_t = cp.tile([128, WB128], f32)
        nc.sync.dma_start(out=wb128_t[:], in_=wb128_sh[:])
        gw_t = cp.tile([128, NSLOT // 16], i16)
        for k in range(8):
            nc.sync.dma_start(out=gw_t[16 * k:16 * (k + 1), :], in_=dgw[:])

        blobs_t = {'h': wbh_t, 'f': wbf_t, '128': wb128_t}
        def W(name, rows=96, width=96):
            bl, r0, c0 = WOFF[name]
            return blobs_t[bl][r0:r0 + rows, c0:c0 + width]

        iota_t = wb128_t[:, WOFF['iotaf'][2]:WOFF['iotaf'][2] + 96]

        one1 = cp.tile([1, 96], f32)
        nc.vector.memset(one1[:], 1.0)
        ones1 = cp.tile([1, 128], bf16)
        nc.vector.memset(ones1[:], 1.0)
        att04 = {}
        for l in range(L):
            for b in range(2):
                att04[(l, b)] = cp.tile([96, 1], bf16, tag=f"att04_{l}_{b}", name=f"att04_{l}_{b}")
                nc.vector.tensor_scalar(out=att04[(l, b)][:], in0=W(f'att_{l}_{b}', 96, 1),
                                        scalar1=0.4, scalar2=None, op0=OP.mult)
        Vt = {}
        for l in range(L):
            Vt[l] = cp.tile([8, 2], bf16, tag=f"V_{l}", name=f"V_{l}")
            nc.vector.tensor_copy(out=Vt[l][:], in_=W(f'V_{l}', 8, 2))

        # h_T feature-major [96, NPAD] (cols beyond NLOC are pad)
        h_T = [hp.tile([96, NCH * 128], bf16, tag=f"h{b}", name=f"h{b}") for b in range(2)]
        for ch in range(NCH):
            n0 = ch * 128
            nreal = max(0, min(NLOC - n0, 128))
            xin8 = sp.tile([128, 128], i8, tag="xin8")
            nc.vector.memset(xin8[:], 0)
            if nreal > 0:
                nc.sync.dma_start(out=xin8[:nreal, :96], in_=dx[n0:n0 + nreal, :])
            xin = sp.tile([128, 128], bf16, tag="xin")
            nc.vector.tensor_copy(out=xin[:], in_=xin8[:])
            pt = psA.tile([128, 128], bf16, tag="pbig")
            nc.tensor.transpose(out=pt[:], in_=xin[:], identity=ident[:])
            for b in range(2):
                nc.vector.tensor_copy(out=h_T[b][:, n0:n0 + 128], in_=pt[:96, :])

        hw_T = [wp.tile([96, NCH * 128], bf16, tag=f"hw{b}", name=f"hw{b}") for b in range(2)]

        for l in range(L):
            # ---------- PASS A ----------
            for b in range(2):
                for cs in range(0, NCH * 128, 512):
                    ce = min(cs + 512, NCH * 128)
                    w_ = ce - cs
                    pl = psA.tile([96, 512], f32, tag="pbig")
                    nc.tensor.matmul(out=pl[:, :w_], lhsT=W(f'Wl_{l}_{b}'),
                                     rhs=h_T[b][:, cs:ce], start=True, stop=True)
                    nc.vector.tensor_copy(out=hw_T[b][:, cs:ce], in_=pl[:, :w_])
            # table slice + allgather
            for ch in range(NCH):
                n0 = ch * 128
                nreal = max(0, min(NLOC - n0, 128))
                if nreal == 0:
                    continue
                stg = sp.tile([128, TROW], bf16, tag="stg")
                nc.vector.memset(stg[:], 0.0)
                for b in range(2):
                    pt = psA.tile([128, 128], bf16, tag="pbig")
                    nc.tensor.transpose(out=pt[:, :96], in_=hw_T[b][:, n0:n0 + 128],
                                        identity=ident[:96, :96])
                    nc.vector.tensor_copy(out=stg[:, b * 128:b * 128 + 96], in_=pt[:, :96])
                    # w = exp(0.6*att.hl) for this chunk; ones at ext row 32
                    pphi = psA.tile([1, 128], f32, tag="pbig")
                    nc.tensor.matmul(out=pphi[:], lhsT=W(f'att_{l}_{b}', 96, 1),
                                     rhs=hw_T[b][:, n0:n0 + 128], start=True, stop=True)
                    ext = sp.tile([64, 128], f32, tag="ext")
                    nc.scalar.activation(out=ext[0:1, :], in_=pphi[:], func=AF.Exp, scale=0.6)
                    nc.vector.memset(ext[32:33, :], 1.0)
                    pt2 = psA.tile([128, 64], f32, tag="pbig")
                    nc.tensor.transpose(out=pt2[:], in_=ext[:], identity=identf[:64, :64])
                    nc.vector.tensor_copy(out=stg[:, b * 128 + 96:b * 128 + 97], in_=pt2[:, 32:33])
                    nc.vector.tensor_copy(out=stg[:, b * 128 + 97:b * 128 + 98], in_=pt2[:, 0:1])
                nc.vector.tensor_copy(out=stg[:, 98:99], in_=stg[:, 225:226])
                nc.sync.dma_start(out=tab_slice[n0:n0 + nreal, :], in_=stg[:nreal, :])
            nc.gpsimd.collective_compute(
                "AllGather", mybir.AluOpType.bypass,
                replica_groups=[list(range(NCORES))],
                ins=[tab_slice[:]], outs=[tab_sh[:]],
            )
            nc.sync.dma_start(out=tab[:], in_=tab_sh[:])

            # ---------- edge phase ----------
            for w in range(0 if not SKIP_EDGE else NWIN, NWIN):
                aggp = {}
                first = {b: True for b in range(2)}
                nagg = {b: 0 for b in range(2)}
                tot = {b: sum(int(Kf[w * 2 + s]) for s in range(2))
                       for b in range(2)}
                for b in range(2):
                    aggp[b] = psG.tile([97, WIN], f32, tag=f"agg{b}", name=f"agg{b}")
                # base lhsT per branch for this window (hr = h @ Wr computed here)
                basel = {}
                for b in range(2):
                    phr = psA.tile([96, WIN], f32, tag="pbig")
                    nc.tensor.matmul(out=phr[:], lhsT=W(f'Wr_{l}_{b}'),
                                     rhs=h_T[b][:, w * WIN:(w + 1) * WIN],
                                     start=True, stop=True)
                    hrs = sp.tile([96, WIN], f32, tag="hrs")
                    nc.vector.tensor_copy(out=hrs[:], in_=phr[:])
                    pt = psA.tile([WIN, 96], f32, tag="pbig")
                    nc.tensor.transpose(out=pt[:], in_=hrs[:], identity=identf[:96, :96])
                    bl = sp.tile([128, 96], bf16, tag=f"basel{b}", name=f"basel{b}")
                    nc.vector.memset(bl[:], 0.0)
                    nc.vector.tensor_copy(out=bl[:WIN, :], in_=pt[:])
                    nc.vector.tensor_copy(out=bl[WIN:WIN + 8, :], in_=W(f'We_{l}_{b}', 8, 96))
                    basel[b] = bl
                if True:
                    for s in range(2):
                        si = w * 2 + s
                        Ks = int(Kf[si])
                        sl0 = int(sec_slot[si])
                        nsl = Ks * 128
                        g = gpl.tile([128, KMAX, TROW], bf16, tag="gath")
                        if SKIP_GATHER:
                            nc.vector.memset(g[:, :Ks, :], 0.0)
                        else:
                            nc.gpsimd.dma_gather(
                                out_ap=g[:, :Ks, :],
                                in_ap=tab[SPLIT:, :] if s else tab[:SPLIT, :],
                                idxs_ap=gw_t[:, sl0 // 16:(sl0 + nsl) // 16],
                                num_idxs=nsl, num_idxs_reg=nsl, elem_size=TROW)
                        # compact edge table slice (i8/u8 on the wire -> bf16)
                        ea8s = sp.tile([8, KMAX * 128], i8, tag="ea8s")
                        nc.sync.dma_start(out=ea8s[:, :nsl], in_=dea8[:, sl0:sl0 + nsl])
                        eavs = sp.tile([8, KMAX * 128], bf16, tag="eavs")
                        nc.vector.tensor_copy(out=eavs[:, :nsl], in_=ea8s[:, :nsl])
                        dc8s = sp.tile([1, KMAX * 128], u8, tag="dc8s")
                        nc.sync.dma_start(out=dc8s[:, :nsl], in_=ddcw[:, sl0:sl0 + nsl])
                        dcw = sp.tile([1, KMAX * 128], bf16, tag="dcw")
                        nc.vector.tensor_copy(out=dcw[:, :nsl], in_=dc8s[:, :nsl])
                        # build Rt on device: rows 0-95 one-hot(dstcol), 96-103
                        # ea. is_equal covers ALL 128 rows (rows 96-127 can
                        # never match dstcol<=147... rows 96-103 overwritten by
                        # ea below, 104-127 exact zeros) so no partition of Rt
                        # is left uninitialized -- the mps matmul reads all 128
                        # partitions and 0 * NaN-garbage = NaN.
                        Rt = sp.tile([128, KMAX * 128], bf16, tag="Rt")
                        for c0 in range(0, nsl, 512):
                            cw = min(512, nsl - c0)
                            pbc = psA.tile([128, 512], f32, tag="pbig")
                            nc.tensor.matmul(out=pbc[:, :cw], lhsT=ones1[:],
                                             rhs=dcw[0:1, c0:c0 + cw], start=True, stop=True)
                            nc.vector.tensor_scalar(out=Rt[:, c0:c0 + cw],
                                                    in0=pbc[:, :cw],
                                                    scalar1=W('iota128', 128, 1),
                                                    scalar2=None, op0=OP.is_equal)
                            nc.vector.tensor_copy(out=Rt[96:104, c0:c0 + cw],
                                                  in_=eavs[0:8, c0:c0 + cw])
                        # per-slot dst col within window: transpose dcw blocks
                        # to partitions (invalid slots carry 147 -> no match)
                        dcsec = sp.tile([128, KMAX], f32, tag="dcs")
                        for j in range(Ks):
                            pt1 = psA.tile([128, 1], bf16, tag="pbig")
                            nc.tensor.transpose(out=pt1[:],
                                                in_=dcw[0:1, j * 128:(j + 1) * 128],
                                                identity=ident[:1, :1])
                            nc.vector.tensor_scalar(out=dcsec[:, j:j + 1], in0=pt1[:],
                                                    scalar1=0.0, scalar2=None,
                                                    op0=OP.add)
                        lgp = psp.tile([128, 16], f32, tag="lgp", bufs=1)
                        for j0 in range(0, Ks, 4):
                            jw = min(4, Ks - j0)
                            for b in range(2):
                                mps = psp.tile([96, 512], f32, tag="mps")
                                nc.tensor.matmul(out=mps[:, :jw * 128], lhsT=basel[b][:],
                                                 rhs=Rt[:, j0 * 128:(j0 + jw) * 128],
                                                 start=True, stop=False)
                                for dj in range(jw):
                                    j = j0 + dj
                                    nc.tensor.matmul(out=mps[:, dj * 128:(dj + 1) * 128],
                                                     lhsT=g[:, j, b * 128:b * 128 + 96],
                                                     rhs=ident[:], start=False,
                                                     stop=(dj == jw - 1),
                                                     skip_group_check=True)
                                am = sp.tile([96, 512], bf16, tag="am")
                                nc.scalar.activation(out=am[:, :jw * 128],
                                                     in_=mps[:, :jw * 128], func=AF.Abs)
                                for dj in range(jw):
                                    j = j0 + dj
                                    nc.tensor.matmul(out=lgp[:, 2 * j + b:2 * j + b + 1],
                                                     lhsT=am[:, dj * 128:(dj + 1) * 128],
                                                     rhs=att04[(l, b)][:],
                                                     start=(j == 0 and b == 0), stop=False,
                                                     skip_group_check=True)
                        # += 0.6*ea.(We@att) per branch (cols 2j|2j+1), on device
                        for j in range(Ks):
                            nc.tensor.matmul(out=lgp[:, 2 * j:2 * j + 2],
                                             lhsT=eavs[0:8, j * 128:(j + 1) * 128],
                                             rhs=Vt[l][:], start=False,
                                             stop=(j == Ks - 1), skip_group_check=True)
                        exw = sp.tile([128, 16], f32, tag="exw")
                        nc.scalar.activation(out=exw[:, :2 * Ks], in_=lgp[:, :2 * Ks],
                                             func=AF.Exp)
                        nc.vector.tensor_tensor(
                            out=exw[:, :2 * Ks].rearrange("p (j b) -> p j b", b=2),
                            in0=exw[:, :2 * Ks].rearrange("p (j b) -> p j b", b=2),
                            in1=g[:, :Ks, 97:99], op=OP.mult)
                        for j in range(Ks):
                            for b in range(2):
                                es = sp.tile([128, WIN], bf16, tag="es")
                                nc.vector.tensor_scalar(
                                    out=es[:], in0=iota_t, scalar1=dcsec[:, j:j + 1],
                                    scalar2=exw[:, 2 * j + b:2 * j + b + 1],
                                    op0=OP.is_equal, op1=OP.mult)
                                nagg[b] += 1
                                nc.tensor.matmul(out=aggp[b][:, :],
                                                 lhsT=g[:, j, b * 128:b * 128 + 97],
                                                 rhs=es[:],
                                                 start=first[b], stop=(nagg[b] == tot[b]),
                                                 skip_group_check=True)
                                first[b] = False
                # finalize window -> h_T
                for b in range(2):
                    num = sp.tile([96, WIN], f32, tag="num")
                    den = sp.tile([1, WIN], f32, tag="den")
                    nc.vector.tensor_copy(out=num[:], in_=aggp[b][:96, :])
                    nc.vector.tensor_scalar(out=den[:], in0=aggp[b][96:97, :],
                                            scalar1=1e-30, scalar2=None, op0=OP.add)
                    rec = sp.tile([1, WIN], f32, tag="rec")
                    nc.vector.reciprocal(out=rec[:], in_=den[:])
                    pb = psp.tile([96, WIN], f32, tag="mps")
                    nc.tensor.matmul(out=pb[:], lhsT=one1[:], rhs=rec[:], start=True, stop=True)
                    tdiv = sp.tile([96, WIN], f32, tag="tdiv")
                    nc.vector.tensor_tensor(out=tdiv[:], in0=num[:], in1=pb[:], op=OP.mult)
                    lin = sp.tile([96, WIN], f32, tag="lin")
                    nc.scalar.activation(out=lin[:], in_=tdiv[:], func=AF.Identity,
                                         bias=W(f'bb_{l}_{b}', 96, 1))
                    ab = sp.tile([96, WIN], f32, tag="ab")
                    nc.scalar.activation(out=ab[:], in_=tdiv[:], func=AF.Abs,
                                         bias=W(f'bb_{l}_{b}', 96, 1))
                    nc.vector.tensor_scalar(out=lin[:], in0=lin[:], scalar1=0.505,
                                            scalar2=None, op0=OP.mult)
                    nc.vector.tensor_scalar(out=ab[:], in0=ab[:], scalar1=0.495,
                                            scalar2=None, op0=OP.mult)
                    nc.vector.tensor_tensor(out=h_T[b][:, w * WIN:(w + 1) * WIN],
                                            in0=lin[:], in1=ab[:], op=OP.add)

        # ---------- head ----------
        hid_T = [wp.tile([128, NCH * 128], f32, tag=f"hw{p}", name=f"hid{p}") for p in range(2)]
        for cs in range(0, NCH * 128, 512):
            ce = min(cs + 512, NCH * 128)
            w_ = ce - cs
            pf = psA.tile([96, 512], f32, tag="pbig")
            nc.tensor.matmul(out=pf[:, :w_], lhsT=W('fusion_Wt'),
                             rhs=h_T[0][:, cs:ce], start=True, stop=False)
            nc.tensor.matmul(out=pf[:, :w_], lhsT=W('fusion_Wb'),
                             rhs=h_T[1][:, cs:ce], start=False, stop=True)
            fus = sp.tile([96, 512], bf16, tag="fus")
            lin = sp.tile([96, 512], f32, tag="flin")
            nc.scalar.activation(out=lin[:, :w_], in_=pf[:, :w_], func=AF.Identity,
                                 bias=W('fusion_b', 96, 1))
            ab = sp.tile([96, 512], f32, tag="fab")
            nc.scalar.activation(out=ab[:, :w_], in_=pf[:, :w_], func=AF.Abs,
                                 bias=W('fusion_b', 96, 1))
            nc.vector.tensor_scalar(out=lin[:, :w_], in0=lin[:, :w_], scalar1=0.505,
                                    scalar2=None, op0=OP.mult)
            nc.vector.tensor_scalar(out=ab[:, :w_], in0=ab[:, :w_], scalar1=0.495,
                                    scalar2=None, op0=OP.mult)
            nc.vector.tensor_tensor(out=fus[:, :w_], in0=lin[:, :w_], in1=ab[:, :w_],
                                    op=OP.add)
            for p, (wk, bk) in enumerate([('pred_W1a', 'pred_b1a'), ('pred_W1b', 'pred_b1b')]):
                ph = psA.tile([128, 512], f32, tag="pbig")
                nc.tensor.matmul(out=ph[:, :w_], lhsT=W(wk, 96, 128), rhs=fus[:, :w_],
                                 start=True, stop=True)
                l2 = sp.tile([128, 512], f32, tag=f"l2{p}")
                a2 = sp.tile([128, 512], f32, tag=f"a2{p}")
                nc.scalar.activation(out=l2[:, :w_], in_=ph[:, :w_], func=AF.Identity,
                                     bias=W(bk, 128, 1))
                nc.scalar.activation(out=a2[:, :w_], in_=ph[:, :w_], func=AF.Abs,
                                     bias=W(bk, 128, 1))
                nc.vector.tensor_scalar(out=l2[:, :w_], in0=l2[:, :w_], scalar1=0.505,
                                        scalar2=None, op0=OP.mult)
                nc.vector.tensor_scalar(out=a2[:, :w_], in0=a2[:, :w_], scalar1=0.495,
                                        scalar2=None, op0=OP.mult)
                nc.vector.tensor_tensor(out=hid_T[p][:, cs:ce], in0=l2[:, :w_],
                                        in1=a2[:, :w_], op=OP.add)
        for ch in range(NCH):
            n0 = ch * 128
            nreal = max(0, min(NLOC - n0, 128))
            if nreal == 0:
                continue
            po = psp.tile([128, 2], f32, tag="mps")
            nc.tensor.matmul(out=po[:], lhsT=hid_T[0][:, n0:n0 + 128],
                             rhs=W('pred_W2a', 128, 2), start=True, stop=False)
            nc.tensor.matmul(out=po[:], lhsT=hid_T[1][:, n0:n0 + 128],
                             rhs=W('pred_W2b', 128, 2), start=False, stop=True)
            ot = sp.tile([128, 2], bf16, tag="ot")
            nc.vector.tensor_tensor(out=ot[:], in0=po[:], in1=W('pred_b2', 128, 2), op=OP.add)
            nc.sync.dma_start(out=dout[n0:n0 + nreal, :], in_=ot[:nreal, :])

    nc.compile()
    return nc


def _make_runner(nc):
    """Build (once) a cached jitted shard_map wrapper around the compiled
    Bass module — same lowering as bass2jax.run_bass_via_pjrt, but the jit
    object is reused across calls so warm calls skip retrace/recompile."""
    import jax
    import jax.core as jcore
    from jax.experimental.shard_map import shard_map
    from jax.sharding import Mesh, PartitionSpec
    from concourse import bass2jax, mybir
    bass2jax.install_neuronx_cc_hook()

    partition_name = nc.partition_id_tensor.name if nc.partition_id_tensor else None
    in_names, out_names, out_avals, zero_shapes = [], [], [], []
    for alloc in nc.m.functions[0].allocations:
        if not isinstance(alloc, mybir.MemoryLocationSet):
            continue
        name = alloc.memorylocations[0].name
        if alloc.kind == "ExternalInput":
            if name != partition_name:
                in_names.append(name)
        elif alloc.kind == "ExternalOutput":
            shape = tuple(alloc.tensor_shape)
            dtype = mybir.dt.np(alloc.dtype)
            out_names.append(name)
            out_avals.append(jcore.ShapedArray(shape, dtype))
            zero_shapes.append((shape, dtype))
    n_params = len(in_names)
    n_outs = len(out_avals)
    all_in = list(in_names) + list(out_names)
    if partition_name is not None:
        all_in.append(partition_name)
    donate = tuple(range(n_params, n_params + n_outs))

    dbg_name = None
    if nc.dbg_addr is not None:
        assert not nc.dbg_callbacks
        dbg_name = nc.dbg_addr.name

    def _body(*args):
        operands = list(args)
        if partition_name is not None:
            operands.append(bass2jax.partition_id_tensor())
        outs = bass2jax._bass_exec_p.bind(
            *operands, out_avals=tuple(out_avals), in_names=tuple(all_in),
            out_names=tuple(out_names), lowering_input_output_aliases=(),
            sim_require_finite=True, sim_require_nnan=True, nc=nc)
        return tuple(outs)

    devices = jax.devices()[:NCORES]
    mesh = Mesh(np.asarray(devices), ("core",))
    in_specs = (PartitionSpec("core"),) * (n_params + n_outs)
    out_specs = (PartitionSpec("core"),) * n_outs
    fn = jax.jit(shard_map(_body, mesh=mesh, in_specs=in_specs,
                           out_specs=out_specs, check_rep=False),
                 donate_argnums=donate, keep_unused=True)
    return dict(fn=fn, in_names=in_names, out_names=out_names,
                out_avals=out_avals, zero_shapes=zero_shapes, dbg_name=dbg_name)


def _execute(runner, in_maps):
    n = len(in_maps)
    if runner['dbg_name'] is not None:
        z = np.zeros((1, 2), np.uint32)
        in_maps = [{**m, runner['dbg_name']: z} for m in in_maps]
    concat_in = [np.concatenate([np.asarray(in_maps[c][nm]) for c in range(n)], axis=0)
                 for nm in runner['in_names']]
    zeros = [np.zeros((n * s[0], *s[1:]), dt) for (s, dt) in runner['zero_shapes']]
    outs = runner['fn'](*concat_in, *zeros)
    return [{nm: np.asarray(outs[i]).reshape(n, *runner['out_avals'][i].shape)[c]
             for i, nm in enumerate(runner['out_names'])} for c in range(n)]


def _in_maps(x, pp, blob):
    bh, bf, b128 = blob
    xb = np.ascontiguousarray(
        np.clip(np.rint(x * XSCALE), -127, 127).astype(np.int8))
    maps = []
    for c in range(NCORES):
        maps.append({'x': xb[c * NLOC:(c + 1) * NLOC],
                     'ea8': pp['ea8'][c],
                     'dcw': pp['dcw'][c],
                     'gw': pp['gw'][c],
                     'wbh': np.ascontiguousarray(bh[c * 12:(c + 1) * 12]),
                     'wbf': bf,
                     'wb128': np.ascontiguousarray(b128[c * 16:(c + 1) * 16])})
    return maps


def kernel(**inputs):
    x = np.asarray(inputs['x'], np.float32)
    ei = np.asarray(inputs['edge_index'])
    ea = np.asarray(inputs['edge_attr'], np.float32)
    pp = _host_prep(x, ei, ea)
    if _CACHE.get('NSLOT') != pp['NSLOT']:
        _CACHE['nc'] = build_kernel(pp)
        _CACHE['runner'] = _make_runner(_CACHE['nc'])
        _CACHE['NSLOT'] = pp['NSLOT']
    blob = _wblob(inputs)
    res = _execute(_CACHE['runner'], _in_maps(x, pp, blob))
    out = np.concatenate([res[c]['out'] for c in range(NCORES)], axis=0)
    return out.astype(np.float32)


# revision 69
# speedup vs baseline: 28.9675x; 1.0167x over previous
"""BiLevelGAT (2-branch x 3-layer GATv2, N=50000, E=500000, D=96) on 8 TRN2 cores.

Sharding: nodes + incoming edges partitioned by dst; per-layer AllGather of a
bf16 per-node table [hl_loc 96|1|w_loc|w_glob|pad29|hl_glob 96|1|w_glob|pad30]
(512B rows) gathered per edge by src.

Math: lrelu(x) = 0.6x+0.4|x| splits the GATv2 logit into linear terms (per-src
w=exp(0.6*att.hl) folded into the softmax weight; per-dst term cancels in
softmax; per-edge ea term = on-device matmul of the edge table against
V=0.6*We@att) plus 0.4*att.|m| computed on device. Softmax max-subtraction
skipped (logits O(1), fp32 safe).

End-to-end wall time is dominated by host->device transfer over the axon
tunnel (~45MB/s), so inputs are aggressively compacted (~2MB/core):
 - x as int8*32 (1/32 folded into the layer-0 Wl/Wr), edge_attr as int8*32
   (1/32 folded into We and V); both add ~5e-3 rel err vs the 2e-2 gate.
 - per-edge one-hot "R" (dst selector + ea rows used as matmul rhs to form
   m = hr[dst] + ea@We) built on device per section: rank-1 broadcast matmul
   of the dst-col row, then is_equal against a partition iota covering ALL
   128 partitions (a partially-initialized Rt causes 0*NaN-garbage = NaN on
   a per-process-random basis -- caught by CoreSim's uninit checker).
 - per-slot dst cols for the scatter one-hot derived on device by per-block
   PE transposes of the dst-col row (invalid slots carry 147 -> miss both
   one-hots).
 - weights packed into 3 blobs (bf16 matrices / f32 96-row biases / f32
   128-row tail + iotas); gather indices shipped once [16, NSLOT/16] and
   replicated to 128 partitions on device.
The jitted shard_map wrapper is cached across calls (fresh jit per call
would retrace + recompile the XLA wrapper each time).
"""
import os
import sys
sys.path.insert(0, '/opt/trn_rl_repo')
os.environ.setdefault('NEURON_RT_RESET_CORES', '1')
import numpy as np
import ml_dtypes

BF16 = ml_dtypes.bfloat16

N, E, D, EDIM, L, DENSE, OUT = 50000, 500000, 96, 8, 3, 256, 2
NCORES = 8
NLOC = N // NCORES            # 6250
WIN, HALF = 96, 48
NWIN = (NLOC + WIN - 1) // WIN  # 66
NPAD = NWIN * WIN
NCH = (NPAD + 127) // 128     # chunks of 128 (PASS A / table)
SPLIT = 32768
TROW = 256
KMAX = 8

_CACHE = {}

# ---- packed weight blobs ----
# wbh [96, WBH] bf16: big matrices (Wl/Wr/att/We/fusion/pred_W1, V).
#   We_{l,b} stacked six-up in two 96-col groups at row bases 0/32/64
#   (any access must start at a partition base that is a multiple of 32);
#   V_l at row base 32*l (matmul rhs).
# wbf [96, WBF] f32: 96-row biases.
# wb128 [128, WB128] f32: 128-row biases/mats + iota constants.
WOFF = {}   # name -> (blob, row0, col0)
_cols = {'h': 0, 'f': 0, '128': 0}
def _al(blob, name, width, row=0):
    WOFF[name] = (blob, row, _cols[blob])
    _cols[blob] += width
for _l in range(L):
    for _b in range(2):
        _al('h', f'Wl_{_l}_{_b}', 96)
        _al('h', f'Wr_{_l}_{_b}', 96)
        _al('h', f'att_{_l}_{_b}', 1)
        _al('f', f'bb_{_l}_{_b}', 1)
for _i in range(6):
    _l, _b = divmod(_i, 2)
    WOFF[f'We_{_l}_{_b}'] = ('h', 32 * (_i % 3), _cols['h'] + 96 * (_i // 3))
_cols['h'] += 192
for _l in range(L):
    WOFF[f'V_{_l}'] = ('h', 32 * _l, _cols['h'])              # shared 2 cols
_cols['h'] += 2
_al('h', 'fusion_Wt', 96)
_al('h', 'fusion_Wb', 96)
_al('f', 'fusion_b', 1)
_al('h', 'pred_W1a', 128)
_al('h', 'pred_W1b', 128)
_al('128', 'pred_b1a', 1)
_al('128', 'pred_b1b', 1)
_al('128', 'pred_W2a', 2)
_al('128', 'pred_W2b', 2)
_al('128', 'pred_b2', 2)
_al('128', 'iotaf', 96)               # all 128 rows = arange(96)
_al('128', 'iota128', 1)              # rows = arange(128)
WBH, WBF, WB128 = _cols['h'], _cols['f'], _cols['128']
EASCALE = 32.0
XSCALE = 32.0


def _host_prep(x, edge_index, edge_attr):
    src = edge_index[0].astype(np.int64)
    dst = edge_index[1].astype(np.int64)
    mean_ea = edge_attr.mean(0).astype(np.float32)
    loop = np.arange(N, dtype=np.int64)
    src_a = np.concatenate([src, loop])
    dst_a = np.concatenate([dst, loop])
    ea_a = np.concatenate([edge_attr.astype(np.float32),
                           np.broadcast_to(mean_ea, (N, EDIM))], 0)

    owner = dst_a // NLOC
    dloc = dst_a - owner * NLOC
    win = dloc // WIN
    stream = (src_a >= SPLIT).astype(np.int64)

    per_core = []
    secs = np.zeros((NCORES, NWIN, 2), np.int64)
    for c in range(NCORES):
        m = owner == c
        s_c, d_c, e_c = src_a[m], dloc[m], ea_a[m]
        w_c, st_c = win[m], stream[m]
        sec = (w_c * 2 + st_c)
        order = np.argsort(sec * NLOC + d_c, kind='stable')
        s_c, d_c, e_c, sec = s_c[order], d_c[order], e_c[order], sec[order]
        st_c = st_c[order]
        per_core.append((s_c, d_c, e_c, sec, st_c))
        secs[c] = np.bincount(sec, minlength=NWIN * 2).reshape(NWIN, 2)

    K = np.maximum((secs.max(0) + 127) // 128, 1)       # [NWIN, 2]
    assert K.max() <= KMAX
    Kf = K.reshape(-1)
    sec_slot = np.zeros(NWIN * 2 + 1, np.int64)
    np.cumsum(Kf * 128, out=sec_slot[1:])
    NSLOT = int(sec_slot[-1])

    gidx = np.zeros((NCORES, NSLOT), np.int16)
    eav = np.zeros((NCORES, 9, NSLOT), np.float32)
    # invalid slots: dstcol=147 misses both the 128-wide Rt one-hot and the
    # 96-wide es window -> no contribution
    eav[:, 8, :] = 147.0

    for c in range(NCORES):
        s_c, d_c, e_c, sec, st_c = per_core[c]
        counts = np.bincount(sec, minlength=NWIN * 2)
        starts = np.concatenate([[0], np.cumsum(counts)])[:-1]
        pos = np.arange(len(s_c)) - starts[sec]
        slot = sec_slot[sec] + pos
        gidx[c, slot] = (s_c - st_c * SPLIT).astype(np.int16)
        eav[c, 0:8, slot] = e_c  # advanced idx puts slot axis first: (nedge, 8)
        eav[c, 8, slot] = (d_c % WIN).astype(np.float32)

    gw = np.ascontiguousarray(
        gidx.reshape(NCORES, -1, 16).transpose(0, 2, 1))     # [NCORES, 16, NSLOT//16]

    ea_q = np.clip(np.rint(eav[:, 0:8, :] * EASCALE), -127, 127).astype(np.int8)
    return dict(K=K, Kf=Kf, sec_slot=sec_slot, NSLOT=NSLOT, NSEC=NWIN * 2,
                gw=gw, ea8=ea_q,
                dcw=eav[:, 8, :].astype(np.uint8).reshape(NCORES, 1, NSLOT))


def _wblob(w):
    bh = np.zeros((96, WBH), BF16)
    bf = np.zeros((96, WBF), np.float32)
    b128 = np.zeros((128, WB128), np.float32)
    blobs = {'h': bh, 'f': bf, '128': b128}
    def put(name, arr):
        a = np.asarray(arr, np.float32)
        if a.ndim == 1:
            a = a.reshape(-1, 1)
        bl, r0, c0 = WOFF[name]
        dst = blobs[bl]
        dst[r0:r0 + a.shape[0], c0:c0 + a.shape[1]] = a.astype(dst.dtype)
    for l in range(L):
        V = np.zeros((8, 2), np.float32)
        # x ships as int8 * XSCALE; fold 1/XSCALE into the layer-0 weights
        xs = XSCALE if l == 0 else 1.0
        for b, p in enumerate(['local', 'global']):
            put(f'Wl_{l}_{b}', np.asarray(w[f'{p}_Wl'][l], np.float32) / xs)
            put(f'Wr_{l}_{b}', np.asarray(w[f'{p}_Wr'][l], np.float32) / xs)
            put(f'att_{l}_{b}', w[f'{p}_att'][l])
            put(f'bb_{l}_{b}', w[f'{p}_b'][l])
            # ea ships as int8 * EASCALE; fold 1/EASCALE into We and V
            put(f'We_{l}_{b}', np.asarray(w[f'{p}_We'][l], np.float32) / EASCALE)
            V[:, b] = (0.6 / EASCALE) * (np.asarray(w[f'{p}_We'][l], np.float32)
                                         @ np.asarray(w[f'{p}_att'][l], np.float32))
        put(f'V_{l}', V)
    put('fusion_Wt', w['fusion_W'][:96])
    put('fusion_Wb', w['fusion_W'][96:])
    put('fusion_b', w['fusion_b'])
    put('pred_W1a', w['pred_W1'][:, :128])
    put('pred_W1b', w['pred_W1'][:, 128:])
    put('pred_b1a', w['pred_b1'][:128])
    put('pred_b1b', w['pred_b1'][128:])
    put('pred_W2a', w['pred_W2'][:128])
    put('pred_W2b', w['pred_W2'][128:])
    put('pred_b2', np.broadcast_to(np.asarray(w['pred_b2']).reshape(1, 2), (128, 2)))
    put('iotaf', np.broadcast_to(np.arange(96, dtype=np.float32), (128, 96)))
    put('iota128', np.arange(128, dtype=np.float32))
    return bh, bf, b128


def build_kernel(pp):
    import os as _os
    SKIP_EDGE = _os.environ.get('SKIP_EDGE', '0') == '1'
    SKIP_GATHER = _os.environ.get('SKIP_GATHER', '0') == '1'
    from concourse import mybir, bacc
    import concourse.tile as tile
    Kf, sec_slot, NSLOT = pp['Kf'], pp['sec_slot'], pp['NSLOT']
    f32, bf16, i16 = mybir.dt.float32, mybir.dt.bfloat16, mybir.dt.int16
    AF = mybir.ActivationFunctionType
    OP = mybir.AluOpType

    i8, u8 = mybir.dt.int8, mybir.dt.uint8
    nc = bacc.Bacc("TRN2", target_bir_lowering=False, debug=False, num_devices=NCORES)
    dx = nc.dram_tensor("x", [NLOC, D], i8, kind="ExternalInput")
    dea8 = nc.dram_tensor("ea8", [8, NSLOT], i8, kind="ExternalInput")
    ddcw = nc.dram_tensor("dcw", [1, NSLOT], u8, kind="ExternalInput")
    dgw = nc.dram_tensor("gw", [16, NSLOT // 16], i16, kind="ExternalInput")
    # weight blobs ship 1/8 per core (identical content host-side) and are
    # reassembled on device with an AllGather -- saves 7/8 of the blob bytes
    # on the transfer-bound axon tunnel
    dwbh = nc.dram_tensor("wbh", [96 // NCORES, WBH], bf16, kind="ExternalInput")
    wbh_loc = nc.dram_tensor("wbh_loc", [96 // NCORES, WBH], bf16)
    wbh_sh = nc.dram_tensor("wbh_sh", [96, WBH], bf16, addr_space="Shared")
    dwbf = nc.dram_tensor("wbf", [96, WBF], f32, kind="ExternalInput")
    dwb128 = nc.dram_tensor("wb128", [128 // NCORES, WB128], f32, kind="ExternalInput")
    wb128_loc = nc.dram_tensor("wb128_loc", [128 // NCORES, WB128], f32)
    wb128_sh = nc.dram_tensor("wb128_sh", [128, WB128], f32, addr_space="Shared")
    dout = nc.dram_tensor("out", [NLOC, OUT], bf16, kind="ExternalOutput")

    tab_slice = nc.dram_tensor("tab_slice", [NLOC, TROW], bf16)
    tab_sh = nc.dram_tensor("tab_sh", [N, TROW], bf16, addr_space="Shared")
    tab = nc.dram_tensor("tab", [N, TROW], bf16)

    def wo(name, rows=96, width=None):
        w_ = width if width is not None else 96
        return (WOFF[name], WOFF[name] + w_, rows)

    with tile.TileContext(nc) as tc:
      with (tc.tile_pool(name="const", bufs=1) as cp,
            tc.tile_pool(name="hp", bufs=1) as hp,
            tc.tile_pool(name="wp", bufs=1) as wp,
            tc.tile_pool(name="sp", bufs=3) as sp,
            tc.tile_pool(name="gpool", bufs=2) as gpl,
            tc.tile_pool(name="ps", bufs=2, space="PSUM") as psp,
            tc.tile_pool(name="psA", bufs=2, space="PSUM") as psA,
            tc.tile_pool(name="psagg", bufs=1, space="PSUM") as psG):

        ident = cp.tile([128, 128], bf16)
        nc.sync.dma_start(out=ident[:], in_=nc.inline_tensor(np.eye(128, dtype=BF16), name="idb").ap())
        identf = cp.tile([128, 128], f32)
        nc.sync.dma_start(out=identf[:], in_=nc.inline_tensor(np.eye(128, dtype=np.float32), name="idf").ap())
        nc.sync.dma_start(out=wbh_loc[:], in_=dwbh[:])
        nc.sync.dma_start(out=wb128_loc[:], in_=dwb128[:])
        nc.gpsimd.collective_compute(
            "AllGather", mybir.AluOpType.bypass,
            replica_groups=[list(range(NCORES))],
            ins=[wbh_loc[:]], outs=[wbh_sh[:]])
        nc.gpsimd.collective_compute(
            "AllGather", mybir.AluOpType.bypass,
            replica_groups=[list(range(NCORES))],
            ins=[wb128_loc[:]], outs=[wb128_sh[:]])
        wbh_t = cp.tile([96, WBH], bf16)
        nc.sync.dma_start(out=wbh_t[:], in_=wbh_sh[:])
        wbf_t = cp.tile([96, WBF], f32)
        nc.sync.dma_start(out=wbf_t[:], in_=dwbf[:])
        wb128_t = cp.tile([128, WB128], f32)
        nc.sync.dma_start(out=wb128_t[:], in_=wb128_sh[:])
        gw_t = cp.tile([128, NSLOT // 16], i16)
        for k in range(8):
            nc.sync.dma_start(out=gw_t[16 * k:16 * (k + 1), :], in_=dgw[:])

        blobs_t = {'h': wbh_t, 'f': wbf_t, '128': wb128_t}
        def W(name, rows=96, width=96):
            bl, r0, c0 = WOFF[name]
            return blobs_t[bl][r0:r0 + rows, c0:c0 + width]

        iota_t = wb128_t[:, WOFF['iotaf'][2]:WOFF['iotaf'][2] + 96]

        one1 = cp.tile([1, 96], f32)
        nc.vector.memset(one1[:], 1.0)
        ones1 = cp.tile([1, 128], bf16)
        nc.vector.memset(ones1[:], 1.0)
        att04 = {}
        for l in range(L):
            for b in range(2):
                att04[(l, b)] = cp.tile([96, 1], bf16, tag=f"att04_{l}_{b}", name=f"att04_{l}_{b}")
                nc.vector.tensor_scalar(out=att04[(l, b)][:], in0=W(f'att_{l}_{b}', 96, 1),
                                        scalar1=0.4, scalar2=None, op0=OP.mult)
        Vt = {}
        for l in range(L):
            Vt[l] = cp.tile([8, 2], bf16, tag=f"V_{l}", name=f"V_{l}")
            nc.vector.tensor_copy(out=Vt[l][:], in_=W(f'V_{l}', 8, 2))

        # h_T feature-major [96, NPAD] (cols beyond NLOC are pad)
        h_T = [hp.tile([96, NCH * 128], bf16, tag=f"h{b}", name=f"h{b}") for b in range(2)]
        for ch in range(NCH):
            n0 = ch * 128
            nreal = max(0, min(NLOC - n0, 128))
            xin8 = sp.tile([128, 128], i8, tag="xin8")
            nc.vector.memset(xin8[:], 0)
            if nreal > 0:
                nc.sync.dma_start(out=xin8[:nreal, :96], in_=dx[n0:n0 + nreal, :])
            xin = sp.tile([128, 128], bf16, tag="xin")
            nc.vector.tensor_copy(out=xin[:], in_=xin8[:])
            pt = psA.tile([128, 128], bf16, tag="pbig")
            nc.tensor.transpose(out=pt[:], in_=xin[:], identity=ident[:])
            for b in range(2):
                nc.vector.tensor_copy(out=h_T[b][:, n0:n0 + 128], in_=pt[:96, :])

        hw_T = [wp.tile([96, NCH * 128], bf16, tag=f"hw{b}", name=f"hw{b}") for b in range(2)]

        for l in range(L):
            # ---------- PASS A ----------
            for b in range(2):
                for cs in range(0, NCH * 128, 512):
                    ce = min(cs + 512, NCH * 128)
                    w_ = ce - cs
                    pl = psA.tile([96, 512], f32, tag="pbig")
                    nc.tensor.matmul(out=pl[:, :w_], lhsT=W(f'Wl_{l}_{b}'),
                                     rhs=h_T[b][:, cs:ce], start=True, stop=True)
                    nc.vector.tensor_copy(out=hw_T[b][:, cs:ce], in_=pl[:, :w_])
            # table slice + allgather
            for ch in range(NCH):
                n0 = ch * 128
                nreal = max(0, min(NLOC - n0, 128))
                if nreal == 0:
                    continue
                stg = sp.tile([128, TROW], bf16, tag="stg")
                nc.vector.memset(stg[:], 0.0)
                for b in range(2):
                    pt = psA.tile([128, 128], bf16, tag="pbig")
                    nc.tensor.transpose(out=pt[:, :96], in_=hw_T[b][:, n0:n0 + 128],
                                        identity=ident[:96, :96])
                    nc.vector.tensor_copy(out=stg[:, b * 128:b * 128 + 96], in_=pt[:, :96])
                    # w = exp(0.6*att.hl) for this chunk; ones at ext row 32
                    pphi = psA.tile([1, 128], f32, tag="pbig")
                    nc.tensor.matmul(out=pphi[:], lhsT=W(f'att_{l}_{b}', 96, 1),
                                     rhs=hw_T[b][:, n0:n0 + 128], start=True, stop=True)
                    ext = sp.tile([64, 128], f32, tag="ext")
                    nc.scalar.activation(out=ext[0:1, :], in_=pphi[:], func=AF.Exp, scale=0.6)
                    nc.vector.memset(ext[32:33, :], 1.0)
                    pt2 = psA.tile([128, 64], f32, tag="pbig")
                    nc.tensor.transpose(out=pt2[:], in_=ext[:], identity=identf[:64, :64])
                    nc.vector.tensor_copy(out=stg[:, b * 128 + 96:b * 128 + 97], in_=pt2[:, 32:33])
                    nc.vector.tensor_copy(out=stg[:, b * 128 + 97:b * 128 + 98], in_=pt2[:, 0:1])
                nc.vector.tensor_copy(out=stg[:, 98:99], in_=stg[:, 225:226])
                nc.sync.dma_start(out=tab_slice[n0:n0 + nreal, :], in_=stg[:nreal, :])
            nc.gpsimd.collective_compute(
                "AllGather", mybir.AluOpType.bypass,
                replica_groups=[list(range(NCORES))],
                ins=[tab_slice[:]], outs=[tab_sh[:]],
            )
            nc.sync.dma_start(out=tab[:], in_=tab_sh[:])

            # ---------- edge phase ----------
            for w in range(0 if not SKIP_EDGE else NWIN, NWIN):
                aggp = {}
                first = {b: True for b in range(2)}
                nagg = {b: 0 for b in range(2)}
                tot = {b: sum(int(Kf[w * 2 + s]) for s in range(2))
                       for b in range(2)}
                for b in range(2):
                    aggp[b] = psG.tile([97, WIN], f32, tag=f"agg{b}", name=f"agg{b}")
                # base lhsT per branch for this window (hr = h @ Wr computed here)
                basel = {}
                for b in range(2):
                    phr = psA.tile([96, WIN], f32, tag="pbig")
                    nc.tensor.matmul(out=phr[:], lhsT=W(f'Wr_{l}_{b}'),
                                     rhs=h_T[b][:, w * WIN:(w + 1) * WIN],
                                     start=True, stop=True)
                    hrs = sp.tile([96, WIN], f32, tag="hrs")
                    nc.vector.tensor_copy(out=hrs[:], in_=phr[:])
                    pt = psA.tile([WIN, 96], f32, tag="pbig")
                    nc.tensor.transpose(out=pt[:], in_=hrs[:], identity=identf[:96, :96])
                    bl = sp.tile([128, 96], bf16, tag=f"basel{b}", name=f"basel{b}")
                    nc.vector.memset(bl[:], 0.0)
                    nc.vector.tensor_copy(out=bl[:WIN, :], in_=pt[:])
                    nc.vector.tensor_copy(out=bl[WIN:WIN + 8, :], in_=W(f'We_{l}_{b}', 8, 96))
                    basel[b] = bl
                if True:
                    for s in range(2):
                        si = w * 2 + s
                        Ks = int(Kf[si])
                        sl0 = int(sec_slot[si])
                        nsl = Ks * 128
                        g = gpl.tile([128, KMAX, TROW], bf16, tag="gath")
                        if SKIP_GATHER:
                            nc.vector.memset(g[:, :Ks, :], 0.0)
                        else:
                            nc.gpsimd.dma_gather(
                                out_ap=g[:, :Ks, :],
                                in_ap=tab[SPLIT:, :] if s else tab[:SPLIT, :],
                                idxs_ap=gw_t[:, sl0 // 16:(sl0 + nsl) // 16],
                                num_idxs=nsl, num_idxs_reg=nsl, elem_size=TROW)
                        # compact edge table slice (i8/u8 on the wire -> bf16)
                        ea8s = sp.tile([8, KMAX * 128], i8, tag="ea8s")
                        nc.sync.dma_start(out=ea8s[:, :nsl], in_=dea8[:, sl0:sl0 + nsl])
                        eavs = sp.tile([8, KMAX * 128], bf16, tag="eavs")
                        nc.vector.tensor_copy(out=eavs[:, :nsl], in_=ea8s[:, :nsl])
                        dc8s = sp.tile([1, KMAX * 128], u8, tag="dc8s")
                        nc.sync.dma_start(out=dc8s[:, :nsl], in_=ddcw[:, sl0:sl0 + nsl])
                        dcw = sp.tile([1, KMAX * 128], bf16, tag="dcw")
                        nc.vector.tensor_copy(out=dcw[:, :nsl], in_=dc8s[:, :nsl])
                        # build Rt on device: rows 0-95 one-hot(dstcol), 96-103
                        # ea. is_equal covers ALL 128 rows (rows 96-127 can
                        # never match dstcol<=147... rows 96-103 overwritten by
                        # ea below, 104-127 exact zeros) so no partition of Rt
                        # is left uninitialized -- the mps matmul reads all 128
                        # partitions and 0 * NaN-garbage = NaN.
                        Rt = sp.tile([128, KMAX * 128], bf16, tag="Rt")
                        for c0 in range(0, nsl, 512):
                            cw = min(512, nsl - c0)
                            pbc = psA.tile([128, 512], f32, tag="pbig")
                            nc.tensor.matmul(out=pbc[:, :cw], lhsT=ones1[:],
                                             rhs=dcw[0:1, c0:c0 + cw], start=True, stop=True)
                            nc.vector.tensor_scalar(out=Rt[:, c0:c0 + cw],
                                                    in0=pbc[:, :cw],
                                                    scalar1=W('iota128', 128, 1),
                                                    scalar2=None, op0=OP.is_equal)
                            nc.vector.tensor_copy(out=Rt[96:104, c0:c0 + cw],
                                                  in_=eavs[0:8, c0:c0 + cw])
                        # per-slot dst col within window: transpose dcw blocks
                        # to partitions (invalid slots carry 147 -> no match)
                        dcsec = sp.tile([128, KMAX], f32, tag="dcs")
                        for j in range(Ks):
                            pt1 = psA.tile([128, 1], bf16, tag="pbig")
                            nc.tensor.transpose(out=pt1[:],
                                                in_=dcw[0:1, j * 128:(j + 1) * 128],
                                                identity=ident[:1, :1])
                            nc.vector.tensor_scalar(out=dcsec[:, j:j + 1], in0=pt1[:],
                                                    scalar1=0.0, scalar2=None,
                                                    op0=OP.add)
                        lgp = psp.tile([128, 16], f32, tag="lgp", bufs=1)
                        for j0 in range(0, Ks, 4):
                            jw = min(4, Ks - j0)
                            for b in range(2):
                                mps = psp.tile([96, 512], f32, tag="mps")
                                nc.tensor.matmul(out=mps[:, :jw * 128], lhsT=basel[b][:],
                                                 rhs=Rt[:, j0 * 128:(j0 + jw) * 128],
                                                 start=True, stop=False)
                                for dj in range(jw):
                                    j = j0 + dj
                                    nc.tensor.matmul(out=mps[:, dj * 128:(dj + 1) * 128],
                                                     lhsT=g[:, j, b * 128:b * 128 + 96],
                                                     rhs=ident[:], start=False,
                                                     stop=(dj == jw - 1),
                                                     skip_group_check=True)
                                am = sp.tile([96, 512], bf16, tag="am")
                                nc.scalar.activation(out=am[:, :jw * 128],
                                                     in_=mps[:, :jw * 128], func=AF.Abs)
                                for dj in range(jw):
                                    j = j0 + dj
                                    nc.tensor.matmul(out=lgp[:, 2 * j + b:2 * j + b + 1],
                                                     lhsT=am[:, dj * 128:(dj + 1) * 128],
                                                     rhs=att04[(l, b)][:],
                                                     start=(j == 0 and b == 0), stop=False,
                                                     skip_group_check=True)
                        # += 0.6*ea.(We@att) per branch (cols 2j|2j+1), on device
                        for j in range(Ks):
                            nc.tensor.matmul(out=lgp[:, 2 * j:2 * j + 2],
                                             lhsT=eavs[0:8, j * 128:(j + 1) * 128],
                                             rhs=Vt[l][:], start=False,
                                             stop=(j == Ks - 1), skip_group_check=True)
                        exw = sp.tile([128, 16], f32, tag="exw")
                        nc.scalar.activation(out=exw[:, :2 * Ks], in_=lgp[:, :2 * Ks],
                                             func=AF.Exp)
                        nc.vector.tensor_tensor(
                            out=exw[:, :2 * Ks].rearrange("p (j b) -> p j b", b=2),
                            in0=exw[:, :2 * Ks].rearrange("p (j b) -> p j b", b=2),
                            in1=g[:, :Ks, 97:99], op=OP.mult)
                        for j in range(Ks):
                            for b in range(2):
                                es = sp.tile([128, WIN], bf16, tag="es")
                                nc.vector.tensor_scalar(
                                    out=es[:], in0=iota_t, scalar1=dcsec[:, j:j + 1],
                                    scalar2=exw[:, 2 * j + b:2 * j + b + 1],
                                    op0=OP.is_equal, op1=OP.mult)
                                nagg[b] += 1
                                nc.tensor.matmul(out=aggp[b][:, :],
                                                 lhsT=g[:, j, b * 128:b * 128 + 97],
                                                 rhs=es[:],
                                                 start=first[b], stop=(nagg[b] == tot[b]),
                                                 skip_group_check=True)
                                first[b] = False
                # finalize window -> h_T
                for b in range(2):
                    num = sp.tile([96, WIN], f32, tag="num")
                    den = sp.tile([1, WIN], f32, tag="den")
                    nc.vector.tensor_copy(out=num[:], in_=aggp[b][:96, :])
                    nc.vector.tensor_scalar(out=den[:], in0=aggp[b][96:97, :],
                                            scalar1=1e-30, scalar2=None, op0=OP.add)
                    rec = sp.tile([1, WIN], f32, tag="rec")
                    nc.vector.reciprocal(out=rec[:], in_=den[:])
                    pb = psp.tile([96, WIN], f32, tag="mps")
                    nc.tensor.matmul(out=pb[:], lhsT=one1[:], rhs=rec[:], start=True, stop=True)
                    tdiv = sp.tile([96, WIN], f32, tag="tdiv")
                    nc.vector.tensor_tensor(out=tdiv[:], in0=num[:], in1=pb[:], op=OP.mult)
                    lin = sp.tile([96, WIN], f32, tag="lin")
                    nc.scalar.activation(out=lin[:], in_=tdiv[:], func=AF.Identity,
                                         bias=W(f'bb_{l}_{b}', 96, 1))
                    ab = sp.tile([96, WIN], f32, tag="ab")
                    nc.scalar.activation(out=ab[:], in_=tdiv[:], func=AF.Abs,
                                         bias=W(f'bb_{l}_{b}', 96, 1))
                    nc.vector.tensor_scalar(out=lin[:], in0=lin[:], scalar1=0.505,
                                            scalar2=None, op0=OP.mult)
                    nc.vector.tensor_scalar(out=ab[:], in0=ab[:], scalar1=0.495,
                                            scalar2=None, op0=OP.mult)
                    nc.vector.tensor_tensor(out=h_T[b][:, w * WIN:(w + 1) * WIN],
                                            in0=lin[:], in1=ab[:], op=OP.add)

        # ---------- head ----------
        hid_T = [wp.tile([128, NCH * 128], f32, tag=f"hw{p}", name=f"hid{p}") for p in range(2)]
        for cs in range(0, NCH * 128, 512):
            ce = min(cs + 512, NCH * 128)
            w_ = ce - cs
            pf = psA.tile([96, 512], f32, tag="pbig")
            nc.tensor.matmul(out=pf[:, :w_], lhsT=W('fusion_Wt'),
                             rhs=h_T[0][:, cs:ce], start=True, stop=False)
            nc.tensor.matmul(out=pf[:, :w_], lhsT=W('fusion_Wb'),
                             rhs=h_T[1][:, cs:ce], start=False, stop=True)
            fus = sp.tile([96, 512], bf16, tag="fus")
            lin = sp.tile([96, 512], f32, tag="flin")
            nc.scalar.activation(out=lin[:, :w_], in_=pf[:, :w_], func=AF.Identity,
                                 bias=W('fusion_b', 96, 1))
            ab = sp.tile([96, 512], f32, tag="fab")
            nc.scalar.activation(out=ab[:, :w_], in_=pf[:, :w_], func=AF.Abs,
                                 bias=W('fusion_b', 96, 1))
            nc.vector.tensor_scalar(out=lin[:, :w_], in0=lin[:, :w_], scalar1=0.505,
                                    scalar2=None, op0=OP.mult)
            nc.vector.tensor_scalar(out=ab[:, :w_], in0=ab[:, :w_], scalar1=0.495,
                                    scalar2=None, op0=OP.mult)
            nc.vector.tensor_tensor(out=fus[:, :w_], in0=lin[:, :w_], in1=ab[:, :w_],
                                    op=OP.add)
            for p, (wk, bk) in enumerate([('pred_W1a', 'pred_b1a'), ('pred_W1b', 'pred_b1b')]):
                ph = psA.tile([128, 512], f32, tag="pbig")
                nc.tensor.matmul(out=ph[:, :w_], lhsT=W(wk, 96, 128), rhs=fus[:, :w_],
                                 start=True, stop=True)
                l2 = sp.tile([128, 512], f32, tag=f"l2{p}")
                a2 = sp.tile([128, 512], f32, tag=f"a2{p}")
                nc.scalar.activation(out=l2[:, :w_], in_=ph[:, :w_], func=AF.Identity,
                                     bias=W(bk, 128, 1))
                nc.scalar.activation(out=a2[:, :w_], in_=ph[:, :w_], func=AF.Abs,
                                     bias=W(bk, 128, 1))
                nc.vector.tensor_scalar(out=l2[:, :w_], in0=l2[:, :w_], scalar1=0.505,
                                        scalar2=None, op0=OP.mult)
                nc.vector.tensor_scalar(out=a2[:, :w_], in0=a2[:, :w_], scalar1=0.495,
                                        scalar2=None, op0=OP.mult)
                nc.vector.tensor_tensor(out=hid_T[p][:, cs:ce], in0=l2[:, :w_],
                                        in1=a2[:, :w_], op=OP.add)
        for ch in range(NCH):
            n0 = ch * 128
            nreal = max(0, min(NLOC - n0, 128))
            if nreal == 0:
                continue
            po = psp.tile([128, 2], f32, tag="mps")
            nc.tensor.matmul(out=po[:], lhsT=hid_T[0][:, n0:n0 + 128],
                             rhs=W('pred_W2a', 128, 2), start=True, stop=False)
            nc.tensor.matmul(out=po[:], lhsT=hid_T[1][:, n0:n0 + 128],
                             rhs=W('pred_W2b', 128, 2), start=False, stop=True)
            ot = sp.tile([128, 2], bf16, tag="ot")
            nc.vector.tensor_tensor(out=ot[:], in0=po[:], in1=W('pred_b2', 128, 2), op=OP.add)
            nc.sync.dma_start(out=dout[n0:n0 + nreal, :], in_=ot[:nreal, :])

    nc.compile()
    return nc


def _make_runner(nc):
    """Build (once) a cached jitted shard_map wrapper around the compiled
    Bass module — same lowering as bass2jax.run_bass_via_pjrt, but the jit
    object is reused across calls so warm calls skip retrace/recompile."""
    import jax
    import jax.core as jcore
    from jax.experimental.shard_map import shard_map
    from jax.sharding import Mesh, PartitionSpec
    from concourse import bass2jax, mybir
    bass2jax.install_neuronx_cc_hook()

    partition_name = nc.partition_id_tensor.name if nc.partition_id_tensor else None
    in_names, out_names, out_avals, zero_shapes = [], [], [], []
    for alloc in nc.m.functions[0].allocations:
        if not isinstance(alloc, mybir.MemoryLocationSet):
            continue
        name = alloc.memorylocations[0].name
        if alloc.kind == "ExternalInput":
            if name != partition_name:
                in_names.append(name)
        elif alloc.kind == "ExternalOutput":
            shape = tuple(alloc.tensor_shape)
            dtype = mybir.dt.np(alloc.dtype)
            out_names.append(name)
            out_avals.append(jcore.ShapedArray(shape, dtype))
            zero_shapes.append((shape, dtype))
    n_params = len(in_names)
    n_outs = len(out_avals)
    all_in = list(in_names) + list(out_names)
    if partition_name is not None:
        all_in.append(partition_name)
    donate = tuple(range(n_params, n_params + n_outs))

    dbg_name = None
    if nc.dbg_addr is not None:
        assert not nc.dbg_callbacks
        dbg_name = nc.dbg_addr.name

    def _body(*args):
        operands = list(args)
        if partition_name is not None:
            operands.append(bass2jax.partition_id_tensor())
        outs = bass2jax._bass_exec_p.bind(
            *operands, out_avals=tuple(out_avals), in_names=tuple(all_in),
            out_names=tuple(out_names), lowering_input_output_aliases=(),
            sim_require_finite=True, sim_require_nnan=True, nc=nc)
        return tuple(outs)

    devices = jax.devices()[:NCORES]
    mesh = Mesh(np.asarray(devices), ("core",))
    in_specs = (PartitionSpec("core"),) * (n_params + n_outs)
    out_specs = (PartitionSpec("core"),) * n_outs
    fn = jax.jit(shard_map(_body, mesh=mesh, in_specs=in_specs,
                           out_specs=out_specs, check_rep=False),
                 donate_argnums=donate, keep_unused=True)
    return dict(fn=fn, in_names=in_names, out_names=out_names,
                out_avals=out_avals, zero_shapes=zero_shapes, dbg_name=dbg_name)


def _execute(runner, in_map, n=NCORES):
    """in_map: dict of GLOBAL (8-core-concatenated) arrays from _in_maps."""
    if runner['dbg_name'] is not None:
        in_map = {**in_map,
                  runner['dbg_name']: np.zeros((n, 2), np.uint32)}
    flat = [in_map[nm] for nm in runner['in_names']]
    zeros = [np.zeros((n * s[0], *s[1:]), dt) for (s, dt) in runner['zero_shapes']]
    outs = runner['fn'](*flat, *zeros)
    return [{nm: np.asarray(outs[i]).reshape(n, *runner['out_avals'][i].shape)[c]
             for i, nm in enumerate(runner['out_names'])} for c in range(n)]


def _in_maps(x, pp, blob):
    """Global (8-core-concatenated) input arrays, zero-copy where possible.

    Every per-core shard is a contiguous slice of one parent array, so the
    axis-0 concatenation the shard_map wrapper needs is just a reshape --
    prepared here (host prep) instead of memcpy'd inside the timed call.
    """
    bh, bf, b128 = blob
    xb = np.ascontiguousarray(
        np.clip(np.rint(x * XSCALE), -127, 127).astype(np.int8))
    return {'x': xb,                                   # [8*NLOC, 96]
            'ea8': pp['ea8'].reshape(NCORES * 8, -1),  # [64, NSLOT]
            'dcw': pp['dcw'].reshape(NCORES, -1),      # [8, NSLOT]
            'gw': pp['gw'].reshape(NCORES * 16, -1),   # [128, NSLOT//16]
            'wbh': bh,                                 # [96, WBH] = 8 x 12 rows
            'wbf': np.tile(bf, (NCORES, 1)),           # [768, WBF]
            'wb128': b128}                             # [128, WB128] = 8 x 16 rows


def kernel(**inputs):
    x = np.asarray(inputs['x'], np.float32)
    ei = np.asarray(inputs['edge_index'])
    ea = np.asarray(inputs['edge_attr'], np.float32)
    pp = _host_prep(x, ei, ea)
    if _CACHE.get('NSLOT') != pp['NSLOT']:
        _CACHE['nc'] = build_kernel(pp)
        _CACHE['runner'] = _make_runner(_CACHE['nc'])
        _CACHE['NSLOT'] = pp['NSLOT']
    blob = _wblob(inputs)
    res = _execute(_CACHE['runner'], _in_maps(x, pp, blob))
    out = np.concatenate([res[c]['out'] for c in range(NCORES)], axis=0)
    return out.astype(np.float32)
